# revision 1
# baseline (speedup 1.0000x reference)
"""GAT 2-layer kernel for TRN2, 8 NeuronCores (self-contained).

Strategy:
- dst-shard: core c owns nodes [c*12500, (c+1)*12500).
- ad pre-pass: own-shard x @ fold(W, a_dst) -> ad1c, so edge pipelines
  never wait on the full dense pass.
- Dense phases (x@W bf16, h1@W2 bf16) replicated on all cores; folded
  a_src gives per-node [h | as] rows in one matmul. 16-tile DMA batches
  (HWDGE calls are ~600ns each on one shared device), bank-sized PSUM
  groups, copy work split across DVE/ACT.
- Edge phase per core: 4 src-chunks (25000 nodes, int16 gather reach),
  per-chunk degree-bucketed padded CSR over dst. 1024-index single-queue
  SWDGE gathers (multi-queue under TileContext races; >16KB rings hang
  HW) pull [h | as] rows. e = exp(leaky(as+ad)) on ACT; segment-fused
  bf16 weighted-feature mult + f32 reduce into per-bucket tiles.
- Per-chunk partials [featsum | denom] -> DRAM staging (bf16); per-chunk
  table tensors + concurrently-open tile pools let edge chunks overlap
  the dense tail.
- Merge pass (natural node order): per-chunk 16-tile gathers, f32
  accumulate, per-head normalize, bias (+relu L1); L1 also computes
  ad2c (h1 @ fold(W2, a2_dst)) from the transposed tiles.
- L1->L2: h1T shard AllGather in two halves (overlaps merge tail and
  dense2 head) -> replicated dense2.
"""
import sys
sys.path.insert(0, "/opt/trn_rl_repo")
import numpy as np
import ml_dtypes

import concourse.bass as bass
import concourse.bacc as bacc
import concourse.tile as tile
from concourse import mybir
from concourse.library_config import mlp as mlp_lib


def make_runner(nc, n_cores):
    """PJRT runner: returns run_fn(in_maps, repeats) -> (results, best_time_s)."""
    import time
    import jax
    from jax.sharding import Mesh, PartitionSpec, NamedSharding
    from jax.experimental.shard_map import shard_map
    from concourse.bass2jax import (_bass_exec_p, install_neuronx_cc_hook,
                                    partition_id_tensor)
    install_neuronx_cc_hook()
    partition_name = nc.partition_id_tensor.name if nc.partition_id_tensor else None
    in_names, out_names, out_avals, zero_outs = [], [], [], []
    for alloc in nc.m.functions[0].allocations:
        if not isinstance(alloc, mybir.MemoryLocationSet):
            continue
        if not alloc.memorylocations:
            continue
        name = alloc.memorylocations[0].name
        if alloc.kind == "ExternalInput":
            if name != partition_name:
                in_names.append(name)
        elif alloc.kind == "ExternalOutput":
            out_names.append(name)
            shape = tuple(alloc.tensor_shape)
            dtype = mybir.dt.np(alloc.dtype)
            out_avals.append(jax.core.ShapedArray(shape, dtype))
            zero_outs.append(np.zeros(shape, dtype))
    n_params = len(in_names)
    n_outs = len(out_avals)
    all_in_names = list(in_names) + list(out_names)
    if partition_name is not None:
        all_in_names.append(partition_name)

    def _body(*args):
        operands = list(args)
        if partition_name is not None:
            operands.append(partition_id_tensor())
        return tuple(_bass_exec_p.bind(
            *operands, out_avals=tuple(out_avals), in_names=tuple(all_in_names),
            out_names=tuple(out_names), lowering_input_output_aliases=(),
            sim_require_finite=False, sim_require_nnan=False, nc=nc))

    devices = jax.devices()[:n_cores]
    mesh = Mesh(np.asarray(devices), ("core",))
    in_specs = (PartitionSpec("core"),) * (n_params + n_outs)
    out_specs = (PartitionSpec("core"),) * n_outs
    donate = tuple(range(n_params, n_params + n_outs))
    sharded = jax.jit(
        shard_map(_body, mesh=mesh, in_specs=in_specs, out_specs=out_specs,
                  check_rep=False),
        donate_argnums=donate, keep_unused=True)

    def run_fn(in_maps, repeats=1):
        per_core = [[np.asarray(m[name]) for name in in_names] for m in in_maps]
        concat_in = [np.concatenate([per_core[c][i] for c in range(n_cores)], 0)
                     for i in range(n_params)]
        sharding = NamedSharding(mesh, PartitionSpec("core"))
        dev_in = [jax.device_put(a, sharding) for a in concat_in]
        for a in dev_in:
            a.block_until_ready()
        times, out_arrs = [], None
        for _ in range(repeats):
            concat_zeros = [jax.device_put(
                np.zeros((n_cores * z.shape[0], *z.shape[1:]), z.dtype), sharding)
                for z in zero_outs]
            for z in concat_zeros:
                z.block_until_ready()
            t0 = time.perf_counter()
            out_arrs = sharded(*dev_in, *concat_zeros)
            for o in out_arrs:
                o.block_until_ready()
            times.append(time.perf_counter() - t0)
        results = [
            {name: np.asarray(out_arrs[i]).reshape(n_cores, *out_avals[i].shape)[c]
             for i, name in enumerate(out_names)}
            for c in range(n_cores)]
        return results, min(times)

    return run_fn

F32 = mybir.dt.float32
BF16 = mybir.dt.bfloat16
I16 = mybir.dt.int16
BF = ml_dtypes.bfloat16

NCORES = 8
N = 100000
IN_DIM = 128
HID = 32
OUT_DIM = 16
NSH = N // NCORES            # 12500
NT = 98                      # merge tiles per core
NSHP = NT * 128              # 12544
CH1 = 25000                  # table1 nodes per chunk
NCH = 4
CH1R = CH1 + 1               # +pad row
R2 = NCORES * NSHP           # 100352
CH2 = R2 // NCH              # 25088 (= 2 padded cores)
CH2R = CH2 + 1
BUCKETS = (1, 2, 3, 4, 5, 6, 8, 16)
NEG = -1.0e30
EPS = 1e-16
MAXD = 1024                  # max descriptors per SWDGE call (HW ring limit)
RING = 16384                 # SWDGE descriptor ring bytes (>16KB hangs HW)
SEG = MAXD // 128            # max gather cols per SWDGE call
DB = 32                      # dense batch (tiles per DMA)
PG = 3                       # dense PSUM group (tiles per PSUM bank tile)
MB = 16                      # merge batch (tiles)
NTA = 49                     # merge tiles in first allgather half
NHA = NTA * 128              # 6272

AluOp = mybir.AluOpType
ActFn = mybir.ActivationFunctionType
Axis = mybir.AxisListType


def _colgroups(D):
    out = []
    c = 0
    while c < D:
        w = min(8, D - c)
        out.append((c, w))
        c += w
    return out


def plan_segments(T):
    """Host/device shared slot-stream layout (order: k, bucket, tile, col, p)."""
    calls = []
    rowbase = {}
    grid_rows = []
    off = 0
    for k in range(NCH):
        rb = 0
        for bi, D in enumerate(BUCKETS):
            for t in range(int(T[k][bi])):
                rowbase[(k, bi, t)] = rb
                for (c0, w) in _colgroups(D):
                    calls.append((k, bi, t, c0, w, off))
                    off += 128 * w
                rb += 128
        grid_rows.append(rb)
    return calls, rowbase, grid_rows, off


def _wrap_idx(flat):
    n = len(flat)
    assert n % 16 == 0
    w = np.asarray(flat, np.int16).reshape(n // 16, 16).T
    return np.ascontiguousarray(np.tile(w, (8, 1)))


def fold(W, a):
    Hh, F = a.shape
    w = np.zeros((W.shape[0], Hh), np.float32)
    for h in range(Hh):
        w[:, h] = W[:, h * F:(h + 1) * F] @ a[h]
    return w


def host_prep(x, edge_index, W1, a1_src, a1_dst, b1, W2, a2_src, a2_dst, b2):
    x = np.asarray(x, np.float32)
    ei = np.asarray(edge_index)
    src = ei[0].astype(np.int64)
    dst = ei[1].astype(np.int64)
    W1 = np.asarray(W1, np.float32)
    W2 = np.asarray(W2, np.float32)
    Waug1 = np.concatenate([W1, fold(W1, np.asarray(a1_src, np.float32))], 1)
    Waug2 = np.concatenate([W2, fold(W2, np.asarray(a2_src, np.float32))], 1)
    Wad1 = fold(W1, np.asarray(a1_dst, np.float32))          # [128, 4]
    Wad2 = fold(W2, np.asarray(a2_dst, np.float32))          # [32, 4]
    xT = np.ascontiguousarray(x.T.astype(BF))

    core_of = dst // NSH
    # ---- per-core, per-chunk CSR ----
    pc = []  # [core][chunk] = (deg, sorted_src_by_dst, starts)
    for c in range(NCORES):
        m = core_of == c
        s_c, d_c = src[m], dst[m] - c * NSH
        ch = s_c // CH1
        info = []
        for k in range(NCH):
            mk = ch == k
            sk, dk = s_c[mk], d_c[mk]
            deg = np.bincount(dk, minlength=NSH)
            order = np.argsort(dk, kind="stable")
            sk = sk[order]
            starts = np.zeros(NSH + 1, np.int64)
            np.cumsum(deg, out=starts[1:])
            info.append((deg, sk, starts))
        pc.append(info)

    # shared tile counts
    T = [[0] * len(BUCKETS) for _ in range(NCH)]
    for c in range(NCORES):
        for k in range(NCH):
            deg = pc[c][k][0]
            for bi, D in enumerate(BUCKETS):
                lo = BUCKETS[bi - 1] if bi else 0
                nb = int(((deg > lo) & (deg <= D)).sum())
                T[k][bi] = max(T[k][bi], (nb + 127) // 128)
            assert deg.max(initial=0) <= BUCKETS[-1], f"deg max {deg.max()}"
    calls, rowbase, grid_rows, stream_len = plan_segments(T)

    b1rep = np.tile(np.asarray(b1, np.float32)[None, :], (128, 1))
    b2rep = np.tile(np.asarray(b2, np.float32)[None, :], (128, 1))
    pad1 = np.zeros((1, 256), BF); pad1[0, 128:132] = NEG
    pad2 = np.zeros((1, 128), BF); pad2[0, 64:68] = NEG
    z256 = np.zeros((1, 256), BF)

    in_maps = []
    for c in range(NCORES):
        slot_nodes = []   # per chunk: grid row -> node (or -1)
        for k in range(NCH):
            gr = grid_rows[k]
            deg, sk, starts = pc[c][k]
            nodes_of = np.full(gr, -1, np.int64)
            for bi, D in enumerate(BUCKETS):
                lo = BUCKETS[bi - 1] if bi else 0
                nd = np.where((deg > lo) & (deg <= D))[0]
                rb = rowbase[(k, bi, 0)] if T[k][bi] else 0
                nodes_of[rb:rb + len(nd)] = nd
            slot_nodes.append(nodes_of)

        s1 = np.full(stream_len, CH1, np.int64)     # pad -> table1 chunk pad row
        s2 = np.full(stream_len, CH2, np.int64)     # pad -> table2 chunk pad row
        for (k, bi, t, c0, w, off) in calls:
            D = BUCKETS[bi]
            rb = rowbase[(k, bi, t)]
            deg, sk, starts = pc[c][k]
            nodes = slot_nodes[k][rb:rb + 128]
            j = off
            for d in range(c0, c0 + w):
                for p in range(128):
                    nd = nodes[p]
                    if nd >= 0 and d < starts[nd + 1] - starts[nd]:
                        s = sk[starts[nd] + d]
                        s1[j] = s % CH1
                        s2[j] = (s // NSH % 2) * NSHP + s % NSH
                    j += 1
        slot1w = _wrap_idx(s1)
        slot2w = _wrap_idx(s2)

        # ad idx: per (k, gridtile) 128 local dst ids (pad -> 0)
        adix = []
        for k in range(NCH):
            nd = slot_nodes[k]
            adix.append(np.where(nd >= 0, nd, 0))
        adw = _wrap_idx(np.concatenate(adix)) if stream_len else None

        # merge idx: per chunk, per natural node (padded to NSHP): grid row or zero-row
        mrg = []
        for k in range(NCH):
            deg = pc[c][k][0]
            pos = np.full(NSHP, grid_rows[k], np.int64)  # zero row
            nd = slot_nodes[k]
            real = nd >= 0
            pos[nd[real]] = np.nonzero(real)[0]
            mrg.append(pos)
        mrgw = _wrap_idx(np.concatenate(mrg))

        in_maps.append(dict(
            xT=xT, Waug1=Waug1.astype(BF), Waug2=Waug2.astype(BF),
            Wad1=Wad1.astype(BF), Wad2=Wad2.astype(BF),
            b1rep=b1rep, b2rep=b2rep, pad1=pad1, pad2=pad2, z256=z256,
            slot1w=slot1w, slot2w=slot2w, adw=adw, mrgw=mrgw,
        ))
    meta = dict(T=T, calls=calls, rowbase=rowbase, grid_rows=grid_rows,
                stream_len=stream_len)
    return in_maps, meta


def vap(t, off, dims):
    a = t[:]
    return bass.AP(a.tensor, a.offset + off, [list(a.ap[0])] + [list(d) for d in dims])


def build_nc(meta):
    T = meta["T"]
    grid_rows = meta["grid_rows"]
    stream_len = meta["stream_len"]
    SW = stream_len // 16
    ADL = sum(grid_rows)
    AW = ADL // 16
    MW = (NCH * NSHP) // 16

    nc = bacc.Bacc("TRN2", target_bir_lowering=False, num_swdge_queues=1,
                   dynamic_dma_scratch_size=RING)
    dp = nc.declare_dram_parameter
    xT = dp("xT", [IN_DIM, N], BF16, isOutput=False)
    Waug1 = dp("Waug1", [128, 132], BF16, isOutput=False)
    Waug2 = dp("Waug2", [32, 68], BF16, isOutput=False)
    Wad1 = dp("Wad1", [128, 4], BF16, isOutput=False)
    Wad2 = dp("Wad2", [32, 4], BF16, isOutput=False)
    b1rep = dp("b1rep", [128, HID], F32, isOutput=False)
    b2rep = dp("b2rep", [128, OUT_DIM], F32, isOutput=False)
    pad1 = dp("pad1", [1, 256], BF16, isOutput=False)
    pad2 = dp("pad2", [1, 128], BF16, isOutput=False)
    z256 = dp("z256", [1, 256], BF16, isOutput=False)
    slot1w = dp("slot1w", [128, SW], I16, isOutput=False)
    slot2w = dp("slot2w", [128, SW], I16, isOutput=False)
    adw = dp("adw", [128, AW], I16, isOutput=False)
    mrgw = dp("mrgw", [128, MW], I16, isOutput=False)
    out2 = dp("out2", [NSHP, OUT_DIM], F32, isOutput=True)

    table1 = [nc.dram_tensor(f"table1_{k}", [CH1R, 256], BF16)
              for k in range(NCH)]
    table2 = [nc.dram_tensor(f"table2_{k}", [CH2R, 128], BF16)
              for k in range(NCH)]
    ad1c = nc.dram_tensor("ad1c", [NSHP, 64], F32)
    ad2c = nc.dram_tensor("ad2c", [NSHP, 64], F32)
    stg1 = [nc.dram_tensor(f"stg1_{k}", [grid_rows[k] + 1, 256], BF16)
            for k in range(NCH)]
    stg2 = [nc.dram_tensor(f"stg2_{k}", [grid_rows[k] + 1, 128], BF16)
            for k in range(NCH)]
    h1T_sh = [nc.dram_tensor("h1T_shA", [32, NHA], BF16),
              nc.dram_tensor("h1T_shB", [32, NSHP - NHA], BF16)]
    h1T_all = [nc.dram_tensor("h1T_allA", [NCORES, 32, NHA], BF16,
                              addr_space="Shared"),
               nc.dram_tensor("h1T_allB", [NCORES, 32, NSHP - NHA], BF16,
                              addr_space="Shared")]

    with tile.TileContext(nc) as tc:
        nc.gpsimd.load_library(mlp_lib)

        # ---------- consts / pads ----------
        with tc.tile_pool(name="konst", bufs=1) as kp:
            w1sb = kp.tile([128, 132], BF16)
            nc.sync.dma_start(out=w1sb[:], in_=Waug1[:, :])
            w2sb = kp.tile([32, 68], BF16)
            nc.sync.dma_start(out=w2sb[:], in_=Waug2[:, :])
            wad1sb = kp.tile([128, 4], BF16)
            nc.sync.dma_start(out=wad1sb[:], in_=Wad1[:, :])
            wad2sb = kp.tile([32, 4], BF16)
            nc.sync.dma_start(out=wad2sb[:], in_=Wad2[:, :])
            # edge/merge index streams, loaded up front so the edge pipelines
            # can start as soon as their table chunks are written
            SW = stream_len // 16
            AW = sum(grid_rows) // 16
            MW = (NCH * NSHP) // 16
            sidx1 = kp.tile([128, SW], I16)
            nc.sync.dma_start(out=sidx1[:], in_=slot1w[:, :])
            sidx2 = kp.tile([128, SW], I16)
            nc.sync.dma_start(out=sidx2[:], in_=slot2w[:, :])
            aidx = kp.tile([128, AW], I16)
            nc.sync.dma_start(out=aidx[:], in_=adw[:, :])
            midx = kp.tile([128, MW], I16)
            nc.sync.dma_start(out=midx[:], in_=mrgw[:, :])
            b1sb = kp.tile([128, HID], F32)
            nc.sync.dma_start(out=b1sb[:], in_=b1rep[:, :])
            b2sb = kp.tile([128, OUT_DIM], F32)
            nc.sync.dma_start(out=b2sb[:], in_=b2rep[:, :])
            for k in range(NCH):
                nc.sync.dma_start(out=table1[k][CH1, :], in_=pad1[0, :])
                nc.sync.dma_start(out=table2[k][CH2, :], in_=pad2[0, :])
                nc.sync.dma_start(out=stg1[k][grid_rows[k], :], in_=z256[0, :])
                nc.sync.dma_start(out=stg2[k][grid_rows[k], :], in_=z256[0, :128])

            # ---------- ad1 pre-pass: own-shard x @ Wad1 -> ad1c ----------
            pid = nc.sync.partition_id()
            with (nc.named_scope("ad1pass"),
                  tc.tile_pool(name="a1", bufs=2) as apool,
                  tc.tile_pool(name="a1p", bufs=2, space="PSUM") as aps):
                ADB = 16
                base = pid * NSH
                b0 = 0
                while b0 < NT:
                    nb = min(ADB, NT - b0)
                    ncol = min(nb * 128, NSH - b0 * 128)
                    xmA = apool.tile([128, ADB * 128], BF16, tag="xmA")
                    nc.sync.dma_start(
                        out=xmA[:, 0:ncol],
                        in_=xT[:, bass.ds(base + b0 * 128, ncol)])
                    psA = aps.tile([128, ADB, 4], F32, tag="psA")
                    if ncol < nb * 128:
                        nc.vector.memset(psA[:], 0.0)
                    for t in range(nb):
                        nn = min(128, ncol - t * 128)
                        if nn <= 0:
                            break
                        nc.tensor.matmul(
                            out=bass.AP(psA[:].tensor, psA[:].offset + t * 4,
                                        [[list(psA[:].ap[0])[0], nn], [1, 4]]),
                            lhsT=xmA[:, t * 128:t * 128 + nn],
                            rhs=wad1sb[:], start=True, stop=True)
                    adt = apool.tile([128, ADB * 4], F32, tag="adt")
                    nc.vector.tensor_copy(out=adt[:, 0:nb * 4],
                                          in_=psA[:, 0:nb, :])
                    nc.sync.dma_start(
                        out=bass.AP(ad1c[:, :].tensor, b0 * 128 * 64,
                                    [[64, 128], [64 * 128, nb], [1, 4]]),
                        in_=vap(adt, 0, [[4, nb], [1, 4]]))
                    b0 += nb

            # ---------- dense1 + edge1 (pools coexist so both overlap) ----
            with (tc.tile_pool(name="eg1", bufs=3) as gp1,
                  tc.tile_pool(name="ea1", bufs=2) as ap1,
                  tc.tile_pool(name="eso1", bufs=2) as sop1,
                  tc.tile_pool(name="ew1", bufs=3) as wp1):
                with (nc.named_scope("dense1"),
                      tc.tile_pool(name="d1", bufs=3) as dpool,
                      tc.tile_pool(name="d1b", bufs=2) as bpool,
                      tc.tile_pool(name="d1p", bufs=2, space="PSUM") as dps):
                    _dense_pass(nc, tc, dpool, bpool, dps, layer=1,
                                src=xT, wsb=w1sb, table=table1, h1T_all=None)

                # ---------- edge pass L1 ----------
                with nc.named_scope("edge1"):
                    _edge_pass(nc, tc, meta, layer=1, sidx=sidx1, aidx=aidx,
                               table=table1, stg=stg1, ad_core=ad1c,
                               pools=(gp1, ap1, sop1, wp1))

            # ---------- merge1 + dense2 + edge2 (L2 pools open early so
            # dense2 need not wait for merge1's pool region to free) ------
            with (tc.tile_pool(name="eg2", bufs=3) as gp2,
                  tc.tile_pool(name="ea2", bufs=2) as ap2,
                  tc.tile_pool(name="eso2", bufs=2) as sop2,
                  tc.tile_pool(name="ew2", bufs=3) as wp2,
                  tc.tile_pool(name="d2", bufs=3) as dpool2,
                  tc.tile_pool(name="d2b", bufs=2) as bpool2,
                  tc.tile_pool(name="d2p", bufs=4, space="PSUM") as dps2):
                with nc.named_scope("merge1"):
                    _merge_pass(nc, tc, meta, layer=1, midx=midx, stg=stg1,
                                bsb=b1sb, out2=None, h1T_sh=h1T_sh,
                                wadsb=wad2sb, adc=ad2c, nta=NTA,
                                h1T_all=h1T_all)

                with nc.named_scope("dense2"):
                    _dense_pass(nc, tc, dpool2, bpool2, dps2, layer=2,
                                src=None, wsb=w2sb, table=table2,
                                h1T_all=h1T_all)

                # ---------- edge pass L2 ----------
                with nc.named_scope("edge2"):
                    _edge_pass(nc, tc, meta, layer=2, sidx=sidx2, aidx=aidx,
                               table=table2, stg=stg2, ad_core=ad2c,
                               pools=(gp2, ap2, sop2, wp2))

            # ---------- merge L2 -> out2 ----------
            with nc.named_scope("merge2"):
                _merge_pass(nc, tc, meta, layer=2, midx=midx, stg=stg2,
                            bsb=b2sb, out2=out2, h1T_sh=None,
                            wadsb=None, adc=None)

    nc.finalize()
    return nc


def _dense_pass(nc, tc, dpool, bpool, dps, layer, src, wsb, table, h1T_all):
    """Replicated dense phase: DB-tile batches, PG-tile PSUM groups.
    layer 1: in xT f32 [128, N] -> table1 rows [h(128)|as_hi(4)|as_lo(4)] bf16.
    layer 2: in h1T_all bf16 -> table2 rows [h(64)|as_hi|as_lo] bf16.
    """
    if layer == 1:
        ntiles, K, MC = (N + 127) // 128, 128, 132   # matmul out cols
        FD = 128
        CHN, Ntot = CH1, N
        PG, PST = 4, 512       # PSUM group; slot stride padded to a full bank
    else:
        ntiles, K, MC = R2 // 128, 32, 68
        FD = 64
        CHN, Ntot = CH2, R2
        PG, PST = 7, 68        # 7 x 272B fits one bank
    RW = 256 if layer == 1 else 128

    b0 = 0
    eng_i = 0
    while b0 < ntiles:
        nb = min(DB, ntiles - b0)
        n0 = b0 * 128
        # ---- batched input load ----
        xm = dpool.tile([K, DB * 128], BF16, tag="xm")
        if layer == 1:
            nn = min(nb * 128, N - n0)
            nc.sync.dma_start(out=xm[:, 0:nn], in_=bass.AP(
                src[:, :].tensor, n0, [[N, K], [1, nn]]))
        else:
            # h1T_all halves [NCORES, 32, NHA/(NSHP-NHA)]; split loads at
            # core and half boundaries
            q = b0
            col = 0
            while q < b0 + nb:
                cc, tt = q // NT, q % NT
                if tt < NTA:
                    hf, tb, hw = 0, 0, NHA
                else:
                    hf, tb, hw = 1, NTA, NSHP - NHA
                run = min((NTA if tt < NTA else NT) - tt, b0 + nb - q)
                nc.sync.dma_start(
                    out=xm[:, col * 128:(col + run) * 128],
                    in_=bass.AP(h1T_all[hf][:, :, :].tensor,
                                cc * 32 * hw + (tt - tb) * 128,
                                [[hw, 32], [1, run * 128]]))
                q += run
                col += run
        # ---- batch output tiles (row = [h | as]) ----
        hrowB = bpool.tile([128, DB, FD + 8], BF16, tag="hrowB")
        g0 = 0
        while g0 < nb:
            ng = min(PG, nb - g0)
            ps = dps.tile([128, PG, PST], F32, tag="ps")
            partial = (n0 + (g0 + ng) * 128) > Ntot
            if partial:
                nc.vector.memset(ps[:], 0.0)
            for t in range(ng):
                tt = g0 + t
                nn = min(128, Ntot - (n0 + tt * 128))
                nc.tensor.matmul(
                    out=bass.AP(ps[:].tensor, ps[:].offset + (t * PST),
                                [[list(ps[:].ap[0])[0], nn], [1, MC]]),
                    lhsT=xm[:, tt * 128:tt * 128 + nn],
                    rhs=wsb[:], start=True, stop=True)
            use_act = (eng_i % 2 == 1)
            eng_i += 1

            def _copy(out, in_):
                if use_act:
                    nc.scalar.activation(out=out, in_=in_, func=ActFn.Copy)
                else:
                    nc.vector.tensor_copy(out=out, in_=in_)

            # bulk copy [h | as_hi] (+ leave as_lo slot) per PSUM group
            _copy(vap(hrowB, g0 * (FD + 8), [[FD + 8, ng], [1, FD + 4]]),
                  vap(ps, 0, [[PST, ng], [1, FD + 4]]))
            g0 += ng
        # table rows per chunk tensor; split at chunk boundary
        t0 = 0
        while t0 < nb:
            gn0 = n0 + t0 * 128
            rows = min(128, Ntot - gn0)
            k = gn0 // CHN
            avail = (k + 1) * CHN - gn0
            if avail >= rows:
                if rows == 128:
                    run = min(nb - t0, avail // 128)
                else:
                    run = 1
                r0 = gn0 - k * CHN
                nc.sync.dma_start(
                    out=bass.AP(table[k][:, :].tensor, r0 * RW,
                                [[RW, rows], [RW * 128 if run > 1 else 1, run],
                                 [1, FD + 4]])
                    if run > 1 else
                    bass.AP(table[k][:, :].tensor, r0 * RW,
                            [[RW, rows], [1, FD + 4]]),
                    in_=hrowB[:, t0:t0 + run, 0:FD + 4] if run > 1
                    else hrowB[0:rows, t0, 0:FD + 4])
                t0 += run
            else:
                # tile straddles the chunk boundary: split by partition range
                nsplit = avail
                r0 = gn0 - k * CHN
                nc.sync.dma_start(
                    out=bass.AP(table[k][:, :].tensor, r0 * RW,
                                [[RW, nsplit], [1, FD + 4]]),
                    in_=hrowB[0:nsplit, t0, 0:FD + 4])
                nc.sync.dma_start(
                    out=bass.AP(table[k + 1][:, :].tensor, 0,
                                [[RW, rows - nsplit], [1, FD + 4]]),
                    in_=hrowB[nsplit:rows, t0, 0:FD + 4])
                t0 += 1
        b0 += nb


def _edge_pass(nc, tc, meta, layer, sidx, aidx, table, stg, ad_core, pools):
    rowbase = meta["rowbase"]
    grid_rows = meta["grid_rows"]
    RW = 256 if layer == 1 else 128       # table row elems (bf16)
    FD = 128 if layer == 1 else 64        # feature elems

    # stream offset of each bucket's first slot (buckets are contiguous)
    bstart = {}
    for (k, bi, t, c0, w, off) in meta["calls"]:
        bstart.setdefault((k, bi), off)

    gp, ap_pool, sop, wp = pools
    if True:
        abase = 0
        for k in range(NCH):
            for bi, D in enumerate(BUCKETS):
                Tb = int(meta["T"][k][bi])
                if Tb == 0:
                    continue
                rb0 = rowbase[(k, bi, 0)]
                # per-bucket ad gather (<=2048-idx calls)
                ADG = ap_pool.tile([128, Tb, 64], F32, tag="ADG")
                na = Tb * 128
                o = 0
                while o < na:
                    nbv = min(MAXD, na - o)
                    nc.gpsimd.dma_gather(
                        ADG[:, o // 128:(o + nbv) // 128, :], ad_core[:, :],
                        aidx[:, (abase + rb0 + o) // 16:
                                (abase + rb0 + o + nbv) // 16],
                        nbv, nbv, 64)
                    o += nbv
                # per-bucket f32 accumulator + bf16 staging copy
                fsB = sop.tile([128, Tb, FD + 4], F32, tag="fsB")
                soB = sop.tile([128, Tb, FD + 4], BF16, tag="soB")
                # segments of <= SEG cols (tile-aligned)
                gt = max(1, SEG // D)      # tiles per segment
                t0 = 0
                off = bstart[(k, bi)]
                while t0 < Tb:
                    gn = min(gt, Tb - t0)
                    ncols = gn * D
                    G = gp.tile([128, max(SEG, D), RW], BF16, tag="G")
                    so = off + 128 * (t0 * D)
                    c = 0
                    while c < ncols:
                        w = min(SEG, ncols - c)
                        nc.gpsimd.dma_gather(
                            G[:, c:c + w, :], table[k][:, :],
                            sidx[:, (so + 128 * c) // 16:
                                    (so + 128 * (c + w)) // 16],
                            128 * w, 128 * w, RW)
                        c += w
                    # e = exp(leaky(as_hi + as_lo + ad))  [f32]
                    e = wp.tile([128, max(SEG, D) * 4], F32, tag="e")
                    nc.vector.tensor_tensor(
                        out=e[:, 0:ncols * 4],
                        in0=vap(G, FD, [[RW, ncols], [1, 4]]),
                        in1=bass.AP(ADG[:].tensor,
                                    ADG[:].offset + t0 * 64,
                                    [list(ADG[:].ap[0]), [64, gn], [0, D],
                                     [1, 4]]),
                        op=AluOp.add)
                    nc.vector.scalar_tensor_tensor(
                        out=e[:, 0:ncols * 4], in0=e[:, 0:ncols * 4],
                        scalar=0.2, in1=e[:, 0:ncols * 4],
                        op0=AluOp.mult, op1=AluOp.max)
                    # exp twice on ACT: f32 (denominators) + bf16 (weights)
                    ebf = wp.tile([128, max(SEG, D) * 4], BF16, tag="ebf")
                    nc.scalar.activation(out=ebf[:, 0:ncols * 4],
                                         in_=e[:, 0:ncols * 4], func=ActFn.Exp)
                    nc.scalar.activation(out=e[:, 0:ncols * 4],
                                         in_=e[:, 0:ncols * 4], func=ActFn.Exp)
                    # denominators (f32 accumulate)
                    nc.vector.tensor_reduce(
                        out=bass.AP(fsB[:].tensor,
                                    fsB[:].offset + t0 * (FD + 4) + FD,
                                    [list(fsB[:].ap[0]), [FD + 4, gn],
                                     [1, 4]]),
                        in_=vap(e, 0, [[4 * D, gn], [1, 4], [4, D]]),
                        axis=Axis.X, op=AluOp.add)
                    # segment-fused weighted features
                    val = wp.tile([128, max(SEG, D) * FD], BF16, tag="val")
                    nc.vector.tensor_tensor(
                        out=vap(val, 0, [[D * FD, gn], [FD, D],
                                         [FD // 4, 4], [1, FD // 4]]),
                        in0=vap(G, 0, [[RW * D, gn], [RW, D],
                                       [FD // 4, 4], [1, FD // 4]]),
                        in1=vap(ebf, 0, [[4 * D, gn], [4, D],
                                         [1, 4], [0, FD // 4]]),
                        op=AluOp.mult)
                    nc.vector.tensor_reduce(
                        out=bass.AP(fsB[:].tensor,
                                    fsB[:].offset + t0 * (FD + 4),
                                    [list(fsB[:].ap[0]), [FD + 4, gn],
                                     [1, FD]]),
                        in_=vap(val, 0, [[D * FD, gn], [1, FD], [FD, D]]),
                        axis=Axis.X, op=AluOp.add)
                    t0 += gn
                # one bf16 round + one staging write per bucket
                nc.vector.tensor_copy(out=soB[:], in_=fsB[:])
                nc.sync.dma_start(
                    out=bass.AP(stg[k][:, :].tensor, rb0 * RW,
                                [[RW, 128], [RW * 128, Tb], [1, FD + 4]]),
                    in_=soB[:])
            abase += grid_rows[k]


def _merge_pass(nc, tc, meta, layer, midx, stg, bsb, out2, h1T_sh,
                wadsb, adc, nta=None, h1T_all=None):
    RW = 256 if layer == 1 else 128
    FD = 128 if layer == 1 else 64
    OD = HID if layer == 1 else OUT_DIM
    W = FD + 4
    if layer == 1:
        ranges = [(0, nta, 0), (nta, NT, 1)]
    else:
        ranges = [(0, NT, 0)]

    with (tc.tile_pool(name=f"mi{layer}", bufs=1) as ip,
          tc.tile_pool(name=f"mg{layer}", bufs=2) as gp,
          tc.tile_pool(name=f"ms{layer}", bufs=2) as sp_pool,
          tc.tile_pool(name=f"mw{layer}", bufs=2) as wp,
          tc.tile_pool(name=f"mp{layer}", bufs=2, space="PSUM") as pp):
        if layer == 1:
            from concourse.masks import make_identity
            ident = ip.tile([128, 128], F32, tag="ident")
            make_identity(nc, ident[:])

        for (t_lo, t_hi, hf) in ranges:
            _merge_range(nc, meta, layer, midx, stg, bsb, out2,
                         h1T_sh[hf] if layer == 1 else None,
                         wadsb, adc, gp, sp_pool, wp, pp,
                         ident if layer == 1 else None,
                         t_lo, t_hi, RW, FD, OD, W)
        if layer == 1:
            for hf in range(2):
                nc.gpsimd.collective_compute(
                    "AllGather", AluOp.bypass,
                    replica_groups=[list(range(NCORES))],
                    ins=[h1T_sh[hf][:, :]], outs=[h1T_all[hf][:, :, :]])


def _merge_range(nc, meta, layer, midx, stg, bsb, out2, h1T_sh, wadsb, adc,
                 gp, sp_pool, wp, pp, ident, t_lo, t_hi, RW, FD, OD, W):
        mt = t_lo
        while mt < t_hi:
            nb = min(MB, t_hi - mt)
            s = sp_pool.tile([128, MB * W], F32, tag="s")
            s01 = wp.tile([128, MB * W], BF16, tag="s01")
            Gprev = None
            for k in range(NCH):
                Gk = gp.tile([128, MB, RW], BF16, tag="MG")
                ioff = k * NSHP + mt * 128
                o = 0
                while o < nb * 128:
                    nbv = min(MAXD, nb * 128 - o)
                    nc.gpsimd.dma_gather(
                        Gk[:, o // 128:(o + nbv) // 128, :], stg[k][:, :],
                        midx[:, (ioff + o) // 16:(ioff + o + nbv) // 16],
                        nbv, nbv, RW)
                    o += nbv
                if k == 1:
                    # bf16 pair-add runs in the DVE 2x fast mode
                    with nc.allow_low_precision(reason="bf16 staged pair"):
                        nc.vector.tensor_tensor(
                            out=s01[:, 0:nb * W],
                            in0=vap(Gprev, 0, [[RW, nb], [1, W]]),
                            in1=vap(Gk, 0, [[RW, nb], [1, W]]), op=AluOp.add)
                elif k == 2:
                    nc.vector.tensor_tensor(
                        out=vap(s, 0, [[W, nb], [1, W]]),
                        in0=s01[:, 0:nb * W],
                        in1=vap(Gk, 0, [[RW, nb], [1, W]]), op=AluOp.add)
                elif k == 3:
                    nc.vector.tensor_tensor(
                        out=vap(s, 0, [[W, nb], [1, W]]),
                        in0=vap(s, 0, [[W, nb], [1, W]]),
                        in1=vap(Gk, 0, [[RW, nb], [1, W]]), op=AluOp.add)
                Gprev = Gk
            rec = wp.tile([128, MB * 4], F32, tag="rec")
            nc.vector.tensor_scalar_add(
                out=vap(rec, 0, [[4, nb], [1, 4]]),
                in0=vap(s, FD, [[W, nb], [1, 4]]), scalar1=EPS)
            nc.vector.reciprocal(out=rec[:, 0:nb * 4], in_=rec[:, 0:nb * 4])
            nc.vector.tensor_scalar_mul(out=rec[:, 0:nb * 4],
                                        in0=rec[:, 0:nb * 4], scalar1=0.25)
            sc = wp.tile([128, MB * FD], F32, tag="sc")
            nc.vector.tensor_tensor(
                out=vap(sc, 0, [[FD, nb], [FD // 4, 4], [1, FD // 4]]),
                in0=vap(s, 0, [[W, nb], [FD // 4, 4], [1, FD // 4]]),
                in1=vap(rec, 0, [[4, nb], [1, 4], [0, FD // 4]]),
                op=AluOp.mult)
            hs = wp.tile([128, MB * OD], F32, tag="hs")
            nc.vector.tensor_reduce(
                out=vap(hs, 0, [[OD, nb], [1, OD]]),
                in_=vap(sc, 0, [[FD, nb], [1, OD], [OD, 4]]),
                axis=Axis.X, op=AluOp.add)
            nc.vector.tensor_tensor(
                out=vap(hs, 0, [[OD, nb], [1, OD]]),
                in0=vap(hs, 0, [[OD, nb], [1, OD]]),
                in1=vap(bsb, 0, [[0, nb], [1, OD]]), op=AluOp.add)
            if layer == 1:
                nc.scalar.activation(out=hs[:, 0:nb * OD], in_=hs[:, 0:nb * OD],
                                     func=ActFn.Relu)
                hsbB = wp.tile([32, MB * 128], BF16, tag="hsbB")
                ti = 0
                while ti < nb:
                    jn = min(4, nb - ti)
                    psT = pp.tile([32, 4, 128], F32, tag="psT")
                    for j in range(jn):
                        nc.tensor.transpose(
                            out=psT[:, j, :],
                            in_=hs[:, (ti + j) * OD:(ti + j + 1) * OD],
                            identity=ident[:])
                    nc.vector.tensor_copy(
                        out=hsbB[:, ti * 128:(ti + jn) * 128],
                        in_=psT[:, 0:jn, :])
                    ti += jn
                nc.scalar.dma_start(
                    out=h1T_sh[:, (mt - t_lo) * 128:(mt - t_lo + nb) * 128],
                    in_=hsbB[:, 0:nb * 128])
                # ad2 for next layer: h1 @ Wad2, straight into ad2c
                psA = pp.tile([128, MB, 4], F32, tag="psA2")
                for ti in range(nb):
                    nc.tensor.matmul(
                        out=bass.AP(psA[:].tensor, psA[:].offset + ti * 4,
                                    [[list(psA[:].ap[0])[0], 128], [1, 4]]),
                        lhsT=hsbB[:, ti * 128:(ti + 1) * 128],
                        rhs=wadsb[:], start=True, stop=True)
                adt = wp.tile([128, MB * 4], F32, tag="adt2")
                nc.vector.tensor_copy(out=adt[:, 0:nb * 4],
                                      in_=psA[:, 0:nb, :])
                nc.scalar.dma_start(
                    out=bass.AP(adc[:, :].tensor, mt * 128 * 64,
                                [[64, 128], [64 * 128, nb], [1, 4]]),
                    in_=vap(adt, 0, [[4, nb], [1, 4]]))
            else:
                nc.sync.dma_start(
                    out=bass.AP(out2[:, :].tensor, mt * 128 * OD,
                                [[OD, 128], [OD * 128, nb], [1, OD]]),
                    in_=vap(hs, 0, [[OD, nb], [1, OD]]))
            mt += nb


_CACHE = {}


def kernel(**inputs):
    in_maps, meta = host_prep(**inputs)
    key = str(meta["T"])
    _CACHE["k"] = key
    if key not in _CACHE:
        nc = build_nc(meta)
        _CACHE[key] = (nc, make_runner(nc, NCORES))
    nc, run = _CACHE[key]
    results, best = run(in_maps, repeats=1)
    _CACHE["last_time"] = best
    out = np.empty((N, OUT_DIM), np.float32)
    for c in range(NCORES):
        out[c * NSH:(c + 1) * NSH] = results[c]["out2"][:NSH]
    return out



# revision 7
# speedup vs baseline: 1.1767x; 1.1767x over previous
"""GAT 2-layer kernel for TRN2, 8 NeuronCores (self-contained).

Strategy:
- dst-shard: core c owns nodes [c*12500, (c+1)*12500).
- ad pre-pass: own-shard x @ fold(W, a_dst) -> ad1c, so edge pipelines
  never wait on the full dense pass.
- Dense phases (x@W bf16, h1@W2 bf16) replicated on all cores; folded
  a_src gives per-node [h | as] rows in one matmul. 16-tile DMA batches
  (HWDGE calls are ~600ns each on one shared device), bank-sized PSUM
  groups, copy work split across DVE/ACT.
- Edge phase per core: 4 src-chunks (25000 nodes, int16 gather reach),
  per-chunk degree-bucketed padded CSR over dst. 1024-index single-queue
  SWDGE gathers (multi-queue under TileContext races; >16KB rings hang
  HW) pull [h | as] rows. e = exp(leaky(as+ad)) on ACT; segment-fused
  bf16 weighted-feature mult + f32 reduce into per-bucket tiles.
- Per-chunk partials [featsum | denom] -> DRAM staging (bf16); per-chunk
  table tensors + concurrently-open tile pools let edge chunks overlap
  the dense tail.
- Merge pass (natural node order): per-chunk 16-tile gathers, f32
  accumulate, per-head normalize, bias (+relu L1); L1 also computes
  ad2c (h1 @ fold(W2, a2_dst)) from the transposed tiles.
- L1->L2: h1T shard AllGather in two halves (overlaps merge tail and
  dense2 head) -> replicated dense2.
"""
import sys
sys.path.insert(0, "/opt/trn_rl_repo")
import numpy as np
import ml_dtypes

import concourse.bass as bass
import concourse.bacc as bacc
import concourse.tile as tile
from concourse import mybir
from concourse.library_config import mlp as mlp_lib


def make_runner(nc, n_cores):
    """PJRT runner: returns run_fn(in_maps, repeats) -> (results, best_time_s)."""
    import time
    import jax
    from jax.sharding import Mesh, PartitionSpec, NamedSharding
    from jax.experimental.shard_map import shard_map
    from concourse.bass2jax import (_bass_exec_p, install_neuronx_cc_hook,
                                    partition_id_tensor)
    install_neuronx_cc_hook()
    partition_name = nc.partition_id_tensor.name if nc.partition_id_tensor else None
    in_names, out_names, out_avals, zero_outs = [], [], [], []
    for alloc in nc.m.functions[0].allocations:
        if not isinstance(alloc, mybir.MemoryLocationSet):
            continue
        if not alloc.memorylocations:
            continue
        name = alloc.memorylocations[0].name
        if alloc.kind == "ExternalInput":
            if name != partition_name:
                in_names.append(name)
        elif alloc.kind == "ExternalOutput":
            out_names.append(name)
            shape = tuple(alloc.tensor_shape)
            dtype = mybir.dt.np(alloc.dtype)
            out_avals.append(jax.core.ShapedArray(shape, dtype))
            zero_outs.append(np.zeros(shape, dtype))
    n_params = len(in_names)
    n_outs = len(out_avals)
    all_in_names = list(in_names) + list(out_names)
    if partition_name is not None:
        all_in_names.append(partition_name)

    def _body(*args):
        operands = list(args)
        if partition_name is not None:
            operands.append(partition_id_tensor())
        return tuple(_bass_exec_p.bind(
            *operands, out_avals=tuple(out_avals), in_names=tuple(all_in_names),
            out_names=tuple(out_names), lowering_input_output_aliases=(),
            sim_require_finite=False, sim_require_nnan=False, nc=nc))

    devices = jax.devices()[:n_cores]
    mesh = Mesh(np.asarray(devices), ("core",))
    in_specs = (PartitionSpec("core"),) * (n_params + n_outs)
    out_specs = (PartitionSpec("core"),) * n_outs
    donate = tuple(range(n_params, n_params + n_outs))
    sharded = jax.jit(
        shard_map(_body, mesh=mesh, in_specs=in_specs, out_specs=out_specs,
                  check_rep=False),
        donate_argnums=donate, keep_unused=True)

    def run_fn(in_maps, repeats=1):
        per_core = [[np.asarray(m[name]) for name in in_names] for m in in_maps]
        concat_in = [np.concatenate([per_core[c][i] for c in range(n_cores)], 0)
                     for i in range(n_params)]
        sharding = NamedSharding(mesh, PartitionSpec("core"))
        dev_in = [jax.device_put(a, sharding) for a in concat_in]
        for a in dev_in:
            a.block_until_ready()
        times, out_arrs = [], None
        for _ in range(repeats):
            concat_zeros = [jax.device_put(
                np.zeros((n_cores * z.shape[0], *z.shape[1:]), z.dtype), sharding)
                for z in zero_outs]
            for z in concat_zeros:
                z.block_until_ready()
            t0 = time.perf_counter()
            out_arrs = sharded(*dev_in, *concat_zeros)
            for o in out_arrs:
                o.block_until_ready()
            times.append(time.perf_counter() - t0)
        results = [
            {name: np.asarray(out_arrs[i]).reshape(n_cores, *out_avals[i].shape)[c]
             for i, name in enumerate(out_names)}
            for c in range(n_cores)]
        return results, min(times)

    return run_fn

F32 = mybir.dt.float32
BF16 = mybir.dt.bfloat16
I16 = mybir.dt.int16
BF = ml_dtypes.bfloat16

NCORES = 8
N = 100000
IN_DIM = 128
HID = 32
OUT_DIM = 16
NSH = N // NCORES            # 12500
NT = 98                      # merge tiles per core
NSHP = NT * 128              # 12544
CH1 = 25000                  # table1 nodes per chunk
NCH = 4
CH1R = CH1 + 1               # +pad row
R2 = NCORES * NSHP           # 100352
CH2 = R2 // NCH              # 25088 (= 2 padded cores)
CH2R = CH2 + 1
BUCKETS = (1, 2, 3, 4, 5, 6, 8, 16)
NEG = -1.0e30
EPS = 1e-16
MAXD = 1024                  # max descriptors per SWDGE call (HW ring limit)
RING = 16384                 # SWDGE descriptor ring bytes (>16KB hangs HW)
NQ = 4                       # SWDGE queues (measured: 1q=92GB/s, 4q=450GB/s)
_QRR = [0]


def qn():
    _QRR[0] = (_QRR[0] + 1) % NQ
    return _QRR[0]
SEG = MAXD // 128            # max gather cols per SWDGE call
DB = 32                      # dense batch (tiles per DMA)
PG = 3                       # dense PSUM group (tiles per PSUM bank tile)
MB = 16                      # merge batch (tiles)
NTA = 49                     # merge tiles in first allgather half
NHA = NTA * 128              # 6272

AluOp = mybir.AluOpType
ActFn = mybir.ActivationFunctionType
Axis = mybir.AxisListType


def _colgroups(D):
    out = []
    c = 0
    while c < D:
        w = min(8, D - c)
        out.append((c, w))
        c += w
    return out


def plan_segments(T):
    """Host/device shared slot-stream layout (order: k, bucket, tile, col, p)."""
    calls = []
    rowbase = {}
    grid_rows = []
    off = 0
    for k in range(NCH):
        rb = 0
        for bi, D in enumerate(BUCKETS):
            for t in range(int(T[k][bi])):
                rowbase[(k, bi, t)] = rb
                for (c0, w) in _colgroups(D):
                    calls.append((k, bi, t, c0, w, off))
                    off += 128 * w
                rb += 128
        grid_rows.append(rb)
    return calls, rowbase, grid_rows, off


def _wrap_idx(flat):
    n = len(flat)
    assert n % 16 == 0
    w = np.asarray(flat, np.int16).reshape(n // 16, 16).T
    return np.ascontiguousarray(np.tile(w, (8, 1)))


def fold(W, a):
    Hh, F = a.shape
    w = np.zeros((W.shape[0], Hh), np.float32)
    for h in range(Hh):
        w[:, h] = W[:, h * F:(h + 1) * F] @ a[h]
    return w


def host_prep(x, edge_index, W1, a1_src, a1_dst, b1, W2, a2_src, a2_dst, b2):
    x = np.asarray(x, np.float32)
    ei = np.asarray(edge_index)
    src = ei[0].astype(np.int64)
    dst = ei[1].astype(np.int64)
    W1 = np.asarray(W1, np.float32)
    W2 = np.asarray(W2, np.float32)
    Waug1 = np.concatenate([W1, fold(W1, np.asarray(a1_src, np.float32))], 1)
    Waug2 = np.concatenate([W2, fold(W2, np.asarray(a2_src, np.float32))], 1)
    Wad1 = fold(W1, np.asarray(a1_dst, np.float32))          # [128, 4]
    Wad2 = fold(W2, np.asarray(a2_dst, np.float32))          # [32, 4]
    xT = np.ascontiguousarray(x.T.astype(BF))

    core_of = dst // NSH
    # ---- per-core, per-chunk CSR ----
    pc = []  # [core][chunk] = (deg, sorted_src_by_dst, starts)
    for c in range(NCORES):
        m = core_of == c
        s_c, d_c = src[m], dst[m] - c * NSH
        ch = s_c // CH1
        info = []
        for k in range(NCH):
            mk = ch == k
            sk, dk = s_c[mk], d_c[mk]
            deg = np.bincount(dk, minlength=NSH)
            order = np.argsort(dk, kind="stable")
            sk = sk[order]
            starts = np.zeros(NSH + 1, np.int64)
            np.cumsum(deg, out=starts[1:])
            info.append((deg, sk, starts))
        pc.append(info)

    # shared tile counts
    T = [[0] * len(BUCKETS) for _ in range(NCH)]
    for c in range(NCORES):
        for k in range(NCH):
            deg = pc[c][k][0]
            for bi, D in enumerate(BUCKETS):
                lo = BUCKETS[bi - 1] if bi else 0
                nb = int(((deg > lo) & (deg <= D)).sum())
                T[k][bi] = max(T[k][bi], (nb + 127) // 128)
            assert deg.max(initial=0) <= BUCKETS[-1], f"deg max {deg.max()}"
    calls, rowbase, grid_rows, stream_len = plan_segments(T)

    b1rep = np.tile(np.asarray(b1, np.float32)[None, :], (128, 1))
    b2rep = np.tile(np.asarray(b2, np.float32)[None, :], (128, 1))
    pad1 = np.zeros((1, 256), BF); pad1[0, 128:132] = NEG
    pad2 = np.zeros((1, 128), BF); pad2[0, 64:68] = NEG
    z256 = np.zeros((1, 256), BF)

    in_maps = []
    for c in range(NCORES):
        slot_nodes = []   # per chunk: grid row -> node (or -1)
        for k in range(NCH):
            gr = grid_rows[k]
            deg, sk, starts = pc[c][k]
            nodes_of = np.full(gr, -1, np.int64)
            for bi, D in enumerate(BUCKETS):
                lo = BUCKETS[bi - 1] if bi else 0
                nd = np.where((deg > lo) & (deg <= D))[0]
                rb = rowbase[(k, bi, 0)] if T[k][bi] else 0
                nodes_of[rb:rb + len(nd)] = nd
            slot_nodes.append(nodes_of)

        s1 = np.full(stream_len, CH1, np.int64)     # pad -> table1 chunk pad row
        s2 = np.full(stream_len, CH2, np.int64)     # pad -> table2 chunk pad row
        for (k, bi, t, c0, w, off) in calls:
            D = BUCKETS[bi]
            rb = rowbase[(k, bi, t)]
            deg, sk, starts = pc[c][k]
            nodes = slot_nodes[k][rb:rb + 128]
            j = off
            for d in range(c0, c0 + w):
                for p in range(128):
                    nd = nodes[p]
                    if nd >= 0 and d < starts[nd + 1] - starts[nd]:
                        s = sk[starts[nd] + d]
                        s1[j] = s % CH1
                        s2[j] = (s // NSH % 2) * NSHP + s % NSH
                    j += 1
        slot1w = _wrap_idx(s1)
        slot2w = _wrap_idx(s2)

        # ad idx: per (k, gridtile) 128 local dst ids (pad -> 0)
        adix = []
        for k in range(NCH):
            nd = slot_nodes[k]
            adix.append(np.where(nd >= 0, nd, 0))
        adw = _wrap_idx(np.concatenate(adix)) if stream_len else None

        # merge idx: per chunk, per natural node (padded to NSHP): grid row or zero-row
        mrg = []
        for k in range(NCH):
            deg = pc[c][k][0]
            pos = np.full(NSHP, grid_rows[k], np.int64)  # zero row
            nd = slot_nodes[k]
            real = nd >= 0
            pos[nd[real]] = np.nonzero(real)[0]
            mrg.append(pos)
        mrgw = _wrap_idx(np.concatenate(mrg))

        in_maps.append(dict(
            xT=xT, Waug1=Waug1.astype(BF), Waug2=Waug2.astype(BF),
            Wad1=Wad1.astype(BF), Wad2=Wad2.astype(BF),
            b1rep=b1rep, b2rep=b2rep, pad1=pad1, pad2=pad2, z256=z256,
            slot1w=slot1w, slot2w=slot2w, adw=adw, mrgw=mrgw,
        ))
    meta = dict(T=T, calls=calls, rowbase=rowbase, grid_rows=grid_rows,
                stream_len=stream_len)
    return in_maps, meta


def vap(t, off, dims):
    a = t[:]
    return bass.AP(a.tensor, a.offset + off, [list(a.ap[0])] + [list(d) for d in dims])


def build_nc(meta):
    _QRR[0] = 0
    T = meta["T"]
    grid_rows = meta["grid_rows"]
    stream_len = meta["stream_len"]
    SW = stream_len // 16
    ADL = sum(grid_rows)
    AW = ADL // 16
    MW = (NCH * NSHP) // 16

    nc = bacc.Bacc("TRN2", target_bir_lowering=False, num_swdge_queues=NQ,
                   dynamic_dma_scratch_size=RING)
    dp = nc.declare_dram_parameter
    xT = dp("xT", [IN_DIM, N], BF16, isOutput=False)
    Waug1 = dp("Waug1", [128, 132], BF16, isOutput=False)
    Waug2 = dp("Waug2", [32, 68], BF16, isOutput=False)
    Wad1 = dp("Wad1", [128, 4], BF16, isOutput=False)
    Wad2 = dp("Wad2", [32, 4], BF16, isOutput=False)
    b1rep = dp("b1rep", [128, HID], F32, isOutput=False)
    b2rep = dp("b2rep", [128, OUT_DIM], F32, isOutput=False)
    pad1 = dp("pad1", [1, 256], BF16, isOutput=False)
    pad2 = dp("pad2", [1, 128], BF16, isOutput=False)
    z256 = dp("z256", [1, 256], BF16, isOutput=False)
    slot1w = dp("slot1w", [128, SW], I16, isOutput=False)
    slot2w = dp("slot2w", [128, SW], I16, isOutput=False)
    adw = dp("adw", [128, AW], I16, isOutput=False)
    mrgw = dp("mrgw", [128, MW], I16, isOutput=False)
    out2 = dp("out2", [NSHP, OUT_DIM], F32, isOutput=True)

    table1 = [nc.dram_tensor(f"table1_{k}", [CH1R, 256], BF16)
              for k in range(NCH)]
    table2 = [nc.dram_tensor(f"table2_{k}", [CH2R, 128], BF16)
              for k in range(NCH)]
    ad1c = nc.dram_tensor("ad1c", [NSHP, 64], F32)
    ad2c = nc.dram_tensor("ad2c", [NSHP, 64], F32)
    stg1 = [nc.dram_tensor(f"stg1_{k}", [grid_rows[k] + 1, 256], BF16)
            for k in range(NCH)]
    stg2 = [nc.dram_tensor(f"stg2_{k}", [grid_rows[k] + 1, 128], BF16)
            for k in range(NCH)]
    h1T_sh = [nc.dram_tensor("h1T_shA", [32, NHA], BF16),
              nc.dram_tensor("h1T_shB", [32, NSHP - NHA], BF16)]
    h1T_all = [nc.dram_tensor("h1T_allA", [NCORES, 32, NHA], BF16,
                              addr_space="Shared"),
               nc.dram_tensor("h1T_allB", [NCORES, 32, NSHP - NHA], BF16,
                              addr_space="Shared")]

    with tile.TileContext(nc) as tc:
        nc.gpsimd.load_library(mlp_lib)

        # ---------- consts / pads ----------
        with tc.tile_pool(name="konst", bufs=1) as kp:
            w1sb = kp.tile([128, 132], BF16)
            nc.sync.dma_start(out=w1sb[:], in_=Waug1[:, :])
            w2sb = kp.tile([32, 68], BF16)
            nc.sync.dma_start(out=w2sb[:], in_=Waug2[:, :])
            wad1sb = kp.tile([128, 4], BF16)
            nc.sync.dma_start(out=wad1sb[:], in_=Wad1[:, :])
            wad2sb = kp.tile([32, 4], BF16)
            nc.sync.dma_start(out=wad2sb[:], in_=Wad2[:, :])
            # edge/merge index streams, loaded up front so the edge pipelines
            # can start as soon as their table chunks are written
            SW = stream_len // 16
            AW = sum(grid_rows) // 16
            MW = (NCH * NSHP) // 16
            sidx1 = kp.tile([128, SW], I16)
            nc.sync.dma_start(out=sidx1[:], in_=slot1w[:, :])
            sidx2 = kp.tile([128, SW], I16)
            nc.sync.dma_start(out=sidx2[:], in_=slot2w[:, :])
            aidx = kp.tile([128, AW], I16)
            nc.sync.dma_start(out=aidx[:], in_=adw[:, :])
            midx = kp.tile([128, MW], I16)
            nc.sync.dma_start(out=midx[:], in_=mrgw[:, :])
            b1sb = kp.tile([128, HID], F32)
            nc.sync.dma_start(out=b1sb[:], in_=b1rep[:, :])
            b2sb = kp.tile([128, OUT_DIM], F32)
            nc.sync.dma_start(out=b2sb[:], in_=b2rep[:, :])
            for k in range(NCH):
                nc.sync.dma_start(out=table1[k][CH1, :], in_=pad1[0, :])
                nc.sync.dma_start(out=table2[k][CH2, :], in_=pad2[0, :])
                nc.sync.dma_start(out=stg1[k][grid_rows[k], :], in_=z256[0, :])
                nc.sync.dma_start(out=stg2[k][grid_rows[k], :], in_=z256[0, :128])

            # ---------- ad1 pre-pass: own-shard x @ Wad1 -> ad1c ----------
            pid = nc.sync.partition_id()
            with (nc.named_scope("ad1pass"),
                  tc.tile_pool(name="a1", bufs=2) as apool,
                  tc.tile_pool(name="a1p", bufs=2, space="PSUM") as aps):
                ADB = 16
                base = pid * NSH
                b0 = 0
                while b0 < NT:
                    nb = min(ADB, NT - b0)
                    ncol = min(nb * 128, NSH - b0 * 128)
                    xmA = apool.tile([128, ADB * 128], BF16, tag="xmA")
                    nc.sync.dma_start(
                        out=xmA[:, 0:ncol],
                        in_=xT[:, bass.ds(base + b0 * 128, ncol)])
                    psA = aps.tile([128, ADB, 4], F32, tag="psA")
                    if ncol < nb * 128:
                        nc.vector.memset(psA[:], 0.0)
                    for t in range(nb):
                        nn = min(128, ncol - t * 128)
                        if nn <= 0:
                            break
                        nc.tensor.matmul(
                            out=bass.AP(psA[:].tensor, psA[:].offset + t * 4,
                                        [[list(psA[:].ap[0])[0], nn], [1, 4]]),
                            lhsT=xmA[:, t * 128:t * 128 + nn],
                            rhs=wad1sb[:], start=True, stop=True)
                    adt = apool.tile([128, ADB * 4], F32, tag="adt")
                    nc.vector.tensor_copy(out=adt[:, 0:nb * 4],
                                          in_=psA[:, 0:nb, :])
                    nc.sync.dma_start(
                        out=bass.AP(ad1c[:, :].tensor, b0 * 128 * 64,
                                    [[64, 128], [64 * 128, nb], [1, 4]]),
                        in_=vap(adt, 0, [[4, nb], [1, 4]]))
                    b0 += nb

            # ---------- dense1 + edge1 (pools coexist so both overlap) ----
            with (tc.tile_pool(name="eg1", bufs=3) as gp1,
                  tc.tile_pool(name="ea1", bufs=2) as ap1,
                  tc.tile_pool(name="eso1", bufs=2) as sop1,
                  tc.tile_pool(name="ew1", bufs=3) as wp1):
                with (nc.named_scope("dense1"),
                      tc.tile_pool(name="d1", bufs=3) as dpool,
                      tc.tile_pool(name="d1b", bufs=2) as bpool,
                      tc.tile_pool(name="d1p", bufs=2, space="PSUM") as dps):
                    _dense_pass(nc, tc, dpool, bpool, dps, layer=1,
                                src=xT, wsb=w1sb, table=table1, h1T_all=None)

                # ---------- edge pass L1 ----------
                with nc.named_scope("edge1"):
                    _edge_pass(nc, tc, meta, layer=1, sidx=sidx1, aidx=aidx,
                               table=table1, stg=stg1, ad_core=ad1c,
                               pools=(gp1, ap1, sop1, wp1))

            # ---------- merge1 + dense2 + edge2 (L2 pools open early so
            # dense2 need not wait for merge1's pool region to free) ------
            with (tc.tile_pool(name="eg2", bufs=3) as gp2,
                  tc.tile_pool(name="ea2", bufs=2) as ap2,
                  tc.tile_pool(name="eso2", bufs=2) as sop2,
                  tc.tile_pool(name="ew2", bufs=3) as wp2,
                  tc.tile_pool(name="d2", bufs=3) as dpool2,
                  tc.tile_pool(name="d2b", bufs=2) as bpool2,
                  tc.tile_pool(name="d2p", bufs=4, space="PSUM") as dps2):
                with nc.named_scope("merge1"):
                    _merge_pass(nc, tc, meta, layer=1, midx=midx, stg=stg1,
                                bsb=b1sb, out2=None, h1T_sh=h1T_sh,
                                wadsb=wad2sb, adc=ad2c, nta=NTA,
                                h1T_all=h1T_all)

                with nc.named_scope("dense2"):
                    _dense_pass(nc, tc, dpool2, bpool2, dps2, layer=2,
                                src=None, wsb=w2sb, table=table2,
                                h1T_all=h1T_all)

                # ---------- edge pass L2 ----------
                with nc.named_scope("edge2"):
                    _edge_pass(nc, tc, meta, layer=2, sidx=sidx2, aidx=aidx,
                               table=table2, stg=stg2, ad_core=ad2c,
                               pools=(gp2, ap2, sop2, wp2))

            # ---------- merge L2 -> out2 ----------
            with nc.named_scope("merge2"):
                _merge_pass(nc, tc, meta, layer=2, midx=midx, stg=stg2,
                            bsb=b2sb, out2=out2, h1T_sh=None,
                            wadsb=None, adc=None)

    nc.finalize()
    return nc


def _dense_pass(nc, tc, dpool, bpool, dps, layer, src, wsb, table, h1T_all):
    """Replicated dense phase: DB-tile batches, PG-tile PSUM groups.
    layer 1: in xT f32 [128, N] -> table1 rows [h(128)|as_hi(4)|as_lo(4)] bf16.
    layer 2: in h1T_all bf16 -> table2 rows [h(64)|as_hi|as_lo] bf16.
    """
    if layer == 1:
        ntiles, K, MC = (N + 127) // 128, 128, 132   # matmul out cols
        FD = 128
        CHN, Ntot = CH1, N
        PG, PST = 4, 512       # PSUM group; slot stride padded to a full bank
    else:
        ntiles, K, MC = R2 // 128, 32, 68
        FD = 64
        CHN, Ntot = CH2, R2
        PG, PST = 7, 68        # 7 x 272B fits one bank
    RW = 256 if layer == 1 else 128

    b0 = 0
    eng_i = 0
    while b0 < ntiles:
        nb = min(DB, ntiles - b0)
        n0 = b0 * 128
        # ---- batched input load ----
        xm = dpool.tile([K, DB * 128], BF16, tag="xm")
        if layer == 1:
            nn = min(nb * 128, N - n0)
            nc.sync.dma_start(out=xm[:, 0:nn], in_=bass.AP(
                src[:, :].tensor, n0, [[N, K], [1, nn]]))
        else:
            # h1T_all halves [NCORES, 32, NHA/(NSHP-NHA)]; split loads at
            # core and half boundaries
            q = b0
            col = 0
            while q < b0 + nb:
                cc, tt = q // NT, q % NT
                if tt < NTA:
                    hf, tb, hw = 0, 0, NHA
                else:
                    hf, tb, hw = 1, NTA, NSHP - NHA
                run = min((NTA if tt < NTA else NT) - tt, b0 + nb - q)
                nc.sync.dma_start(
                    out=xm[:, col * 128:(col + run) * 128],
                    in_=bass.AP(h1T_all[hf][:, :, :].tensor,
                                cc * 32 * hw + (tt - tb) * 128,
                                [[hw, 32], [1, run * 128]]))
                q += run
                col += run
        # ---- batch output tiles (row = [h | as]) ----
        hrowB = bpool.tile([128, DB, FD + 8], BF16, tag="hrowB")
        g0 = 0
        while g0 < nb:
            ng = min(PG, nb - g0)
            ps = dps.tile([128, PG, PST], F32, tag="ps")
            partial = (n0 + (g0 + ng) * 128) > Ntot
            if partial:
                nc.vector.memset(ps[:], 0.0)
            for t in range(ng):
                tt = g0 + t
                nn = min(128, Ntot - (n0 + tt * 128))
                nc.tensor.matmul(
                    out=bass.AP(ps[:].tensor, ps[:].offset + (t * PST),
                                [[list(ps[:].ap[0])[0], nn], [1, MC]]),
                    lhsT=xm[:, tt * 128:tt * 128 + nn],
                    rhs=wsb[:], start=True, stop=True)
            use_act = (eng_i % 2 == 1)
            eng_i += 1

            def _copy(out, in_):
                if use_act:
                    nc.scalar.activation(out=out, in_=in_, func=ActFn.Copy)
                else:
                    nc.vector.tensor_copy(out=out, in_=in_)

            # bulk copy [h | as_hi] (+ leave as_lo slot) per PSUM group
            _copy(vap(hrowB, g0 * (FD + 8), [[FD + 8, ng], [1, FD + 4]]),
                  vap(ps, 0, [[PST, ng], [1, FD + 4]]))
            g0 += ng
        # table rows per chunk tensor; split at chunk boundary
        t0 = 0
        while t0 < nb:
            gn0 = n0 + t0 * 128
            rows = min(128, Ntot - gn0)
            k = gn0 // CHN
            avail = (k + 1) * CHN - gn0
            if avail >= rows:
                if rows == 128:
                    run = min(nb - t0, avail // 128)
                else:
                    run = 1
                r0 = gn0 - k * CHN
                nc.sync.dma_start(
                    out=bass.AP(table[k][:, :].tensor, r0 * RW,
                                [[RW, rows], [RW * 128 if run > 1 else 1, run],
                                 [1, FD + 4]])
                    if run > 1 else
                    bass.AP(table[k][:, :].tensor, r0 * RW,
                            [[RW, rows], [1, FD + 4]]),
                    in_=hrowB[:, t0:t0 + run, 0:FD + 4] if run > 1
                    else hrowB[0:rows, t0, 0:FD + 4])
                t0 += run
            else:
                # tile straddles the chunk boundary: split by partition range
                nsplit = avail
                r0 = gn0 - k * CHN
                nc.sync.dma_start(
                    out=bass.AP(table[k][:, :].tensor, r0 * RW,
                                [[RW, nsplit], [1, FD + 4]]),
                    in_=hrowB[0:nsplit, t0, 0:FD + 4])
                nc.sync.dma_start(
                    out=bass.AP(table[k + 1][:, :].tensor, 0,
                                [[RW, rows - nsplit], [1, FD + 4]]),
                    in_=hrowB[nsplit:rows, t0, 0:FD + 4])
                t0 += 1
        b0 += nb


def _edge_pass(nc, tc, meta, layer, sidx, aidx, table, stg, ad_core, pools):
    rowbase = meta["rowbase"]
    grid_rows = meta["grid_rows"]
    RW = 256 if layer == 1 else 128       # table row elems (bf16)
    FD = 128 if layer == 1 else 64        # feature elems

    # stream offset of each bucket's first slot (buckets are contiguous)
    bstart = {}
    for (k, bi, t, c0, w, off) in meta["calls"]:
        bstart.setdefault((k, bi), off)

    gp, ap_pool, sop, wp = pools
    if True:
        abase = 0
        for k in range(NCH):
            for bi, D in enumerate(BUCKETS):
                Tb = int(meta["T"][k][bi])
                if Tb == 0:
                    continue
                rb0 = rowbase[(k, bi, 0)]
                # per-bucket ad gather (<=2048-idx calls)
                ADG = ap_pool.tile([128, Tb, 64], F32, tag="ADG")
                na = Tb * 128
                o = 0
                while o < na:
                    nbv = min(MAXD, na - o)
                    nc.gpsimd.dma_gather(
                        ADG[:, o // 128:(o + nbv) // 128, :], ad_core[:, :],
                        aidx[:, (abase + rb0 + o) // 16:
                                (abase + rb0 + o + nbv) // 16],
                        nbv, nbv, 64, queue_num=qn())
                    o += nbv
                # per-bucket f32 accumulator + bf16 staging copy
                fsB = sop.tile([128, Tb, FD + 4], F32, tag="fsB")
                soB = sop.tile([128, Tb, FD + 4], BF16, tag="soB")
                # segments of <= SEG cols (tile-aligned)
                gt = max(1, SEG // D)      # tiles per segment
                t0 = 0
                off = bstart[(k, bi)]
                while t0 < Tb:
                    gn = min(gt, Tb - t0)
                    ncols = gn * D
                    G = gp.tile([128, max(SEG, D), RW], BF16, tag="G")
                    so = off + 128 * (t0 * D)
                    c = 0
                    while c < ncols:
                        w = min(SEG, ncols - c)
                        nc.gpsimd.dma_gather(
                            G[:, c:c + w, :], table[k][:, :],
                            sidx[:, (so + 128 * c) // 16:
                                    (so + 128 * (c + w)) // 16],
                            128 * w, 128 * w, RW, queue_num=qn())
                        c += w
                    # e = exp(leaky(as_hi + as_lo + ad))  [f32]
                    e = wp.tile([128, max(SEG, D) * 4], F32, tag="e")
                    nc.vector.tensor_tensor(
                        out=e[:, 0:ncols * 4],
                        in0=vap(G, FD, [[RW, ncols], [1, 4]]),
                        in1=bass.AP(ADG[:].tensor,
                                    ADG[:].offset + t0 * 64,
                                    [list(ADG[:].ap[0]), [64, gn], [0, D],
                                     [1, 4]]),
                        op=AluOp.add)
                    nc.vector.scalar_tensor_tensor(
                        out=e[:, 0:ncols * 4], in0=e[:, 0:ncols * 4],
                        scalar=0.2, in1=e[:, 0:ncols * 4],
                        op0=AluOp.mult, op1=AluOp.max)
                    # exp twice on ACT: f32 (denominators) + bf16 (weights)
                    ebf = wp.tile([128, max(SEG, D) * 4], BF16, tag="ebf")
                    nc.scalar.activation(out=ebf[:, 0:ncols * 4],
                                         in_=e[:, 0:ncols * 4], func=ActFn.Exp)
                    nc.scalar.activation(out=e[:, 0:ncols * 4],
                                         in_=e[:, 0:ncols * 4], func=ActFn.Exp)
                    # denominators (f32 accumulate)
                    nc.vector.tensor_reduce(
                        out=bass.AP(fsB[:].tensor,
                                    fsB[:].offset + t0 * (FD + 4) + FD,
                                    [list(fsB[:].ap[0]), [FD + 4, gn],
                                     [1, 4]]),
                        in_=vap(e, 0, [[4 * D, gn], [1, 4], [4, D]]),
                        axis=Axis.X, op=AluOp.add)
                    # segment-fused weighted features
                    val = wp.tile([128, max(SEG, D) * FD], BF16, tag="val")
                    nc.vector.tensor_tensor(
                        out=vap(val, 0, [[D * FD, gn], [FD, D],
                                         [FD // 4, 4], [1, FD // 4]]),
                        in0=vap(G, 0, [[RW * D, gn], [RW, D],
                                       [FD // 4, 4], [1, FD // 4]]),
                        in1=vap(ebf, 0, [[4 * D, gn], [4, D],
                                         [1, 4], [0, FD // 4]]),
                        op=AluOp.mult)
                    nc.vector.tensor_reduce(
                        out=bass.AP(fsB[:].tensor,
                                    fsB[:].offset + t0 * (FD + 4),
                                    [list(fsB[:].ap[0]), [FD + 4, gn],
                                     [1, FD]]),
                        in_=vap(val, 0, [[D * FD, gn], [1, FD], [FD, D]]),
                        axis=Axis.X, op=AluOp.add)
                    t0 += gn
                # one bf16 round + one staging write per bucket
                nc.vector.tensor_copy(out=soB[:], in_=fsB[:])
                nc.sync.dma_start(
                    out=bass.AP(stg[k][:, :].tensor, rb0 * RW,
                                [[RW, 128], [RW * 128, Tb], [1, FD + 4]]),
                    in_=soB[:])
            abase += grid_rows[k]


def _merge_pass(nc, tc, meta, layer, midx, stg, bsb, out2, h1T_sh,
                wadsb, adc, nta=None, h1T_all=None):
    RW = 256 if layer == 1 else 128
    FD = 128 if layer == 1 else 64
    OD = HID if layer == 1 else OUT_DIM
    W = FD + 4
    if layer == 1:
        ranges = [(0, nta, 0), (nta, NT, 1)]
    else:
        ranges = [(0, NT, 0)]

    with (tc.tile_pool(name=f"mi{layer}", bufs=1) as ip,
          tc.tile_pool(name=f"mg{layer}", bufs=2) as gp,
          tc.tile_pool(name=f"ms{layer}", bufs=2) as sp_pool,
          tc.tile_pool(name=f"mw{layer}", bufs=2) as wp,
          tc.tile_pool(name=f"mp{layer}", bufs=2, space="PSUM") as pp):
        if layer == 1:
            from concourse.masks import make_identity
            ident = ip.tile([128, 128], F32, tag="ident")
            make_identity(nc, ident[:])

        for (t_lo, t_hi, hf) in ranges:
            _merge_range(nc, meta, layer, midx, stg, bsb, out2,
                         h1T_sh[hf] if layer == 1 else None,
                         wadsb, adc, gp, sp_pool, wp, pp,
                         ident if layer == 1 else None,
                         t_lo, t_hi, RW, FD, OD, W)
        if layer == 1:
            for hf in range(2):
                nc.gpsimd.collective_compute(
                    "AllGather", AluOp.bypass,
                    replica_groups=[list(range(NCORES))],
                    ins=[h1T_sh[hf][:, :]], outs=[h1T_all[hf][:, :, :]])


def _merge_range(nc, meta, layer, midx, stg, bsb, out2, h1T_sh, wadsb, adc,
                 gp, sp_pool, wp, pp, ident, t_lo, t_hi, RW, FD, OD, W):
        mt = t_lo
        while mt < t_hi:
            nb = min(MB, t_hi - mt)
            s = sp_pool.tile([128, MB * W], F32, tag="s")
            s01 = wp.tile([128, MB * W], BF16, tag="s01")
            Gprev = None
            for k in range(NCH):
                Gk = gp.tile([128, MB, RW], BF16, tag="MG")
                ioff = k * NSHP + mt * 128
                o = 0
                while o < nb * 128:
                    nbv = min(MAXD, nb * 128 - o)
                    nc.gpsimd.dma_gather(
                        Gk[:, o // 128:(o + nbv) // 128, :], stg[k][:, :],
                        midx[:, (ioff + o) // 16:(ioff + o + nbv) // 16],
                        nbv, nbv, RW, queue_num=qn())
                    o += nbv
                if k == 1:
                    # bf16 pair-add runs in the DVE 2x fast mode
                    with nc.allow_low_precision(reason="bf16 staged pair"):
                        nc.vector.tensor_tensor(
                            out=s01[:, 0:nb * W],
                            in0=vap(Gprev, 0, [[RW, nb], [1, W]]),
                            in1=vap(Gk, 0, [[RW, nb], [1, W]]), op=AluOp.add)
                elif k == 2:
                    nc.vector.tensor_tensor(
                        out=vap(s, 0, [[W, nb], [1, W]]),
                        in0=s01[:, 0:nb * W],
                        in1=vap(Gk, 0, [[RW, nb], [1, W]]), op=AluOp.add)
                elif k == 3:
                    nc.vector.tensor_tensor(
                        out=vap(s, 0, [[W, nb], [1, W]]),
                        in0=vap(s, 0, [[W, nb], [1, W]]),
                        in1=vap(Gk, 0, [[RW, nb], [1, W]]), op=AluOp.add)
                Gprev = Gk
            rec = wp.tile([128, MB * 4], F32, tag="rec")
            nc.vector.tensor_scalar_add(
                out=vap(rec, 0, [[4, nb], [1, 4]]),
                in0=vap(s, FD, [[W, nb], [1, 4]]), scalar1=EPS)
            nc.vector.reciprocal(out=rec[:, 0:nb * 4], in_=rec[:, 0:nb * 4])
            nc.vector.tensor_scalar_mul(out=rec[:, 0:nb * 4],
                                        in0=rec[:, 0:nb * 4], scalar1=0.25)
            sc = wp.tile([128, MB * FD], F32, tag="sc")
            nc.vector.tensor_tensor(
                out=vap(sc, 0, [[FD, nb], [FD // 4, 4], [1, FD // 4]]),
                in0=vap(s, 0, [[W, nb], [FD // 4, 4], [1, FD // 4]]),
                in1=vap(rec, 0, [[4, nb], [1, 4], [0, FD // 4]]),
                op=AluOp.mult)
            hs = wp.tile([128, MB * OD], F32, tag="hs")
            nc.vector.tensor_reduce(
                out=vap(hs, 0, [[OD, nb], [1, OD]]),
                in_=vap(sc, 0, [[FD, nb], [1, OD], [OD, 4]]),
                axis=Axis.X, op=AluOp.add)
            nc.vector.tensor_tensor(
                out=vap(hs, 0, [[OD, nb], [1, OD]]),
                in0=vap(hs, 0, [[OD, nb], [1, OD]]),
                in1=vap(bsb, 0, [[0, nb], [1, OD]]), op=AluOp.add)
            if layer == 1:
                nc.scalar.activation(out=hs[:, 0:nb * OD], in_=hs[:, 0:nb * OD],
                                     func=ActFn.Relu)
                hsbB = wp.tile([32, MB * 128], BF16, tag="hsbB")
                ti = 0
                while ti < nb:
                    jn = min(4, nb - ti)
                    psT = pp.tile([32, 4, 128], F32, tag="psT")
                    for j in range(jn):
                        nc.tensor.transpose(
                            out=psT[:, j, :],
                            in_=hs[:, (ti + j) * OD:(ti + j + 1) * OD],
                            identity=ident[:])
                    nc.vector.tensor_copy(
                        out=hsbB[:, ti * 128:(ti + jn) * 128],
                        in_=psT[:, 0:jn, :])
                    ti += jn
                nc.scalar.dma_start(
                    out=h1T_sh[:, (mt - t_lo) * 128:(mt - t_lo + nb) * 128],
                    in_=hsbB[:, 0:nb * 128])
                # ad2 for next layer: h1 @ Wad2, straight into ad2c
                psA = pp.tile([128, MB, 4], F32, tag="psA2")
                for ti in range(nb):
                    nc.tensor.matmul(
                        out=bass.AP(psA[:].tensor, psA[:].offset + ti * 4,
                                    [[list(psA[:].ap[0])[0], 128], [1, 4]]),
                        lhsT=hsbB[:, ti * 128:(ti + 1) * 128],
                        rhs=wadsb[:], start=True, stop=True)
                adt = wp.tile([128, MB * 4], F32, tag="adt2")
                nc.vector.tensor_copy(out=adt[:, 0:nb * 4],
                                      in_=psA[:, 0:nb, :])
                nc.scalar.dma_start(
                    out=bass.AP(adc[:, :].tensor, mt * 128 * 64,
                                [[64, 128], [64 * 128, nb], [1, 4]]),
                    in_=vap(adt, 0, [[4, nb], [1, 4]]))
            else:
                nc.sync.dma_start(
                    out=bass.AP(out2[:, :].tensor, mt * 128 * OD,
                                [[OD, 128], [OD * 128, nb], [1, OD]]),
                    in_=vap(hs, 0, [[OD, nb], [1, OD]]))
            mt += nb


_CACHE = {}


def kernel(**inputs):
    in_maps, meta = host_prep(**inputs)
    key = str(meta["T"])
    _CACHE["k"] = key
    if key not in _CACHE:
        nc = build_nc(meta)
        _CACHE[key] = (nc, make_runner(nc, NCORES))
    nc, run = _CACHE[key]
    results, best = run(in_maps, repeats=1)
    _CACHE["last_time"] = best
    out = np.empty((N, OUT_DIM), np.float32)
    for c in range(NCORES):
        out[c * NSH:(c + 1) * NSH] = results[c]["out2"][:NSH]
    return out



# revision 21
# speedup vs baseline: 1.5133x; 1.2861x over previous
"""GAT 2-layer kernel for TRN2, 8 NeuronCores (self-contained).

Strategy:
- dst-shard: core c owns nodes [c*12500, (c+1)*12500).
- ad pre-pass: own-shard x @ fold(W, a_dst) -> ad1c, so edge pipelines
  never wait on the full dense pass.
- Dense phases (x@W bf16, h1@W2 bf16) replicated on all cores; folded
  a_src gives per-node [h | as] rows in one matmul. 16-tile DMA batches
  (HWDGE calls are ~600ns each on one shared device), bank-sized PSUM
  groups, copy work split across DVE/ACT.
- Edge phase per core: 4 src-chunks (25000 nodes, int16 gather reach),
  per-chunk degree-bucketed padded CSR over dst. 1024-index single-queue
  SWDGE gathers (multi-queue under TileContext races; >16KB rings hang
  HW) pull [h | as] rows. e = exp(leaky(as+ad)) on ACT; segment-fused
  bf16 weighted-feature mult + f32 reduce into per-bucket tiles.
- Per-chunk partials [featsum | denom] -> DRAM staging (bf16); per-chunk
  table tensors + concurrently-open tile pools let edge chunks overlap
  the dense tail.
- Merge pass (natural node order): per-chunk 16-tile gathers, f32
  accumulate, per-head normalize, bias (+relu L1); L1 also computes
  ad2c (h1 @ fold(W2, a2_dst)) from the transposed tiles.
- L1->L2: h1T shard AllGather in two halves (overlaps merge tail and
  dense2 head) -> replicated dense2.
"""
import sys
sys.path.insert(0, "/opt/trn_rl_repo")
import numpy as np
import ml_dtypes

import concourse.bass as bass
import concourse.bacc as bacc
import concourse.tile as tile
from concourse import mybir
from concourse.library_config import mlp as mlp_lib


def make_runner(nc, n_cores):
    """PJRT runner: returns run_fn(in_maps, repeats) -> (results, best_time_s)."""
    import time
    import jax
    from jax.sharding import Mesh, PartitionSpec, NamedSharding
    from jax.experimental.shard_map import shard_map
    from concourse.bass2jax import (_bass_exec_p, install_neuronx_cc_hook,
                                    partition_id_tensor)
    install_neuronx_cc_hook()
    partition_name = nc.partition_id_tensor.name if nc.partition_id_tensor else None
    in_names, out_names, out_avals, zero_outs = [], [], [], []
    for alloc in nc.m.functions[0].allocations:
        if not isinstance(alloc, mybir.MemoryLocationSet):
            continue
        if not alloc.memorylocations:
            continue
        name = alloc.memorylocations[0].name
        if alloc.kind == "ExternalInput":
            if name != partition_name:
                in_names.append(name)
        elif alloc.kind == "ExternalOutput":
            out_names.append(name)
            shape = tuple(alloc.tensor_shape)
            dtype = mybir.dt.np(alloc.dtype)
            out_avals.append(jax.core.ShapedArray(shape, dtype))
            zero_outs.append(np.zeros(shape, dtype))
    n_params = len(in_names)
    n_outs = len(out_avals)
    all_in_names = list(in_names) + list(out_names)
    if partition_name is not None:
        all_in_names.append(partition_name)

    def _body(*args):
        operands = list(args)
        if partition_name is not None:
            operands.append(partition_id_tensor())
        return tuple(_bass_exec_p.bind(
            *operands, out_avals=tuple(out_avals), in_names=tuple(all_in_names),
            out_names=tuple(out_names), lowering_input_output_aliases=(),
            sim_require_finite=False, sim_require_nnan=False, nc=nc))

    devices = jax.devices()[:n_cores]
    mesh = Mesh(np.asarray(devices), ("core",))
    in_specs = (PartitionSpec("core"),) * (n_params + n_outs)
    out_specs = (PartitionSpec("core"),) * n_outs
    donate = tuple(range(n_params, n_params + n_outs))
    sharded = jax.jit(
        shard_map(_body, mesh=mesh, in_specs=in_specs, out_specs=out_specs,
                  check_rep=False),
        donate_argnums=donate, keep_unused=True)

    def run_fn(in_maps, repeats=1):
        per_core = [[np.asarray(m[name]) for name in in_names] for m in in_maps]
        concat_in = [np.concatenate([per_core[c][i] for c in range(n_cores)], 0)
                     for i in range(n_params)]
        sharding = NamedSharding(mesh, PartitionSpec("core"))
        dev_in = [jax.device_put(a, sharding) for a in concat_in]
        for a in dev_in:
            a.block_until_ready()
        times, out_arrs = [], None
        for _ in range(repeats):
            concat_zeros = [jax.device_put(
                np.zeros((n_cores * z.shape[0], *z.shape[1:]), z.dtype), sharding)
                for z in zero_outs]
            for z in concat_zeros:
                z.block_until_ready()
            t0 = time.perf_counter()
            out_arrs = sharded(*dev_in, *concat_zeros)
            for o in out_arrs:
                o.block_until_ready()
            times.append(time.perf_counter() - t0)
        results = [
            {name: np.asarray(out_arrs[i]).reshape(n_cores, *out_avals[i].shape)[c]
             for i, name in enumerate(out_names)}
            for c in range(n_cores)]
        return results, min(times)

    return run_fn

F32 = mybir.dt.float32
BF16 = mybir.dt.bfloat16
I16 = mybir.dt.int16
BF = ml_dtypes.bfloat16

NCORES = 8
N = 100000
IN_DIM = 128
HID = 32
OUT_DIM = 16
NSH = N // NCORES            # 12500
NT = 98                      # merge tiles per core
NSHP = NT * 128              # 12544
CH1 = 25000                  # table1 nodes per chunk
NCH = 4
CH1R = CH1 + 1               # +pad row
R2 = NCORES * NSHP           # 100352
CH2 = R2 // NCH              # 25088 (= 2 padded cores)
CH2R = CH2 + 1
BUCKETS = (1, 2, 3, 4, 5, 6, 8, 16)
NEG = -1.0e30
EPS = 1e-16
MAXD = 1024                  # max descriptors per SWDGE call (HW ring limit)
RING = 16384                 # SWDGE descriptor ring bytes (>16KB hangs HW)
NQ = 4                       # SWDGE queues (measured: 1q=92GB/s, 4q=450GB/s)
_QRR = [0]


def qn():
    _QRR[0] = (_QRR[0] + 1) % NQ
    return _QRR[0]
SEG = MAXD // 128            # max gather cols per SWDGE call
CAP = 32                     # edge-pass segment cols (DVE op granularity)
DB = 32                      # dense batch (tiles per DMA)
PG = 3                       # dense PSUM group (tiles per PSUM bank tile)
MB = 8                       # merge batch (tiles)
NTA = 49                     # merge tiles in first allgather half
NHA = NTA * 128              # 6272

AluOp = mybir.AluOpType
ActFn = mybir.ActivationFunctionType
Axis = mybir.AxisListType


def _colgroups(D):
    out = []
    c = 0
    while c < D:
        w = min(8, D - c)
        out.append((c, w))
        c += w
    return out


def plan_segments(T):
    """Host/device shared slot-stream layout (order: k, bucket, tile, col, p)."""
    calls = []
    rowbase = {}
    grid_rows = []
    off = 0
    for k in range(NCH):
        rb = 0
        for bi, D in enumerate(BUCKETS):
            for t in range(int(T[k][bi])):
                rowbase[(k, bi, t)] = rb
                for (c0, w) in _colgroups(D):
                    calls.append((k, bi, t, c0, w, off))
                    off += 128 * w
                rb += 128
        grid_rows.append(rb)
    return calls, rowbase, grid_rows, off


def _wrap_idx(flat):
    n = len(flat)
    assert n % 16 == 0
    w = np.asarray(flat, np.int16).reshape(n // 16, 16).T
    return np.ascontiguousarray(np.tile(w, (8, 1)))


def fold(W, a):
    Hh, F = a.shape
    w = np.zeros((W.shape[0], Hh), np.float32)
    for h in range(Hh):
        w[:, h] = W[:, h * F:(h + 1) * F] @ a[h]
    return w


def host_prep(x, edge_index, W1, a1_src, a1_dst, b1, W2, a2_src, a2_dst, b2):
    x = np.asarray(x, np.float32)
    ei = np.asarray(edge_index)
    src = ei[0].astype(np.int64)
    dst = ei[1].astype(np.int64)
    W1 = np.asarray(W1, np.float32)
    W2 = np.asarray(W2, np.float32)
    Waug1 = np.concatenate([W1, fold(W1, np.asarray(a1_src, np.float32))], 1)
    Waug2 = np.concatenate([W2, fold(W2, np.asarray(a2_src, np.float32))], 1)
    Wad1 = fold(W1, np.asarray(a1_dst, np.float32))          # [128, 4]
    Wad2 = fold(W2, np.asarray(a2_dst, np.float32))          # [32, 4]
    xT = np.ascontiguousarray(x.T.astype(BF))

    core_of = dst // NSH
    # ---- per-core, per-chunk CSR ----
    pc = []  # [core][chunk] = (deg, sorted_src_by_dst, starts)
    for c in range(NCORES):
        m = core_of == c
        s_c, d_c = src[m], dst[m] - c * NSH
        ch = s_c // CH1
        info = []
        for k in range(NCH):
            mk = ch == k
            sk, dk = s_c[mk], d_c[mk]
            deg = np.bincount(dk, minlength=NSH)
            order = np.argsort(dk, kind="stable")
            sk = sk[order]
            starts = np.zeros(NSH + 1, np.int64)
            np.cumsum(deg, out=starts[1:])
            info.append((deg, sk, starts))
        pc.append(info)

    # shared tile counts
    T = [[0] * len(BUCKETS) for _ in range(NCH)]
    for c in range(NCORES):
        for k in range(NCH):
            deg = pc[c][k][0]
            for bi, D in enumerate(BUCKETS):
                lo = BUCKETS[bi - 1] if bi else 0
                nb = int(((deg > lo) & (deg <= D)).sum())
                T[k][bi] = max(T[k][bi], (nb + 127) // 128)
            assert deg.max(initial=0) <= BUCKETS[-1], f"deg max {deg.max()}"
    calls, rowbase, grid_rows, stream_len = plan_segments(T)

    b1rep = np.tile(np.asarray(b1, np.float32)[None, :], (128, 1))
    b2rep = np.tile(np.asarray(b2, np.float32)[None, :], (128, 1))
    pad1 = np.zeros((1, 256), BF); pad1[0, 128:132] = NEG
    pad2 = np.zeros((1, 256), BF); pad2[0, 64:68] = NEG
    z256 = np.zeros((1, 256), BF)

    in_maps = []
    for c in range(NCORES):
        slot_nodes = []   # per chunk: grid row -> node (or -1)
        for k in range(NCH):
            gr = grid_rows[k]
            deg, sk, starts = pc[c][k]
            nodes_of = np.full(gr, -1, np.int64)
            for bi, D in enumerate(BUCKETS):
                lo = BUCKETS[bi - 1] if bi else 0
                nd = np.where((deg > lo) & (deg <= D))[0]
                rb = rowbase[(k, bi, 0)] if T[k][bi] else 0
                nodes_of[rb:rb + len(nd)] = nd
            slot_nodes.append(nodes_of)

        s1 = np.full(stream_len, CH1, np.int64)     # pad -> table1 chunk pad row
        s2 = np.full(stream_len, CH2, np.int64)     # pad -> table2 chunk pad row
        for (k, bi, t, c0, w, off) in calls:
            D = BUCKETS[bi]
            rb = rowbase[(k, bi, t)]
            deg, sk, starts = pc[c][k]
            nodes = slot_nodes[k][rb:rb + 128]
            j = off
            for d in range(c0, c0 + w):
                for p in range(128):
                    nd = nodes[p]
                    if nd >= 0 and d < starts[nd + 1] - starts[nd]:
                        s = sk[starts[nd] + d]
                        s1[j] = s % CH1
                        s2[j] = (s // NSH % 2) * NSHP + s % NSH
                    j += 1
        slot1w = _wrap_idx(s1)
        slot2w = _wrap_idx(s2)

        # ad idx: per (k, gridtile) 128 local dst ids (pad -> 0)
        adix = []
        for k in range(NCH):
            nd = slot_nodes[k]
            adix.append(np.where(nd >= 0, nd, 0))
        adw = _wrap_idx(np.concatenate(adix)) if stream_len else None

        # merge idx: per chunk, per natural node (padded to NSHP): grid row or zero-row
        mrg = []
        for k in range(NCH):
            deg = pc[c][k][0]
            pos = np.full(NSHP, grid_rows[k], np.int64)  # zero row
            nd = slot_nodes[k]
            real = nd >= 0
            pos[nd[real]] = np.nonzero(real)[0]
            mrg.append(pos)
        mrgw = _wrap_idx(np.concatenate(mrg))

        in_maps.append(dict(
            xT=xT, Waug1=Waug1.astype(BF), Waug2=Waug2.astype(BF),
            Wad1=Wad1.astype(BF), Wad2=Wad2.astype(BF),
            b1rep=b1rep, b2rep=b2rep, pad1=pad1, pad2=pad2, z256=z256,
            slot1w=slot1w, slot2w=slot2w, adw=adw, mrgw=mrgw,
        ))
    meta = dict(T=T, calls=calls, rowbase=rowbase, grid_rows=grid_rows,
                stream_len=stream_len)
    return in_maps, meta


def vap(t, off, dims):
    a = t[:]
    return bass.AP(a.tensor, a.offset + off, [list(a.ap[0])] + [list(d) for d in dims])


def build_nc(meta):
    _QRR[0] = 0
    T = meta["T"]
    grid_rows = meta["grid_rows"]
    stream_len = meta["stream_len"]
    SW = stream_len // 16
    ADL = sum(grid_rows)
    AW = ADL // 16
    MW = (NCH * NSHP) // 16

    nc = bacc.Bacc("TRN2", target_bir_lowering=False, num_swdge_queues=NQ,
                   dynamic_dma_scratch_size=RING)
    dp = nc.declare_dram_parameter
    xT = dp("xT", [IN_DIM, N], BF16, isOutput=False)
    Waug1 = dp("Waug1", [128, 132], BF16, isOutput=False)
    Waug2 = dp("Waug2", [32, 68], BF16, isOutput=False)
    Wad1 = dp("Wad1", [128, 4], BF16, isOutput=False)
    Wad2 = dp("Wad2", [32, 4], BF16, isOutput=False)
    b1rep = dp("b1rep", [128, HID], F32, isOutput=False)
    b2rep = dp("b2rep", [128, OUT_DIM], F32, isOutput=False)
    pad1 = dp("pad1", [1, 256], BF16, isOutput=False)
    pad2 = dp("pad2", [1, 256], BF16, isOutput=False)
    z256 = dp("z256", [1, 256], BF16, isOutput=False)
    slot1w = dp("slot1w", [128, SW], I16, isOutput=False)
    slot2w = dp("slot2w", [128, SW], I16, isOutput=False)
    adw = dp("adw", [128, AW], I16, isOutput=False)
    mrgw = dp("mrgw", [128, MW], I16, isOutput=False)
    out2 = dp("out2", [NSHP, OUT_DIM], F32, isOutput=True)

    table1 = [nc.dram_tensor(f"table1_{k}", [CH1R, 256], BF16)
              for k in range(NCH)]
    table2 = [nc.dram_tensor(f"table2_{k}", [CH2R, 256], BF16)
              for k in range(NCH)]
    ad1c = nc.dram_tensor("ad1c", [NSHP, 64], F32)
    ad2c = nc.dram_tensor("ad2c", [NSHP, 64], F32)
    stg1 = [nc.dram_tensor(f"stg1_{k}", [grid_rows[k] + 1, 256], BF16)
            for k in range(NCH)]
    stg2 = [nc.dram_tensor(f"stg2_{k}", [grid_rows[k] + 1, 256], BF16)
            for k in range(NCH)]
    h1T_sh = [nc.dram_tensor("h1T_shA", [32, NHA], BF16),
              nc.dram_tensor("h1T_shB", [32, NSHP - NHA], BF16)]
    h1T_all = [nc.dram_tensor("h1T_allA", [NCORES, 32, NHA], BF16,
                              addr_space="Shared"),
               nc.dram_tensor("h1T_allB", [NCORES, 32, NSHP - NHA], BF16,
                              addr_space="Shared")]

    with tile.TileContext(nc) as tc:
        nc.gpsimd.load_library(mlp_lib)

        # ---------- consts / pads ----------
        with tc.tile_pool(name="konst", bufs=1) as kp:
            w1sb = kp.tile([128, 132], BF16)
            nc.sync.dma_start(out=w1sb[:], in_=Waug1[:, :])
            w2sb = kp.tile([32, 68], BF16)
            nc.sync.dma_start(out=w2sb[:], in_=Waug2[:, :])
            wad1sb = kp.tile([128, 4], BF16)
            nc.sync.dma_start(out=wad1sb[:], in_=Wad1[:, :])
            wad2sb = kp.tile([32, 4], BF16)
            nc.sync.dma_start(out=wad2sb[:], in_=Wad2[:, :])
            # edge/merge index streams, loaded up front so the edge pipelines
            # can start as soon as their table chunks are written
            SW = stream_len // 16
            AW = sum(grid_rows) // 16
            MW = (NCH * NSHP) // 16
            sidx1 = kp.tile([128, SW], I16)
            nc.sync.dma_start(out=sidx1[:], in_=slot1w[:, :])
            sidx2 = kp.tile([128, SW], I16)
            nc.sync.dma_start(out=sidx2[:], in_=slot2w[:, :])
            aidx = kp.tile([128, AW], I16)
            nc.sync.dma_start(out=aidx[:], in_=adw[:, :])
            midx = kp.tile([128, MW], I16)
            nc.sync.dma_start(out=midx[:], in_=mrgw[:, :])
            b1sb = kp.tile([128, HID], F32)
            nc.sync.dma_start(out=b1sb[:], in_=b1rep[:, :])
            b2sb = kp.tile([128, OUT_DIM], F32)
            nc.sync.dma_start(out=b2sb[:], in_=b2rep[:, :])
            for k in range(NCH):
                nc.sync.dma_start(out=table1[k][CH1, :], in_=pad1[0, :])
                nc.sync.dma_start(out=table2[k][CH2, :], in_=pad2[0, :])
                nc.sync.dma_start(out=stg1[k][grid_rows[k], :], in_=z256[0, :])
                nc.sync.dma_start(out=stg2[k][grid_rows[k], :], in_=z256[0, :])

            # ---------- ad1 pre-pass: own-shard x @ Wad1 -> ad1c ----------
            pid = nc.sync.partition_id()
            with (nc.named_scope("ad1pass"),
                  tc.tile_pool(name="a1", bufs=2) as apool,
                  tc.tile_pool(name="a1p", bufs=2, space="PSUM") as aps):
                ADB = 16
                base = pid * NSH
                b0 = 0
                while b0 < NT:
                    nb = min(ADB, NT - b0)
                    ncol = min(nb * 128, NSH - b0 * 128)
                    xmA = apool.tile([128, ADB * 128], BF16, tag="xmA")
                    nc.sync.dma_start(
                        out=xmA[:, 0:ncol],
                        in_=xT[:, bass.ds(base + b0 * 128, ncol)])
                    psA = aps.tile([128, ADB, 4], F32, tag="psA")
                    if ncol < nb * 128:
                        nc.vector.memset(psA[:], 0.0)
                    for t in range(nb):
                        nn = min(128, ncol - t * 128)
                        if nn <= 0:
                            break
                        nc.tensor.matmul(
                            out=bass.AP(psA[:].tensor, psA[:].offset + t * 4,
                                        [[list(psA[:].ap[0])[0], nn], [1, 4]]),
                            lhsT=xmA[:, t * 128:t * 128 + nn],
                            rhs=wad1sb[:], start=True, stop=True)
                    adt = apool.tile([128, ADB * 4], F32, tag="adt")
                    nc.vector.tensor_copy(out=adt[:, 0:nb * 4],
                                          in_=psA[:, 0:nb, :])
                    nc.sync.dma_start(
                        out=bass.AP(ad1c[:, :].tensor, b0 * 128 * 64,
                                    [[64, 128], [64 * 128, nb], [1, 4]]),
                        in_=vap(adt, 0, [[4, nb], [1, 4]]))
                    b0 += nb

            # ---------- dense1 + edge1 (pools coexist so both overlap) ----
            with (tc.tile_pool(name="eg1", bufs=2) as gp1,
                  tc.tile_pool(name="ea1", bufs=2) as ap1,
                  tc.tile_pool(name="eso1", bufs=2) as sop1,
                  tc.tile_pool(name="ew1", bufs=3) as wp1):
                with (nc.named_scope("dense1"),
                      tc.tile_pool(name="d1", bufs=3) as dpool,
                      tc.tile_pool(name="d1b", bufs=2) as bpool,
                      tc.tile_pool(name="d1p", bufs=2, space="PSUM") as dps):
                    _dense_pass(nc, tc, dpool, bpool, dps, layer=1,
                                src=xT, wsb=w1sb, table=table1, h1T_all=None)

                # ---------- edge pass L1 ----------
                with nc.named_scope("edge1"):
                    _edge_pass(nc, tc, meta, layer=1, sidx=sidx1, aidx=aidx,
                               table=table1, stg=stg1, ad_core=ad1c,
                               pools=(gp1, ap1, sop1, wp1))

            # ---------- merge1 + dense2 + edge2 (L2 pools open early so
            # dense2 need not wait for merge1's pool region to free) ------
            with (tc.tile_pool(name="eg2", bufs=2) as gp2,
                  tc.tile_pool(name="ea2", bufs=2) as ap2,
                  tc.tile_pool(name="eso2", bufs=2) as sop2,
                  tc.tile_pool(name="ew2", bufs=3) as wp2,
                  tc.tile_pool(name="d2", bufs=3) as dpool2,
                  tc.tile_pool(name="d2b", bufs=2) as bpool2,
                  tc.tile_pool(name="d2p", bufs=4, space="PSUM") as dps2):
                with nc.named_scope("merge1"):
                    _merge_pass(nc, tc, meta, layer=1, midx=midx, stg=stg1,
                                bsb=b1sb, out2=None, h1T_sh=h1T_sh,
                                wadsb=wad2sb, adc=ad2c, nta=NTA,
                                h1T_all=h1T_all)

                with nc.named_scope("dense2"):
                    _dense_pass(nc, tc, dpool2, bpool2, dps2, layer=2,
                                src=None, wsb=w2sb, table=table2,
                                h1T_all=h1T_all)

                # ---------- edge pass L2 ----------
                with nc.named_scope("edge2"):
                    _edge_pass(nc, tc, meta, layer=2, sidx=sidx2, aidx=aidx,
                               table=table2, stg=stg2, ad_core=ad2c,
                               pools=(gp2, ap2, sop2, wp2))

            # ---------- merge L2 -> out2 ----------
            with nc.named_scope("merge2"):
                _merge_pass(nc, tc, meta, layer=2, midx=midx, stg=stg2,
                            bsb=b2sb, out2=out2, h1T_sh=None,
                            wadsb=None, adc=None)

    nc.finalize()
    return nc


def _dense_pass(nc, tc, dpool, bpool, dps, layer, src, wsb, table, h1T_all):
    """Replicated dense phase: DB-tile batches, PG-tile PSUM groups.
    layer 1: in xT f32 [128, N] -> table1 rows [h(128)|as_hi(4)|as_lo(4)] bf16.
    layer 2: in h1T_all bf16 -> table2 rows [h(64)|as_hi|as_lo] bf16.
    """
    if layer == 1:
        ntiles, K, MC = (N + 127) // 128, 128, 132   # matmul out cols
        FD = 128
        CHN, Ntot = CH1, N
        PG, PST = 4, 512       # PSUM group; slot stride padded to a full bank
    else:
        ntiles, K, MC = R2 // 128, 32, 68
        FD = 64
        CHN, Ntot = CH2, R2
        PG, PST = 7, 68        # 7 x 272B fits one bank
    RW = 256

    b0 = 0
    eng_i = 0
    while b0 < ntiles:
        nb = min(DB, ntiles - b0)
        n0 = b0 * 128
        # ---- batched input load ----
        xm = dpool.tile([K, DB * 128], BF16, tag="xm")
        if layer == 1:
            nn = min(nb * 128, N - n0)
            nc.sync.dma_start(out=xm[:, 0:nn], in_=bass.AP(
                src[:, :].tensor, n0, [[N, K], [1, nn]]))
        else:
            # h1T_all halves [NCORES, 32, NHA/(NSHP-NHA)]; split loads at
            # core and half boundaries
            q = b0
            col = 0
            while q < b0 + nb:
                cc, tt = q // NT, q % NT
                if tt < NTA:
                    hf, tb, hw = 0, 0, NHA
                else:
                    hf, tb, hw = 1, NTA, NSHP - NHA
                run = min((NTA if tt < NTA else NT) - tt, b0 + nb - q)
                nc.sync.dma_start(
                    out=xm[:, col * 128:(col + run) * 128],
                    in_=bass.AP(h1T_all[hf][:, :, :].tensor,
                                cc * 32 * hw + (tt - tb) * 128,
                                [[hw, 32], [1, run * 128]]))
                q += run
                col += run
        # ---- batch output tiles (row = [h | as]) ----
        hrowB = bpool.tile([128, DB, FD + 8], BF16, tag="hrowB")
        g0 = 0
        while g0 < nb:
            ng = min(PG, nb - g0)
            ps = dps.tile([128, PG, PST], F32, tag="ps")
            partial = (n0 + (g0 + ng) * 128) > Ntot
            if partial:
                nc.vector.memset(ps[:], 0.0)
            for t in range(ng):
                tt = g0 + t
                nn = min(128, Ntot - (n0 + tt * 128))
                nc.tensor.matmul(
                    out=bass.AP(ps[:].tensor, ps[:].offset + (t * PST),
                                [[list(ps[:].ap[0])[0], nn], [1, MC]]),
                    lhsT=xm[:, tt * 128:tt * 128 + nn],
                    rhs=wsb[:], start=True, stop=True)
            use_act = (eng_i % 2 == 1)
            eng_i += 1

            def _copy(out, in_):
                if use_act:
                    nc.scalar.activation(out=out, in_=in_, func=ActFn.Copy)
                else:
                    nc.vector.tensor_copy(out=out, in_=in_)

            # bulk copy [h | as_hi] (+ leave as_lo slot) per PSUM group
            _copy(vap(hrowB, g0 * (FD + 8), [[FD + 8, ng], [1, FD + 4]]),
                  vap(ps, 0, [[PST, ng], [1, FD + 4]]))
            g0 += ng
        # table rows per chunk tensor; split at chunk boundary
        t0 = 0
        while t0 < nb:
            gn0 = n0 + t0 * 128
            rows = min(128, Ntot - gn0)
            k = gn0 // CHN
            avail = (k + 1) * CHN - gn0
            if avail >= rows:
                if rows == 128:
                    run = min(nb - t0, avail // 128)
                else:
                    run = 1
                r0 = gn0 - k * CHN
                nc.sync.dma_start(
                    out=bass.AP(table[k][:, :].tensor, r0 * RW,
                                [[RW, rows], [RW * 128 if run > 1 else 1, run],
                                 [1, FD + 4]])
                    if run > 1 else
                    bass.AP(table[k][:, :].tensor, r0 * RW,
                            [[RW, rows], [1, FD + 4]]),
                    in_=hrowB[:, t0:t0 + run, 0:FD + 4] if run > 1
                    else hrowB[0:rows, t0, 0:FD + 4])
                t0 += run
            else:
                # tile straddles the chunk boundary: split by partition range
                nsplit = avail
                r0 = gn0 - k * CHN
                nc.sync.dma_start(
                    out=bass.AP(table[k][:, :].tensor, r0 * RW,
                                [[RW, nsplit], [1, FD + 4]]),
                    in_=hrowB[0:nsplit, t0, 0:FD + 4])
                nc.sync.dma_start(
                    out=bass.AP(table[k + 1][:, :].tensor, 0,
                                [[RW, rows - nsplit], [1, FD + 4]]),
                    in_=hrowB[nsplit:rows, t0, 0:FD + 4])
                t0 += 1
        b0 += nb


def _edge_pass(nc, tc, meta, layer, sidx, aidx, table, stg, ad_core, pools):
    rowbase = meta["rowbase"]
    grid_rows = meta["grid_rows"]
    RW = 256                              # table row elems (bf16)
    FD = 128 if layer == 1 else 64        # feature elems

    # stream offset of each bucket's first slot (buckets are contiguous)
    bstart = {}
    for (k, bi, t, c0, w, off) in meta["calls"]:
        bstart.setdefault((k, bi), off)

    gp, ap_pool, sop, wp = pools
    if True:
        abase = 0
        for k in range(NCH):
            for bi, D in enumerate(BUCKETS):
                Tb = int(meta["T"][k][bi])
                if Tb == 0:
                    continue
                rb0 = rowbase[(k, bi, 0)]
                # per-bucket ad gather (<=2048-idx calls)
                ADG = ap_pool.tile([128, Tb, 64], F32, tag="ADG")
                na = Tb * 128
                o = 0
                while o < na:
                    nbv = min(MAXD, na - o)
                    nc.gpsimd.dma_gather(
                        ADG[:, o // 128:(o + nbv) // 128, :], ad_core[:, :],
                        aidx[:, (abase + rb0 + o) // 16:
                                (abase + rb0 + o + nbv) // 16],
                        nbv, nbv, 64, queue_num=qn())
                    o += nbv
                # per-bucket f32 accumulator + bf16 staging copy
                fsB = sop.tile([128, Tb, FD + 4], F32, tag="fsB")
                soB = sop.tile([128, Tb, FD + 4], BF16, tag="soB")
                # segments of <= CAP cols (tile-aligned); gathered in <= SEG
                # col calls, processed per segment to amortize DVE dispatch
                gt = max(1, CAP // D)      # tiles per segment
                t0 = 0
                off = bstart[(k, bi)]
                while t0 < Tb:
                    gn = min(gt, Tb - t0)
                    ncols = gn * D
                    G = gp.tile([128, max(CAP, D), RW], BF16, tag="G")
                    so = off + 128 * (t0 * D)
                    c = 0
                    while c < ncols:
                        w = min(SEG, ncols - c)
                        nc.gpsimd.dma_gather(
                            G[:, c:c + w, :], table[k][:, :],
                            sidx[:, (so + 128 * c) // 16:
                                    (so + 128 * (c + w)) // 16],
                            128 * w, 128 * w, RW, queue_num=qn())
                        c += w
                    # e = exp(leaky(as_hi + as_lo + ad))  [f32]
                    e = wp.tile([128, max(CAP, D) * 4], F32, tag="e")
                    nc.vector.tensor_tensor(
                        out=e[:, 0:ncols * 4],
                        in0=vap(G, FD, [[RW, ncols], [1, 4]]),
                        in1=bass.AP(ADG[:].tensor,
                                    ADG[:].offset + t0 * 64,
                                    [list(ADG[:].ap[0]), [64, gn], [0, D],
                                     [1, 4]]),
                        op=AluOp.add)
                    nc.vector.scalar_tensor_tensor(
                        out=e[:, 0:ncols * 4], in0=e[:, 0:ncols * 4],
                        scalar=0.2, in1=e[:, 0:ncols * 4],
                        op0=AluOp.mult, op1=AluOp.max)
                    # single bf16 exp on ACT serves weights AND denominators
                    ebf = wp.tile([128, max(CAP, D) * 4], BF16, tag="ebf")
                    nc.scalar.activation(out=ebf[:, 0:ncols * 4],
                                         in_=e[:, 0:ncols * 4], func=ActFn.Exp)
                    # denominators (f32 accumulate)
                    nc.vector.tensor_reduce(
                        out=bass.AP(fsB[:].tensor,
                                    fsB[:].offset + t0 * (FD + 4) + FD,
                                    [list(fsB[:].ap[0]), [FD + 4, gn],
                                     [1, 4]]),
                        in_=vap(ebf, 0, [[4 * D, gn], [1, 4], [4, D]]),
                        axis=Axis.X, op=AluOp.add)
                    # segment-fused weighted features
                    val = wp.tile([128, max(CAP, D) * FD], BF16, tag="val")
                    nc.vector.tensor_tensor(
                        out=vap(val, 0, [[D * FD, gn], [FD, D],
                                         [FD // 4, 4], [1, FD // 4]]),
                        in0=vap(G, 0, [[RW * D, gn], [RW, D],
                                       [FD // 4, 4], [1, FD // 4]]),
                        in1=vap(ebf, 0, [[4 * D, gn], [4, D],
                                         [1, 4], [0, FD // 4]]),
                        op=AluOp.mult)
                    nc.vector.tensor_reduce(
                        out=bass.AP(fsB[:].tensor,
                                    fsB[:].offset + t0 * (FD + 4),
                                    [list(fsB[:].ap[0]), [FD + 4, gn],
                                     [1, FD]]),
                        in_=vap(val, 0, [[D * FD, gn], [1, FD], [FD, D]]),
                        axis=Axis.X, op=AluOp.add)
                    t0 += gn
                # one bf16 round + one staging write per bucket
                nc.vector.tensor_copy(out=soB[:], in_=fsB[:])
                nc.sync.dma_start(
                    out=bass.AP(stg[k][:, :].tensor, rb0 * RW,
                                [[RW, 128], [RW * 128, Tb], [1, FD + 4]]),
                    in_=soB[:])
            abase += grid_rows[k]


def _merge_pass(nc, tc, meta, layer, midx, stg, bsb, out2, h1T_sh,
                wadsb, adc, nta=None, h1T_all=None):
    RW = 256
    FD = 128 if layer == 1 else 64
    OD = HID if layer == 1 else OUT_DIM
    W = FD + 4
    if layer == 1:
        ranges = [(0, nta, 0), (nta, NT, 1)]
    else:
        ranges = [(0, NT, 0)]

    with (tc.tile_pool(name=f"mi{layer}", bufs=1) as ip,
          tc.tile_pool(name=f"mg{layer}", bufs=2) as gp,
          tc.tile_pool(name=f"ms{layer}", bufs=2) as sp_pool,
          tc.tile_pool(name=f"mw{layer}", bufs=2) as wp,
          tc.tile_pool(name=f"mp{layer}", bufs=2, space="PSUM") as pp):
        if layer == 1:
            from concourse.masks import make_identity
            ident = ip.tile([128, 128], F32, tag="ident")
            make_identity(nc, ident[:])

        for (t_lo, t_hi, hf) in ranges:
            _merge_range(nc, meta, layer, midx, stg, bsb, out2,
                         h1T_sh[hf] if layer == 1 else None,
                         wadsb, adc, gp, sp_pool, wp, pp,
                         ident if layer == 1 else None,
                         t_lo, t_hi, RW, FD, OD, W)
        if layer == 1:
            for hf in range(2):
                nc.gpsimd.collective_compute(
                    "AllGather", AluOp.bypass,
                    replica_groups=[list(range(NCORES))],
                    ins=[h1T_sh[hf][:, :]], outs=[h1T_all[hf][:, :, :]])


def _merge_range(nc, meta, layer, midx, stg, bsb, out2, h1T_sh, wadsb, adc,
                 gp, sp_pool, wp, pp, ident, t_lo, t_hi, RW, FD, OD, W):
        mt = t_lo
        while mt < t_hi:
            nb = min(MB, t_hi - mt)
            s = sp_pool.tile([128, MB * W], F32, tag="s")
            s01 = wp.tile([128, MB * W], BF16, tag="s01")
            Gprev = None
            for k in range(NCH):
                Gk = gp.tile([128, MB, RW], BF16, tag="MG")
                ioff = k * NSHP + mt * 128
                o = 0
                while o < nb * 128:
                    nbv = min(MAXD, nb * 128 - o)
                    nc.gpsimd.dma_gather(
                        Gk[:, o // 128:(o + nbv) // 128, :], stg[k][:, :],
                        midx[:, (ioff + o) // 16:(ioff + o + nbv) // 16],
                        nbv, nbv, RW, queue_num=qn())
                    o += nbv
                if k == 1:
                    # bf16 pair-add runs in the DVE 2x fast mode
                    with nc.allow_low_precision(reason="bf16 staged pair"):
                        nc.vector.tensor_tensor(
                            out=s01[:, 0:nb * W],
                            in0=vap(Gprev, 0, [[RW, nb], [1, W]]),
                            in1=vap(Gk, 0, [[RW, nb], [1, W]]), op=AluOp.add)
                elif k == 2:
                    nc.vector.tensor_tensor(
                        out=vap(s, 0, [[W, nb], [1, W]]),
                        in0=s01[:, 0:nb * W],
                        in1=vap(Gk, 0, [[RW, nb], [1, W]]), op=AluOp.add)
                elif k == 3:
                    nc.vector.tensor_tensor(
                        out=vap(s, 0, [[W, nb], [1, W]]),
                        in0=vap(s, 0, [[W, nb], [1, W]]),
                        in1=vap(Gk, 0, [[RW, nb], [1, W]]), op=AluOp.add)
                Gprev = Gk
            rec = wp.tile([128, MB * 4], F32, tag="rec")
            nc.vector.tensor_scalar_add(
                out=vap(rec, 0, [[4, nb], [1, 4]]),
                in0=vap(s, FD, [[W, nb], [1, 4]]), scalar1=EPS)
            nc.vector.reciprocal(out=rec[:, 0:nb * 4], in_=rec[:, 0:nb * 4])
            nc.vector.tensor_scalar_mul(out=rec[:, 0:nb * 4],
                                        in0=rec[:, 0:nb * 4], scalar1=0.25)
            sc = wp.tile([128, MB * FD], F32, tag="sc")
            nc.vector.tensor_tensor(
                out=vap(sc, 0, [[FD, nb], [FD // 4, 4], [1, FD // 4]]),
                in0=vap(s, 0, [[W, nb], [FD // 4, 4], [1, FD // 4]]),
                in1=vap(rec, 0, [[4, nb], [1, 4], [0, FD // 4]]),
                op=AluOp.mult)
            hs = wp.tile([128, MB * OD], F32, tag="hs")
            nc.vector.tensor_reduce(
                out=vap(hs, 0, [[OD, nb], [1, OD]]),
                in_=vap(sc, 0, [[FD, nb], [1, OD], [OD, 4]]),
                axis=Axis.X, op=AluOp.add)
            nc.vector.tensor_tensor(
                out=vap(hs, 0, [[OD, nb], [1, OD]]),
                in0=vap(hs, 0, [[OD, nb], [1, OD]]),
                in1=vap(bsb, 0, [[0, nb], [1, OD]]), op=AluOp.add)
            if layer == 1:
                nc.scalar.activation(out=hs[:, 0:nb * OD], in_=hs[:, 0:nb * OD],
                                     func=ActFn.Relu)
                hsbB = wp.tile([32, MB * 128], BF16, tag="hsbB")
                ti = 0
                while ti < nb:
                    jn = min(4, nb - ti)
                    psT = pp.tile([32, 4, 128], F32, tag="psT")
                    for j in range(jn):
                        nc.tensor.transpose(
                            out=psT[:, j, :],
                            in_=hs[:, (ti + j) * OD:(ti + j + 1) * OD],
                            identity=ident[:])
                    nc.vector.tensor_copy(
                        out=hsbB[:, ti * 128:(ti + jn) * 128],
                        in_=psT[:, 0:jn, :])
                    ti += jn
                nc.scalar.dma_start(
                    out=h1T_sh[:, (mt - t_lo) * 128:(mt - t_lo + nb) * 128],
                    in_=hsbB[:, 0:nb * 128])
                # ad2 for next layer: h1 @ Wad2, straight into ad2c
                psA = pp.tile([128, MB, 4], F32, tag="psA2")
                for ti in range(nb):
                    nc.tensor.matmul(
                        out=bass.AP(psA[:].tensor, psA[:].offset + ti * 4,
                                    [[list(psA[:].ap[0])[0], 128], [1, 4]]),
                        lhsT=hsbB[:, ti * 128:(ti + 1) * 128],
                        rhs=wadsb[:], start=True, stop=True)
                adt = wp.tile([128, MB * 4], F32, tag="adt2")
                nc.vector.tensor_copy(out=adt[:, 0:nb * 4],
                                      in_=psA[:, 0:nb, :])
                nc.scalar.dma_start(
                    out=bass.AP(adc[:, :].tensor, mt * 128 * 64,
                                [[64, 128], [64 * 128, nb], [1, 4]]),
                    in_=vap(adt, 0, [[4, nb], [1, 4]]))
            else:
                nc.sync.dma_start(
                    out=bass.AP(out2[:, :].tensor, mt * 128 * OD,
                                [[OD, 128], [OD * 128, nb], [1, OD]]),
                    in_=vap(hs, 0, [[OD, nb], [1, OD]]))
            mt += nb


_CACHE = {}


def kernel(**inputs):
    in_maps, meta = host_prep(**inputs)
    key = str(meta["T"])
    _CACHE["k"] = key
    if key not in _CACHE:
        nc = build_nc(meta)
        _CACHE[key] = (nc, make_runner(nc, NCORES))
    nc, run = _CACHE[key]
    results, best = run(in_maps, repeats=1)
    _CACHE["last_time"] = best
    out = np.empty((N, OUT_DIM), np.float32)
    for c in range(NCORES):
        out[c * NSH:(c + 1) * NSH] = results[c]["out2"][:NSH]
    return out



# revision 31
# speedup vs baseline: 1.9284x; 1.2743x over previous
"""GAT 2-layer kernel for TRN2, 8 NeuronCores (self-contained).

Strategy:
- dst-shard: core c owns nodes [c*12500, (c+1)*12500).
- ad pre-pass: own-shard x @ fold(W, a_dst) -> ad1c, so edge pipelines
  never wait on the full dense pass.
- Dense phases (x@W bf16, h1@W2 bf16) replicated on all cores; folded
  a_src gives per-node [h | as] rows in one matmul. 16-tile DMA batches
  (HWDGE calls are ~600ns each on one shared device), bank-sized PSUM
  groups, copy work split across DVE/ACT.
- Edge phase per core: 4 src-chunks (25000 nodes, int16 gather reach),
  per-chunk degree-bucketed padded CSR over dst. 1024-index single-queue
  SWDGE gathers (multi-queue under TileContext races; >16KB rings hang
  HW) pull [h | as] rows. e = exp(leaky(as+ad)) on ACT; segment-fused
  bf16 weighted-feature mult + f32 reduce into per-bucket tiles.
- Per-chunk partials [featsum | denom] -> DRAM staging (bf16); per-chunk
  table tensors + concurrently-open tile pools let edge chunks overlap
  the dense tail.
- Merge pass (natural node order): per-chunk 16-tile gathers, f32
  accumulate, per-head normalize, bias (+relu L1); L1 also computes
  ad2c (h1 @ fold(W2, a2_dst)) from the transposed tiles.
- L1->L2: h1T shard AllGather in two halves (overlaps merge tail and
  dense2 head) -> replicated dense2.
"""
import sys
sys.path.insert(0, "/opt/trn_rl_repo")
import numpy as np
import ml_dtypes

import concourse.bass as bass
import concourse.bacc as bacc
import concourse.tile as tile
from concourse import mybir
from concourse.library_config import mlp as mlp_lib


def make_runner(nc, n_cores):
    """PJRT runner: returns run_fn(in_maps, repeats) -> (results, best_time_s)."""
    import time
    import jax
    from jax.sharding import Mesh, PartitionSpec, NamedSharding
    from jax.experimental.shard_map import shard_map
    from concourse.bass2jax import (_bass_exec_p, install_neuronx_cc_hook,
                                    partition_id_tensor)
    install_neuronx_cc_hook()
    partition_name = nc.partition_id_tensor.name if nc.partition_id_tensor else None
    in_names, out_names, out_avals, zero_outs = [], [], [], []
    for alloc in nc.m.functions[0].allocations:
        if not isinstance(alloc, mybir.MemoryLocationSet):
            continue
        if not alloc.memorylocations:
            continue
        name = alloc.memorylocations[0].name
        if alloc.kind == "ExternalInput":
            if name != partition_name:
                in_names.append(name)
        elif alloc.kind == "ExternalOutput":
            out_names.append(name)
            shape = tuple(alloc.tensor_shape)
            dtype = mybir.dt.np(alloc.dtype)
            out_avals.append(jax.core.ShapedArray(shape, dtype))
            zero_outs.append(np.zeros(shape, dtype))
    n_params = len(in_names)
    n_outs = len(out_avals)
    all_in_names = list(in_names) + list(out_names)
    if partition_name is not None:
        all_in_names.append(partition_name)

    def _body(*args):
        operands = list(args)
        if partition_name is not None:
            operands.append(partition_id_tensor())
        return tuple(_bass_exec_p.bind(
            *operands, out_avals=tuple(out_avals), in_names=tuple(all_in_names),
            out_names=tuple(out_names), lowering_input_output_aliases=(),
            sim_require_finite=False, sim_require_nnan=False, nc=nc))

    devices = jax.devices()[:n_cores]
    mesh = Mesh(np.asarray(devices), ("core",))
    in_specs = (PartitionSpec("core"),) * (n_params + n_outs)
    out_specs = (PartitionSpec("core"),) * n_outs
    donate = tuple(range(n_params, n_params + n_outs))
    sharded = jax.jit(
        shard_map(_body, mesh=mesh, in_specs=in_specs, out_specs=out_specs,
                  check_rep=False),
        donate_argnums=donate, keep_unused=True)

    def run_fn(in_maps, repeats=1):
        per_core = [[np.asarray(m[name]) for name in in_names] for m in in_maps]
        concat_in = [np.concatenate([per_core[c][i] for c in range(n_cores)], 0)
                     for i in range(n_params)]
        sharding = NamedSharding(mesh, PartitionSpec("core"))
        dev_in = [jax.device_put(a, sharding) for a in concat_in]
        for a in dev_in:
            a.block_until_ready()
        times, out_arrs = [], None
        for _ in range(repeats):
            concat_zeros = [jax.device_put(
                np.zeros((n_cores * z.shape[0], *z.shape[1:]), z.dtype), sharding)
                for z in zero_outs]
            for z in concat_zeros:
                z.block_until_ready()
            t0 = time.perf_counter()
            out_arrs = sharded(*dev_in, *concat_zeros)
            for o in out_arrs:
                o.block_until_ready()
            times.append(time.perf_counter() - t0)
        results = [
            {name: np.asarray(out_arrs[i]).reshape(n_cores, *out_avals[i].shape)[c]
             for i, name in enumerate(out_names)}
            for c in range(n_cores)]
        return results, min(times)

    return run_fn

F32 = mybir.dt.float32
BF16 = mybir.dt.bfloat16
I16 = mybir.dt.int16
BF = ml_dtypes.bfloat16

NCORES = 8
N = 100000
IN_DIM = 128
HID = 32
OUT_DIM = 16
NSH = N // NCORES            # 12500
NT = 98                      # merge tiles per core
NSHP = NT * 128              # 12544
CH1 = 25000                  # table1 nodes per chunk
NCH = 4
CH1R = CH1 + 1               # +pad row
R2 = NCORES * NSHP           # 100352
CH2 = R2 // NCH              # 25088 (= 2 padded cores)
CH2R = CH2 + 1
BUCKETS = (1, 2, 3, 4, 5, 6, 8, 16)
NEG = -1.0e30
EPS = 1e-16
MAXD = 1024                  # max descriptors per SWDGE call (HW ring limit)
RING = 16384                 # SWDGE descriptor ring bytes (>16KB hangs HW)
NQ = 4                       # SWDGE queues (measured: 1q=92GB/s, 4q=450GB/s)
_QRR = [0]


def qn():
    _QRR[0] = (_QRR[0] + 1) % NQ
    return _QRR[0]
SEG = MAXD // 128            # max gather cols per SWDGE call
CAP = 32                     # edge-pass segment cols (DVE op granularity)
DB = 24                      # dense batch (tiles per DMA)
PG = 3                       # dense PSUM group (tiles per PSUM bank tile)
MB = 8                       # merge batch (tiles)
NTA = 49                     # merge tiles in first allgather half
NHA = NTA * 128              # 6272

AluOp = mybir.AluOpType
ActFn = mybir.ActivationFunctionType
Axis = mybir.AxisListType

SKIP = set()                 # timing-sensitivity knobs (empty in production)


def _colgroups(D):
    out = []
    c = 0
    while c < D:
        w = min(8, D - c)
        out.append((c, w))
        c += w
    return out


def plan_segments(T):
    """Host/device shared slot-stream layout (order: k, bucket, tile, col, p)."""
    calls = []
    rowbase = {}
    grid_rows = []
    off = 0
    for k in range(NCH):
        rb = 0
        for bi, D in enumerate(BUCKETS):
            for t in range(int(T[k][bi])):
                rowbase[(k, bi, t)] = rb
                for (c0, w) in _colgroups(D):
                    calls.append((k, bi, t, c0, w, off))
                    off += 128 * w
                rb += 128
        grid_rows.append(rb)
    return calls, rowbase, grid_rows, off


def _wrap_idx(flat):
    n = len(flat)
    assert n % 16 == 0
    w = np.asarray(flat, np.int16).reshape(n // 16, 16).T
    return np.ascontiguousarray(np.tile(w, (8, 1)))


def fold(W, a):
    Hh, F = a.shape
    w = np.zeros((W.shape[0], Hh), np.float32)
    for h in range(Hh):
        w[:, h] = W[:, h * F:(h + 1) * F] @ a[h]
    return w


def host_prep(x, edge_index, W1, a1_src, a1_dst, b1, W2, a2_src, a2_dst, b2):
    x = np.asarray(x, np.float32)
    ei = np.asarray(edge_index)
    src = ei[0].astype(np.int64)
    dst = ei[1].astype(np.int64)
    W1 = np.asarray(W1, np.float32)
    W2 = np.asarray(W2, np.float32)
    Waug1 = np.concatenate([W1, fold(W1, np.asarray(a1_src, np.float32))], 1)
    Waug2 = np.concatenate([W2, fold(W2, np.asarray(a2_src, np.float32))], 1)
    Wad1 = fold(W1, np.asarray(a1_dst, np.float32))          # [128, 4]
    Wad2 = fold(W2, np.asarray(a2_dst, np.float32))          # [32, 4]
    xT = np.ascontiguousarray(x.T.astype(BF))

    core_of = dst // NSH
    # ---- per-core, per-chunk CSR ----
    pc = []  # [core][chunk] = (deg, sorted_src_by_dst, starts)
    for c in range(NCORES):
        m = core_of == c
        s_c, d_c = src[m], dst[m] - c * NSH
        ch = s_c // CH1
        info = []
        for k in range(NCH):
            mk = ch == k
            sk, dk = s_c[mk], d_c[mk]
            deg = np.bincount(dk, minlength=NSH)
            order = np.argsort(dk, kind="stable")
            sk = sk[order]
            starts = np.zeros(NSH + 1, np.int64)
            np.cumsum(deg, out=starts[1:])
            info.append((deg, sk, starts))
        pc.append(info)

    # shared tile counts
    T = [[0] * len(BUCKETS) for _ in range(NCH)]
    for c in range(NCORES):
        for k in range(NCH):
            deg = pc[c][k][0]
            for bi, D in enumerate(BUCKETS):
                lo = BUCKETS[bi - 1] if bi else 0
                nb = int(((deg > lo) & (deg <= D)).sum())
                T[k][bi] = max(T[k][bi], (nb + 127) // 128)
            assert deg.max(initial=0) <= BUCKETS[-1], f"deg max {deg.max()}"
    calls, rowbase, grid_rows, stream_len = plan_segments(T)

    b1rep = np.tile(np.asarray(b1, np.float32)[None, :], (128, 1))
    b2rep = np.tile(np.asarray(b2, np.float32)[None, :], (128, 1))
    pad1 = np.zeros((1, 256), BF); pad1[0, 128:132] = NEG
    pad2 = np.zeros((1, 256), BF); pad2[0, 64:68] = NEG
    z256 = np.zeros((1, 256), BF)

    in_maps = []
    for c in range(NCORES):
        slot_nodes = []   # per chunk: grid row -> node (or -1)
        for k in range(NCH):
            gr = grid_rows[k]
            deg, sk, starts = pc[c][k]
            nodes_of = np.full(gr, -1, np.int64)
            for bi, D in enumerate(BUCKETS):
                lo = BUCKETS[bi - 1] if bi else 0
                nd = np.where((deg > lo) & (deg <= D))[0]
                rb = rowbase[(k, bi, 0)] if T[k][bi] else 0
                nodes_of[rb:rb + len(nd)] = nd
            slot_nodes.append(nodes_of)

        s1 = np.full(stream_len, CH1, np.int64)     # pad -> table1 chunk pad row
        s2 = np.full(stream_len, CH2, np.int64)     # pad -> table2 chunk pad row
        for (k, bi, t, c0, w, off) in calls:
            D = BUCKETS[bi]
            rb = rowbase[(k, bi, t)]
            deg, sk, starts = pc[c][k]
            nodes = slot_nodes[k][rb:rb + 128]
            j = off
            for d in range(c0, c0 + w):
                for p in range(128):
                    nd = nodes[p]
                    if nd >= 0 and d < starts[nd + 1] - starts[nd]:
                        s = sk[starts[nd] + d]
                        s1[j] = s % CH1
                        s2[j] = (s // NSH % 2) * NSHP + s % NSH
                    j += 1
        slot1w = _wrap_idx(s1)
        slot2w = _wrap_idx(s2)

        # ad idx: per (k, gridtile) 128 local dst ids (pad -> 0)
        adix = []
        for k in range(NCH):
            nd = slot_nodes[k]
            adix.append(np.where(nd >= 0, nd, 0))
        adw = _wrap_idx(np.concatenate(adix)) if stream_len else None

        # merge idx: per chunk, per natural node (padded to NSHP): grid row or zero-row
        mrg = []
        for k in range(NCH):
            deg = pc[c][k][0]
            pos = np.full(NSHP, grid_rows[k], np.int64)  # zero row
            nd = slot_nodes[k]
            real = nd >= 0
            pos[nd[real]] = np.nonzero(real)[0]
            mrg.append(pos)
        mrgw = _wrap_idx(np.concatenate(mrg))

        in_maps.append(dict(
            xT=xT, Waug1=Waug1.astype(BF), Waug2=Waug2.astype(BF),
            Wad1=Wad1.astype(BF), Wad2=Wad2.astype(BF),
            b1rep=b1rep, b2rep=b2rep, pad1=pad1, pad2=pad2, z256=z256,
            slot1w=slot1w, slot2w=slot2w, adw=adw, mrgw=mrgw,
        ))
    meta = dict(T=T, calls=calls, rowbase=rowbase, grid_rows=grid_rows,
                stream_len=stream_len)
    return in_maps, meta


def vap(t, off, dims):
    a = t[:]
    return bass.AP(a.tensor, a.offset + off, [list(a.ap[0])] + [list(d) for d in dims])


def build_nc(meta):
    _QRR[0] = 0
    T = meta["T"]
    grid_rows = meta["grid_rows"]
    stream_len = meta["stream_len"]
    SW = stream_len // 16
    ADL = sum(grid_rows)
    AW = ADL // 16
    MW = (NCH * NSHP) // 16

    nc = bacc.Bacc("TRN2", target_bir_lowering=False, num_swdge_queues=NQ,
                   dynamic_dma_scratch_size=RING)
    dp = nc.declare_dram_parameter
    xT = dp("xT", [IN_DIM, N], BF16, isOutput=False)
    Waug1 = dp("Waug1", [128, 132], BF16, isOutput=False)
    Waug2 = dp("Waug2", [32, 68], BF16, isOutput=False)
    Wad1 = dp("Wad1", [128, 4], BF16, isOutput=False)
    Wad2 = dp("Wad2", [32, 4], BF16, isOutput=False)
    b1rep = dp("b1rep", [128, HID], F32, isOutput=False)
    b2rep = dp("b2rep", [128, OUT_DIM], F32, isOutput=False)
    pad1 = dp("pad1", [1, 256], BF16, isOutput=False)
    pad2 = dp("pad2", [1, 256], BF16, isOutput=False)
    z256 = dp("z256", [1, 256], BF16, isOutput=False)
    slot1w = dp("slot1w", [128, SW], I16, isOutput=False)
    slot2w = dp("slot2w", [128, SW], I16, isOutput=False)
    adw = dp("adw", [128, AW], I16, isOutput=False)
    mrgw = dp("mrgw", [128, MW], I16, isOutput=False)
    out2 = dp("out2", [NSHP, OUT_DIM], F32, isOutput=True)

    table1 = [nc.dram_tensor(f"table1_{k}", [CH1R, 256], BF16)
              for k in range(NCH)]
    table2 = [nc.dram_tensor(f"table2_{k}", [CH2R, 256], BF16)
              for k in range(NCH)]
    ad1c = nc.dram_tensor("ad1c", [NSHP, 64], F32)
    ad2c = nc.dram_tensor("ad2c", [NSHP, 64], F32)
    stg1 = [nc.dram_tensor(f"stg1_{k}", [grid_rows[k] + 1, 256], BF16)
            for k in range(NCH)]
    stg2 = [nc.dram_tensor(f"stg2_{k}", [grid_rows[k] + 1, 256], BF16)
            for k in range(NCH)]
    h1T_sh = [nc.dram_tensor("h1T_shA", [32, NHA], BF16),
              nc.dram_tensor("h1T_shB", [32, NSHP - NHA], BF16)]
    h1T_all = [nc.dram_tensor("h1T_allA", [NCORES, 32, NHA], BF16,
                              addr_space="Shared"),
               nc.dram_tensor("h1T_allB", [NCORES, 32, NSHP - NHA], BF16,
                              addr_space="Shared")]

    with tile.TileContext(nc) as tc:
        nc.gpsimd.load_library(mlp_lib)

        # ---------- consts / pads ----------
        with tc.tile_pool(name="konst", bufs=1) as kp:
            w1sb = kp.tile([128, 132], BF16)
            nc.sync.dma_start(out=w1sb[:], in_=Waug1[:, :])
            w2sb = kp.tile([32, 68], BF16)
            nc.sync.dma_start(out=w2sb[:], in_=Waug2[:, :])
            wad1sb = kp.tile([128, 4], BF16)
            nc.sync.dma_start(out=wad1sb[:], in_=Wad1[:, :])
            wad2sb = kp.tile([32, 4], BF16)
            nc.sync.dma_start(out=wad2sb[:], in_=Wad2[:, :])
            # edge/merge index streams, loaded up front so the edge pipelines
            # can start as soon as their table chunks are written
            SW = stream_len // 16
            AW = sum(grid_rows) // 16
            MW = (NCH * NSHP) // 16
            sidx1 = kp.tile([128, SW], I16)
            nc.sync.dma_start(out=sidx1[:], in_=slot1w[:, :])
            sidx2 = kp.tile([128, SW], I16)
            nc.sync.dma_start(out=sidx2[:], in_=slot2w[:, :])
            aidx = kp.tile([128, AW], I16)
            nc.sync.dma_start(out=aidx[:], in_=adw[:, :])
            midx = kp.tile([128, MW], I16)
            nc.sync.dma_start(out=midx[:], in_=mrgw[:, :])
            b1sb = kp.tile([128, HID], F32)
            nc.sync.dma_start(out=b1sb[:], in_=b1rep[:, :])
            b2sb = kp.tile([128, OUT_DIM], F32)
            nc.sync.dma_start(out=b2sb[:], in_=b2rep[:, :])
            for k in range(NCH):
                nc.sync.dma_start(out=table1[k][CH1, :], in_=pad1[0, :])
                nc.sync.dma_start(out=table2[k][CH2, :], in_=pad2[0, :])
                nc.sync.dma_start(out=stg1[k][grid_rows[k], :], in_=z256[0, :])
                nc.sync.dma_start(out=stg2[k][grid_rows[k], :], in_=z256[0, :])

            # ---------- ad1 pre-pass: own-shard x @ Wad1 -> ad1c ----------
            pid = nc.sync.partition_id()
            with (nc.named_scope("ad1pass"),
                  tc.tile_pool(name="a1", bufs=2) as apool,
                  tc.tile_pool(name="a1p", bufs=2, space="PSUM") as aps):
                ADB = 16
                base = pid * NSH
                b0 = 0
                while b0 < NT:
                    nb = min(ADB, NT - b0)
                    ncol = min(nb * 128, NSH - b0 * 128)
                    xmA = apool.tile([128, ADB * 128], BF16, tag="xmA")
                    nc.sync.dma_start(
                        out=xmA[:, 0:ncol],
                        in_=xT[:, bass.ds(base + b0 * 128, ncol)])
                    psA = aps.tile([128, ADB, 4], F32, tag="psA")
                    if ncol < nb * 128:
                        nc.vector.memset(psA[:], 0.0)
                    for t in range(nb):
                        nn = min(128, ncol - t * 128)
                        if nn <= 0:
                            break
                        nc.tensor.matmul(
                            out=bass.AP(psA[:].tensor, psA[:].offset + t * 4,
                                        [[list(psA[:].ap[0])[0], nn], [1, 4]]),
                            lhsT=xmA[:, t * 128:t * 128 + nn],
                            rhs=wad1sb[:], start=True, stop=True)
                    adt = apool.tile([128, ADB * 4], F32, tag="adt")
                    nc.vector.tensor_copy(out=adt[:, 0:nb * 4],
                                          in_=psA[:, 0:nb, :])
                    nc.sync.dma_start(
                        out=bass.AP(ad1c[:, :].tensor, b0 * 128 * 64,
                                    [[64, 128], [64 * 128, nb], [1, 4]]),
                        in_=vap(adt, 0, [[4, nb], [1, 4]]))
                    b0 += nb

            # ---------- dense1 + edge1 (pools coexist so both overlap) ----
            with (tc.tile_pool(name="eg1", bufs=2) as gp1,
                  tc.tile_pool(name="ea1", bufs=2) as ap1,
                  tc.tile_pool(name="eso1", bufs=2) as sop1,
                  tc.tile_pool(name="ew1", bufs=3) as wp1):
                with (nc.named_scope("dense1"),
                      tc.tile_pool(name="d1", bufs=3) as dpool,
                      tc.tile_pool(name="d1b", bufs=2) as bpool,
                      tc.tile_pool(name="d1p", bufs=2, space="PSUM") as dps):
                    _dense_pass(nc, tc, dpool, bpool, dps, layer=1,
                                src=xT, wsb=w1sb, table=table1, h1T_all=None)

                # ---------- edge pass L1 ----------
                with nc.named_scope("edge1"):
                    _edge_pass(nc, tc, meta, layer=1, sidx=sidx1, aidx=aidx,
                               table=table1, stg=stg1, ad_core=ad1c,
                               pools=(gp1, ap1, sop1, wp1))

            # ---------- merge1 + dense2 + edge2 (L2 pools open early so
            # dense2 need not wait for merge1's pool region to free) ------
            with (tc.tile_pool(name="eg2", bufs=2) as gp2,
                  tc.tile_pool(name="ea2", bufs=2) as ap2,
                  tc.tile_pool(name="eso2", bufs=2) as sop2,
                  tc.tile_pool(name="ew2", bufs=3) as wp2,
                  tc.tile_pool(name="d2", bufs=3) as dpool2,
                  tc.tile_pool(name="d2b", bufs=2) as bpool2,
                  tc.tile_pool(name="d2p", bufs=4, space="PSUM") as dps2):
                with nc.named_scope("merge1"):
                    _merge_pass(nc, tc, meta, layer=1, midx=midx, stg=stg1,
                                bsb=b1sb, out2=None, h1T_sh=h1T_sh,
                                wadsb=wad2sb, adc=ad2c, nta=NTA,
                                h1T_all=h1T_all)

                with nc.named_scope("dense2"):
                    _dense_pass(nc, tc, dpool2, bpool2, dps2, layer=2,
                                src=None, wsb=w2sb, table=table2,
                                h1T_all=h1T_all)

                # ---------- edge pass L2 ----------
                with nc.named_scope("edge2"):
                    _edge_pass(nc, tc, meta, layer=2, sidx=sidx2, aidx=aidx,
                               table=table2, stg=stg2, ad_core=ad2c,
                               pools=(gp2, ap2, sop2, wp2))

            # ---------- merge L2 -> out2 ----------
            with nc.named_scope("merge2"):
                _merge_pass(nc, tc, meta, layer=2, midx=midx, stg=stg2,
                            bsb=b2sb, out2=out2, h1T_sh=None,
                            wadsb=None, adc=None)

    nc.finalize()
    return nc


def _dense_pass(nc, tc, dpool, bpool, dps, layer, src, wsb, table, h1T_all):
    """Replicated dense phase: DB-tile batches, PG-tile PSUM groups.
    layer 1: in xT f32 [128, N] -> table1 rows [h(128)|as_hi(4)|as_lo(4)] bf16.
    layer 2: in h1T_all bf16 -> table2 rows [h(64)|as_hi|as_lo] bf16.
    """
    if layer == 1:
        ntiles, K, MC = (N + 127) // 128, 128, 132   # matmul out cols
        FD = 128
        CHN, Ntot = CH1, N
        PG, PST = 4, 512       # PSUM group; slot stride padded to a full bank
    else:
        ntiles, K, MC = R2 // 128, 32, 68
        FD = 64
        CHN, Ntot = CH2, R2
        PG, PST = 7, 68        # 7 x 272B fits one bank
    RW = 256

    b0 = 0
    eng_i = 0
    while b0 < ntiles:
        nb = min(DB, ntiles - b0)
        n0 = b0 * 128
        # ---- batched input load ----
        xm = dpool.tile([K, DB * 128], BF16, tag="xm")
        if layer == 1:
            nn = min(nb * 128, N - n0)
            nc.sync.dma_start(out=xm[:, 0:nn], in_=bass.AP(
                src[:, :].tensor, n0, [[N, K], [1, nn]]))
        else:
            # h1T_all halves [NCORES, 32, NHA/(NSHP-NHA)]; split loads at
            # core and half boundaries
            q = b0
            col = 0
            while q < b0 + nb:
                cc, tt = q // NT, q % NT
                if tt < NTA:
                    hf, tb, hw = 0, 0, NHA
                else:
                    hf, tb, hw = 1, NTA, NSHP - NHA
                run = min((NTA if tt < NTA else NT) - tt, b0 + nb - q)
                nc.sync.dma_start(
                    out=xm[:, col * 128:(col + run) * 128],
                    in_=bass.AP(h1T_all[hf][:, :, :].tensor,
                                cc * 32 * hw + (tt - tb) * 128,
                                [[hw, 32], [1, run * 128]]))
                q += run
                col += run
        # ---- batch output tiles (row = [h | as | garbage pad to RW]) ----
        # full-RW rows make the table write a single linear DMA region
        hrowB = bpool.tile([128, DB, RW], BF16, tag="hrowB")
        g0 = 0
        while g0 < nb:
            ng = min(PG, nb - g0)
            ps = dps.tile([128, PG, PST], F32, tag="ps")
            partial = (n0 + (g0 + ng) * 128) > Ntot
            if partial:
                nc.vector.memset(ps[:], 0.0)
            for t in range(ng):
                tt = g0 + t
                nn = min(128, Ntot - (n0 + tt * 128))
                nc.tensor.matmul(
                    out=bass.AP(ps[:].tensor, ps[:].offset + (t * PST),
                                [[list(ps[:].ap[0])[0], nn], [1, MC]]),
                    lhsT=xm[:, tt * 128:tt * 128 + nn],
                    rhs=wsb[:], start=True, stop=True)
            use_act = (eng_i % 2 == 1)
            eng_i += 1

            def _copy(out, in_):
                if use_act:
                    nc.scalar.activation(out=out, in_=in_, func=ActFn.Copy)
                else:
                    nc.vector.tensor_copy(out=out, in_=in_)

            # bulk copy [h | as_hi] (+ leave as_lo slot) per PSUM group
            _copy(vap(hrowB, g0 * RW, [[RW, ng], [1, FD + 4]]),
                  vap(ps, 0, [[PST, ng], [1, FD + 4]]))
            g0 += ng
        # table rows per chunk tensor; split at chunk boundary
        t0 = 0
        while t0 < nb:
            gn0 = n0 + t0 * 128
            rows = min(128, Ntot - gn0)
            k = gn0 // CHN
            avail = (k + 1) * CHN - gn0
            if avail >= rows:
                if rows == 128:
                    run = min(nb - t0, avail // 128)
                else:
                    run = 1
                r0 = gn0 - k * CHN
                nc.sync.dma_start(
                    out=bass.AP(table[k][:, :].tensor, r0 * RW,
                                [[RW, rows], [RW * 128 if run > 1 else 1, run],
                                 [1, RW]])
                    if run > 1 else
                    bass.AP(table[k][:, :].tensor, r0 * RW,
                            [[RW, rows], [1, RW]]),
                    in_=hrowB[:, t0:t0 + run, 0:RW] if run > 1
                    else hrowB[0:rows, t0, 0:RW])
                t0 += run
            else:
                # tile straddles the chunk boundary: split by partition range
                nsplit = avail
                r0 = gn0 - k * CHN
                nc.sync.dma_start(
                    out=bass.AP(table[k][:, :].tensor, r0 * RW,
                                [[RW, nsplit], [1, RW]]),
                    in_=hrowB[0:nsplit, t0, 0:RW])
                nc.sync.dma_start(
                    out=bass.AP(table[k + 1][:, :].tensor, 0,
                                [[RW, rows - nsplit], [1, RW]]),
                    in_=hrowB[nsplit:rows, t0, 0:RW])
                t0 += 1
        b0 += nb


def _edge_pass(nc, tc, meta, layer, sidx, aidx, table, stg, ad_core, pools):
    rowbase = meta["rowbase"]
    grid_rows = meta["grid_rows"]
    RW = 256                              # table row elems (bf16)
    FD = 128 if layer == 1 else 64        # feature elems

    # stream offset of each bucket's first slot (buckets are contiguous)
    bstart = {}
    for (k, bi, t, c0, w, off) in meta["calls"]:
        bstart.setdefault((k, bi), off)

    gp, ap_pool, sop, wp = pools
    if True:
        abase = 0
        for k in range(NCH):
            for bi, D in enumerate(BUCKETS):
                Tb = int(meta["T"][k][bi])
                if Tb == 0:
                    continue
                rb0 = rowbase[(k, bi, 0)]
                # per-bucket ad gather (<=2048-idx calls)
                ADG = ap_pool.tile([128, Tb, 64], F32, tag="ADG")
                na = Tb * 128
                o = 0
                while o < na:
                    nbv = min(MAXD, na - o)
                    gv = 128 if "adgather_small" in SKIP else nbv
                    nc.gpsimd.dma_gather(
                        ADG[:, o // 128:(o + gv) // 128, :], ad_core[:, :],
                        aidx[:, (abase + rb0 + o) // 16:
                                (abase + rb0 + o + gv) // 16],
                        gv, gv, 64, queue_num=qn())
                    o += nbv
                # per-bucket f32 accumulator + bf16 staging copy
                fsB = sop.tile([128, Tb, FD + 4], F32, tag="fsB")
                soB = sop.tile([128, Tb, FD + 4], BF16, tag="soB")
                # segments of <= CAP cols (tile-aligned); gathered in <= SEG
                # col calls, processed per segment to amortize DVE dispatch
                gt = max(1, CAP // D)      # tiles per segment
                t0 = 0
                off = bstart[(k, bi)]
                while t0 < Tb:
                    gn = min(gt, Tb - t0)
                    ncols = gn * D
                    G = gp.tile([128, max(CAP, D), RW], BF16, tag="G")
                    so = off + 128 * (t0 * D)
                    c = 0
                    while c < ncols:
                        w = min(SEG, ncols - c)
                        gw = 1 if "egather_small" in SKIP else w
                        nc.gpsimd.dma_gather(
                            G[:, c:c + gw, :], table[k][:, :],
                            sidx[:, (so + 128 * c) // 16:
                                    (so + 128 * (c + gw)) // 16],
                            128 * gw, 128 * gw, RW, queue_num=qn())
                        c += w
                    # e = exp(leaky(as_hi + as_lo + ad))  [f32]
                    e = wp.tile([128, max(CAP, D) * 4], F32, tag="e")
                    ebf = wp.tile([128, max(CAP, D) * 4], BF16, tag="ebf")
                    if "eops" not in SKIP:
                        nc.vector.tensor_tensor(
                            out=e[:, 0:ncols * 4],
                            in0=vap(G, FD, [[RW, ncols], [1, 4]]),
                            in1=bass.AP(ADG[:].tensor,
                                        ADG[:].offset + t0 * 64,
                                        [list(ADG[:].ap[0]), [64, gn], [0, D],
                                         [1, 4]]),
                            op=AluOp.add)
                        nc.vector.scalar_tensor_tensor(
                            out=e[:, 0:ncols * 4], in0=e[:, 0:ncols * 4],
                            scalar=0.2, in1=e[:, 0:ncols * 4],
                            op0=AluOp.mult, op1=AluOp.max)
                        # single bf16 exp serves weights AND denominators
                        nc.scalar.activation(out=ebf[:, 0:ncols * 4],
                                             in_=e[:, 0:ncols * 4],
                                             func=ActFn.Exp)
                        # denominators (f32 accumulate)
                        nc.vector.tensor_reduce(
                            out=bass.AP(fsB[:].tensor,
                                        fsB[:].offset + t0 * (FD + 4) + FD,
                                        [list(fsB[:].ap[0]), [FD + 4, gn],
                                         [1, 4]]),
                            in_=vap(ebf, 0, [[4 * D, gn], [1, 4], [4, D]]),
                            axis=Axis.X, op=AluOp.add)
                    # segment-fused weighted features
                    val = wp.tile([128, max(CAP, D) * FD], BF16, tag="val")
                    if "val" not in SKIP:
                        nc.vector.tensor_tensor(
                            out=vap(val, 0, [[D * FD, gn], [FD, D],
                                             [FD // 4, 4], [1, FD // 4]]),
                            in0=vap(G, 0, [[RW * D, gn], [RW, D],
                                           [FD // 4, 4], [1, FD // 4]]),
                            in1=vap(ebf, 0, [[4 * D, gn], [4, D],
                                             [1, 4], [0, FD // 4]]),
                            op=AluOp.mult)
                        nc.vector.tensor_reduce(
                            out=bass.AP(fsB[:].tensor,
                                        fsB[:].offset + t0 * (FD + 4),
                                        [list(fsB[:].ap[0]), [FD + 4, gn],
                                         [1, FD]]),
                            in_=vap(val, 0, [[D * FD, gn], [1, FD], [FD, D]]),
                            axis=Axis.X, op=AluOp.add)
                    t0 += gn
                # one bf16 round + one staging write per bucket
                nc.vector.tensor_copy(out=soB[:], in_=fsB[:])
                nc.sync.dma_start(
                    out=bass.AP(stg[k][:, :].tensor, rb0 * RW,
                                [[RW, 128], [RW * 128, Tb], [1, FD + 4]]),
                    in_=soB[:])
            abase += grid_rows[k]


def _merge_pass(nc, tc, meta, layer, midx, stg, bsb, out2, h1T_sh,
                wadsb, adc, nta=None, h1T_all=None):
    RW = 256
    FD = 128 if layer == 1 else 64
    OD = HID if layer == 1 else OUT_DIM
    W = FD + 4
    if layer == 1:
        ranges = [(0, nta, 0), (nta, NT, 1)]
    else:
        ranges = [(0, NT, 0)]

    with (tc.tile_pool(name=f"mi{layer}", bufs=1) as ip,
          tc.tile_pool(name=f"mg{layer}", bufs=2) as gp,
          tc.tile_pool(name=f"ms{layer}", bufs=2) as sp_pool,
          tc.tile_pool(name=f"mw{layer}", bufs=2) as wp,
          tc.tile_pool(name=f"mp{layer}", bufs=2, space="PSUM") as pp):
        if layer == 1:
            from concourse.masks import make_identity
            ident = ip.tile([128, 128], F32, tag="ident")
            make_identity(nc, ident[:])

        for (t_lo, t_hi, hf) in ranges:
            _merge_range(nc, meta, layer, midx, stg, bsb, out2,
                         h1T_sh[hf] if layer == 1 else None,
                         wadsb, adc, gp, sp_pool, wp, pp,
                         ident if layer == 1 else None,
                         t_lo, t_hi, RW, FD, OD, W)
        if layer == 1:
            for hf in range(2):
                if "ag_small" in SKIP:
                    nc.gpsimd.collective_compute(
                        "AllGather", AluOp.bypass,
                        replica_groups=[list(range(NCORES))],
                        ins=[h1T_sh[hf][:, 0:64]],
                        outs=[h1T_all[hf][:, :, 0:64]])
                else:
                    nc.gpsimd.collective_compute(
                        "AllGather", AluOp.bypass,
                        replica_groups=[list(range(NCORES))],
                        ins=[h1T_sh[hf][:, :]], outs=[h1T_all[hf][:, :, :]])


def _merge_range(nc, meta, layer, midx, stg, bsb, out2, h1T_sh, wadsb, adc,
                 gp, sp_pool, wp, pp, ident, t_lo, t_hi, RW, FD, OD, W):
        mt = t_lo
        while mt < t_hi:
            nb = min(MB, t_hi - mt)
            s = sp_pool.tile([128, MB * W], F32, tag="s")
            s01 = wp.tile([128, MB * W], BF16, tag="s01")
            Gprev = None
            for k in range(NCH):
                Gk = gp.tile([128, MB, RW], BF16, tag="MG")
                ioff = k * NSHP + mt * 128
                o = 0
                while o < nb * 128:
                    nbv = min(MAXD, nb * 128 - o)
                    gv = 128 if "mgather_small" in SKIP else nbv
                    nc.gpsimd.dma_gather(
                        Gk[:, o // 128:(o + gv) // 128, :], stg[k][:, :],
                        midx[:, (ioff + o) // 16:(ioff + o + gv) // 16],
                        gv, gv, RW, queue_num=qn())
                    o += nbv
                if k == 1:
                    # bf16 pair-add runs in the DVE 2x fast mode
                    with nc.allow_low_precision(reason="bf16 staged pair"):
                        nc.vector.tensor_tensor(
                            out=s01[:, 0:nb * W],
                            in0=vap(Gprev, 0, [[RW, nb], [1, W]]),
                            in1=vap(Gk, 0, [[RW, nb], [1, W]]), op=AluOp.add)
                elif k == 2:
                    nc.vector.tensor_tensor(
                        out=vap(s, 0, [[W, nb], [1, W]]),
                        in0=s01[:, 0:nb * W],
                        in1=vap(Gk, 0, [[RW, nb], [1, W]]), op=AluOp.add)
                elif k == 3:
                    nc.vector.tensor_tensor(
                        out=vap(s, 0, [[W, nb], [1, W]]),
                        in0=vap(s, 0, [[W, nb], [1, W]]),
                        in1=vap(Gk, 0, [[RW, nb], [1, W]]), op=AluOp.add)
                Gprev = Gk
            rec = wp.tile([128, MB * 4], F32, tag="rec")
            nc.vector.tensor_scalar_add(
                out=vap(rec, 0, [[4, nb], [1, 4]]),
                in0=vap(s, FD, [[W, nb], [1, 4]]), scalar1=EPS)
            nc.vector.reciprocal(out=rec[:, 0:nb * 4], in_=rec[:, 0:nb * 4])
            nc.vector.tensor_scalar_mul(out=rec[:, 0:nb * 4],
                                        in0=rec[:, 0:nb * 4], scalar1=0.25)
            sc = wp.tile([128, MB * FD], F32, tag="sc")
            nc.vector.tensor_tensor(
                out=vap(sc, 0, [[FD, nb], [FD // 4, 4], [1, FD // 4]]),
                in0=vap(s, 0, [[W, nb], [FD // 4, 4], [1, FD // 4]]),
                in1=vap(rec, 0, [[4, nb], [1, 4], [0, FD // 4]]),
                op=AluOp.mult)
            hs = wp.tile([128, MB * OD], F32, tag="hs")
            nc.vector.tensor_reduce(
                out=vap(hs, 0, [[OD, nb], [1, OD]]),
                in_=vap(sc, 0, [[FD, nb], [1, OD], [OD, 4]]),
                axis=Axis.X, op=AluOp.add)
            nc.vector.tensor_tensor(
                out=vap(hs, 0, [[OD, nb], [1, OD]]),
                in0=vap(hs, 0, [[OD, nb], [1, OD]]),
                in1=vap(bsb, 0, [[0, nb], [1, OD]]), op=AluOp.add)
            if layer == 1:
                nc.scalar.activation(out=hs[:, 0:nb * OD], in_=hs[:, 0:nb * OD],
                                     func=ActFn.Relu)
                hsbB = wp.tile([32, MB * 128], BF16, tag="hsbB")
                ti = 0
                while ti < nb:
                    jn = min(4, nb - ti)
                    psT = pp.tile([32, 4, 128], F32, tag="psT")
                    for j in range(jn):
                        nc.tensor.transpose(
                            out=psT[:, j, :],
                            in_=hs[:, (ti + j) * OD:(ti + j + 1) * OD],
                            identity=ident[:])
                    nc.vector.tensor_copy(
                        out=hsbB[:, ti * 128:(ti + jn) * 128],
                        in_=psT[:, 0:jn, :])
                    ti += jn
                nc.scalar.dma_start(
                    out=h1T_sh[:, (mt - t_lo) * 128:(mt - t_lo + nb) * 128],
                    in_=hsbB[:, 0:nb * 128])
                # ad2 for next layer: h1 @ Wad2, straight into ad2c
                psA = pp.tile([128, MB, 4], F32, tag="psA2")
                for ti in range(nb):
                    nc.tensor.matmul(
                        out=bass.AP(psA[:].tensor, psA[:].offset + ti * 4,
                                    [[list(psA[:].ap[0])[0], 128], [1, 4]]),
                        lhsT=hsbB[:, ti * 128:(ti + 1) * 128],
                        rhs=wadsb[:], start=True, stop=True)
                adt = wp.tile([128, MB * 4], F32, tag="adt2")
                nc.vector.tensor_copy(out=adt[:, 0:nb * 4],
                                      in_=psA[:, 0:nb, :])
                nc.scalar.dma_start(
                    out=bass.AP(adc[:, :].tensor, mt * 128 * 64,
                                [[64, 128], [64 * 128, nb], [1, 4]]),
                    in_=vap(adt, 0, [[4, nb], [1, 4]]))
            else:
                nc.sync.dma_start(
                    out=bass.AP(out2[:, :].tensor, mt * 128 * OD,
                                [[OD, 128], [OD * 128, nb], [1, OD]]),
                    in_=vap(hs, 0, [[OD, nb], [1, OD]]))
            mt += nb


_CACHE = {}


def kernel(**inputs):
    in_maps, meta = host_prep(**inputs)
    key = str(meta["T"])
    _CACHE["k"] = key
    if key not in _CACHE:
        nc = build_nc(meta)
        _CACHE[key] = (nc, make_runner(nc, NCORES))
    nc, run = _CACHE[key]
    results, best = run(in_maps, repeats=1)
    _CACHE["last_time"] = best
    out = np.empty((N, OUT_DIM), np.float32)
    for c in range(NCORES):
        out[c * NSH:(c + 1) * NSH] = results[c]["out2"][:NSH]
    return out



# revision 39
# speedup vs baseline: 2.9445x; 1.5269x over previous
"""GAT 2-layer kernel for TRN2, 8 NeuronCores (self-contained).

Strategy:
- dst-shard: core c owns nodes [c*12500, (c+1)*12500).
- ad pre-pass: own-shard x @ fold(W, a_dst) -> ad1c, so edge pipelines
  never wait on the full dense pass.
- Dense phases (x@W bf16, h1@W2 bf16) replicated on all cores; folded
  a_src gives per-node [h | as] rows in one matmul. 16-tile DMA batches
  (HWDGE calls are ~600ns each on one shared device), bank-sized PSUM
  groups, copy work split across DVE/ACT.
- Edge phase per core: 4 src-chunks (25000 nodes, int16 gather reach),
  per-chunk degree-bucketed padded CSR over dst. 1024-index single-queue
  SWDGE gathers (multi-queue under TileContext races; >16KB rings hang
  HW) pull [h | as] rows. e = exp(leaky(as+ad)) on ACT; segment-fused
  bf16 weighted-feature mult + f32 reduce into per-bucket tiles.
- Per-chunk partials [featsum | denom] -> DRAM staging (bf16); per-chunk
  table tensors + concurrently-open tile pools let edge chunks overlap
  the dense tail.
- Merge pass (natural node order): per-chunk 16-tile gathers, f32
  accumulate, per-head normalize, bias (+relu L1); L1 also computes
  ad2c (h1 @ fold(W2, a2_dst)) from the transposed tiles.
- L1->L2: h1T shard AllGather in two halves (overlaps merge tail and
  dense2 head) -> replicated dense2.
"""
import sys
sys.path.insert(0, "/opt/trn_rl_repo")
import numpy as np
import ml_dtypes

import concourse.bass as bass
import concourse.bacc as bacc
import concourse.tile as tile
from concourse import mybir
from concourse.library_config import mlp as mlp_lib


def make_runner(nc, n_cores):
    """PJRT runner: returns run_fn(in_maps, repeats) -> (results, best_time_s)."""
    import time
    import jax
    from jax.sharding import Mesh, PartitionSpec, NamedSharding
    from jax.experimental.shard_map import shard_map
    from concourse.bass2jax import (_bass_exec_p, install_neuronx_cc_hook,
                                    partition_id_tensor)
    install_neuronx_cc_hook()
    partition_name = nc.partition_id_tensor.name if nc.partition_id_tensor else None
    in_names, out_names, out_avals, zero_outs = [], [], [], []
    for alloc in nc.m.functions[0].allocations:
        if not isinstance(alloc, mybir.MemoryLocationSet):
            continue
        if not alloc.memorylocations:
            continue
        name = alloc.memorylocations[0].name
        if alloc.kind == "ExternalInput":
            if name != partition_name:
                in_names.append(name)
        elif alloc.kind == "ExternalOutput":
            out_names.append(name)
            shape = tuple(alloc.tensor_shape)
            dtype = mybir.dt.np(alloc.dtype)
            out_avals.append(jax.core.ShapedArray(shape, dtype))
            zero_outs.append(np.zeros(shape, dtype))
    n_params = len(in_names)
    n_outs = len(out_avals)
    all_in_names = list(in_names) + list(out_names)
    if partition_name is not None:
        all_in_names.append(partition_name)

    def _body(*args):
        operands = list(args)
        if partition_name is not None:
            operands.append(partition_id_tensor())
        return tuple(_bass_exec_p.bind(
            *operands, out_avals=tuple(out_avals), in_names=tuple(all_in_names),
            out_names=tuple(out_names), lowering_input_output_aliases=(),
            sim_require_finite=False, sim_require_nnan=False, nc=nc))

    devices = jax.devices()[:n_cores]
    mesh = Mesh(np.asarray(devices), ("core",))
    in_specs = (PartitionSpec("core"),) * (n_params + n_outs)
    out_specs = (PartitionSpec("core"),) * n_outs
    donate = tuple(range(n_params, n_params + n_outs))
    sharded = jax.jit(
        shard_map(_body, mesh=mesh, in_specs=in_specs, out_specs=out_specs,
                  check_rep=False),
        donate_argnums=donate, keep_unused=True)

    def run_fn(in_maps, repeats=1):
        per_core = [[np.asarray(m[name]) for name in in_names] for m in in_maps]
        concat_in = [np.concatenate([per_core[c][i] for c in range(n_cores)], 0)
                     for i in range(n_params)]
        sharding = NamedSharding(mesh, PartitionSpec("core"))
        dev_in = [jax.device_put(a, sharding) for a in concat_in]
        for a in dev_in:
            a.block_until_ready()
        times, out_arrs = [], None
        for _ in range(repeats):
            concat_zeros = [jax.device_put(
                np.zeros((n_cores * z.shape[0], *z.shape[1:]), z.dtype), sharding)
                for z in zero_outs]
            for z in concat_zeros:
                z.block_until_ready()
            t0 = time.perf_counter()
            out_arrs = sharded(*dev_in, *concat_zeros)
            for o in out_arrs:
                o.block_until_ready()
            times.append(time.perf_counter() - t0)
        results = [
            {name: np.asarray(out_arrs[i]).reshape(n_cores, *out_avals[i].shape)[c]
             for i, name in enumerate(out_names)}
            for c in range(n_cores)]
        return results, min(times)

    return run_fn

F32 = mybir.dt.float32
BF16 = mybir.dt.bfloat16
I16 = mybir.dt.int16
BF = ml_dtypes.bfloat16

NCORES = 8
N = 100000
IN_DIM = 128
HID = 32
OUT_DIM = 16
NSH = N // NCORES            # 12500
NT = 98                      # merge tiles per core
NSHP = NT * 128              # 12544
CH1 = 25000                  # table1 nodes per chunk
NCH = 4
CH1R = CH1 + 1               # +pad row
R2 = NCORES * NSHP           # 100352
CH2 = R2 // NCH              # 25088 (= 2 padded cores)
CH2R = CH2 + 1
BUCKETS = (1, 2, 3, 4, 5, 6, 8, 16)
NEG = -1.0e30
EPS = 1e-16
MAXD = 1024                  # max descriptors per SWDGE call (HW ring limit)
RING = 16384                 # SWDGE descriptor ring bytes (>16KB hangs HW)
NQ = 4                       # SWDGE queues (measured: 1q=92GB/s, 4q=450GB/s)
_QRR = [0]


def qn():
    _QRR[0] = (_QRR[0] + 1) % NQ
    return _QRR[0]
SEG = MAXD // 128            # max gather cols per SWDGE call
CAP = 32                     # edge-pass segment cols (DVE op granularity)
DB = 24                      # dense batch (tiles per DMA)
PG = 3                       # dense PSUM group (tiles per PSUM bank tile)
MB = 8                       # merge batch (tiles)
NTA = 49                     # merge tiles in first allgather half
NHA = NTA * 128              # 6272

AluOp = mybir.AluOpType
ActFn = mybir.ActivationFunctionType
Axis = mybir.AxisListType

SKIP = set()                 # timing-sensitivity knobs (empty in production)


def _colgroups(D):
    out = []
    c = 0
    while c < D:
        w = min(8, D - c)
        out.append((c, w))
        c += w
    return out


def plan_segments(T):
    """Host/device shared slot-stream layout (order: k, bucket, tile, col, p)."""
    calls = []
    rowbase = {}
    grid_rows = []
    off = 0
    for k in range(NCH):
        rb = 0
        for bi, D in enumerate(BUCKETS):
            for t in range(int(T[k][bi])):
                rowbase[(k, bi, t)] = rb
                for (c0, w) in _colgroups(D):
                    calls.append((k, bi, t, c0, w, off))
                    off += 128 * w
                rb += 128
        grid_rows.append(rb)
    return calls, rowbase, grid_rows, off


def _wrap_idx(flat):
    n = len(flat)
    assert n % 16 == 0
    w = np.asarray(flat, np.int16).reshape(n // 16, 16).T
    return np.ascontiguousarray(np.tile(w, (8, 1)))


def fold(W, a):
    Hh, F = a.shape
    w = np.zeros((W.shape[0], Hh), np.float32)
    for h in range(Hh):
        w[:, h] = W[:, h * F:(h + 1) * F] @ a[h]
    return w


def host_prep(x, edge_index, W1, a1_src, a1_dst, b1, W2, a2_src, a2_dst, b2):
    x = np.asarray(x, np.float32)
    ei = np.asarray(edge_index)
    src = ei[0].astype(np.int64)
    dst = ei[1].astype(np.int64)
    W1 = np.asarray(W1, np.float32)
    W2 = np.asarray(W2, np.float32)
    Waug1 = np.concatenate([W1, fold(W1, np.asarray(a1_src, np.float32))], 1)
    Waug2 = np.concatenate([W2, fold(W2, np.asarray(a2_src, np.float32))], 1)
    Wad1 = fold(W1, np.asarray(a1_dst, np.float32))          # [128, 4]
    Wad2 = fold(W2, np.asarray(a2_dst, np.float32))          # [32, 4]
    xT = np.ascontiguousarray(x.T.astype(BF))

    core_of = dst // NSH
    # ---- per-core, per-chunk CSR ----
    pc = []  # [core][chunk] = (deg, sorted_src_by_dst, starts)
    for c in range(NCORES):
        m = core_of == c
        s_c, d_c = src[m], dst[m] - c * NSH
        ch = s_c // CH1
        info = []
        for k in range(NCH):
            mk = ch == k
            sk, dk = s_c[mk], d_c[mk]
            deg = np.bincount(dk, minlength=NSH)
            order = np.argsort(dk, kind="stable")
            sk = sk[order]
            starts = np.zeros(NSH + 1, np.int64)
            np.cumsum(deg, out=starts[1:])
            info.append((deg, sk, starts))
        pc.append(info)

    # shared tile counts
    T = [[0] * len(BUCKETS) for _ in range(NCH)]
    for c in range(NCORES):
        for k in range(NCH):
            deg = pc[c][k][0]
            for bi, D in enumerate(BUCKETS):
                lo = BUCKETS[bi - 1] if bi else 0
                nb = int(((deg > lo) & (deg <= D)).sum())
                T[k][bi] = max(T[k][bi], (nb + 127) // 128)
            assert deg.max(initial=0) <= BUCKETS[-1], f"deg max {deg.max()}"
    calls, rowbase, grid_rows, stream_len = plan_segments(T)

    b1rep = np.tile(np.asarray(b1, np.float32)[None, :], (128, 1))
    b2rep = np.tile(np.asarray(b2, np.float32)[None, :], (128, 1))
    pad1 = np.zeros((1, 256), BF); pad1[0, 128:132] = NEG
    pad2 = np.zeros((1, 256), BF); pad2[0, 64:68] = NEG
    z256 = np.zeros((1, 256), BF)

    in_maps = []
    for c in range(NCORES):
        slot_nodes = []   # per chunk: grid row -> node (or -1)
        for k in range(NCH):
            gr = grid_rows[k]
            deg, sk, starts = pc[c][k]
            nodes_of = np.full(gr, -1, np.int64)
            for bi, D in enumerate(BUCKETS):
                lo = BUCKETS[bi - 1] if bi else 0
                nd = np.where((deg > lo) & (deg <= D))[0]
                rb = rowbase[(k, bi, 0)] if T[k][bi] else 0
                nodes_of[rb:rb + len(nd)] = nd
            slot_nodes.append(nodes_of)

        s1 = np.full(stream_len, CH1, np.int64)     # pad -> table1 chunk pad row
        s2 = np.full(stream_len, CH2, np.int64)     # pad -> table2 chunk pad row
        for (k, bi, t, c0, w, off) in calls:
            D = BUCKETS[bi]
            rb = rowbase[(k, bi, t)]
            deg, sk, starts = pc[c][k]
            nodes = slot_nodes[k][rb:rb + 128]
            j = off
            for d in range(c0, c0 + w):
                for p in range(128):
                    nd = nodes[p]
                    if nd >= 0 and d < starts[nd + 1] - starts[nd]:
                        s = sk[starts[nd] + d]
                        s1[j] = s % CH1
                        s2[j] = (s // NSH % 2) * NSHP + s % NSH
                    j += 1
        slot1w = _wrap_idx(s1)
        slot2w = _wrap_idx(s2)

        # ad idx: per (k, gridtile) 128 local dst ids (pad -> 0)
        adix = []
        for k in range(NCH):
            nd = slot_nodes[k]
            adix.append(np.where(nd >= 0, nd, 0))
        adw = _wrap_idx(np.concatenate(adix)) if stream_len else None

        # merge idx: per chunk, per natural node (padded to NSHP): grid row or zero-row
        mrg = []
        for k in range(NCH):
            deg = pc[c][k][0]
            pos = np.full(NSHP, grid_rows[k], np.int64)  # zero row
            nd = slot_nodes[k]
            real = nd >= 0
            pos[nd[real]] = np.nonzero(real)[0]
            mrg.append(pos)
        mrgw = _wrap_idx(np.concatenate(mrg))

        in_maps.append(dict(
            xT=xT, Waug1=Waug1.astype(BF), Waug2=Waug2.astype(BF),
            Wad1=Wad1.astype(BF), Wad2=Wad2.astype(BF),
            b1rep=b1rep, b2rep=b2rep, pad1=pad1, pad2=pad2, z256=z256,
            slot1w=slot1w, slot2w=slot2w, adw=adw, mrgw=mrgw,
        ))
    meta = dict(T=T, calls=calls, rowbase=rowbase, grid_rows=grid_rows,
                stream_len=stream_len)
    return in_maps, meta


def vap(t, off, dims):
    a = t[:]
    return bass.AP(a.tensor, a.offset + off, [list(a.ap[0])] + [list(d) for d in dims])


def build_nc(meta):
    _QRR[0] = 0
    T = meta["T"]
    grid_rows = meta["grid_rows"]
    stream_len = meta["stream_len"]
    SW = stream_len // 16
    ADL = sum(grid_rows)
    AW = ADL // 16
    MW = (NCH * NSHP) // 16

    nc = bacc.Bacc("TRN2", target_bir_lowering=False, num_swdge_queues=NQ,
                   dynamic_dma_scratch_size=RING)
    dp = nc.declare_dram_parameter
    xT = dp("xT", [IN_DIM, N], BF16, isOutput=False)
    Waug1 = dp("Waug1", [128, 132], BF16, isOutput=False)
    Waug2 = dp("Waug2", [32, 68], BF16, isOutput=False)
    Wad1 = dp("Wad1", [128, 4], BF16, isOutput=False)
    Wad2 = dp("Wad2", [32, 4], BF16, isOutput=False)
    b1rep = dp("b1rep", [128, HID], F32, isOutput=False)
    b2rep = dp("b2rep", [128, OUT_DIM], F32, isOutput=False)
    pad1 = dp("pad1", [1, 256], BF16, isOutput=False)
    pad2 = dp("pad2", [1, 256], BF16, isOutput=False)
    z256 = dp("z256", [1, 256], BF16, isOutput=False)
    slot1w = dp("slot1w", [128, SW], I16, isOutput=False)
    slot2w = dp("slot2w", [128, SW], I16, isOutput=False)
    adw = dp("adw", [128, AW], I16, isOutput=False)
    mrgw = dp("mrgw", [128, MW], I16, isOutput=False)
    out2 = dp("out2", [NSHP, OUT_DIM], F32, isOutput=True)

    table1 = [nc.dram_tensor(f"table1_{k}", [CH1R, 256], BF16)
              for k in range(NCH)]
    table2 = [nc.dram_tensor(f"table2_{k}", [CH2R, 256], BF16)
              for k in range(NCH)]
    ad1c = nc.dram_tensor("ad1c", [NSHP, 256], BF16)
    ad2c = nc.dram_tensor("ad2c", [NSHP, 256], BF16)
    stg1 = [nc.dram_tensor(f"stg1_{k}", [grid_rows[k] + 1, 256], BF16)
            for k in range(NCH)]
    stg2 = [nc.dram_tensor(f"stg2_{k}", [grid_rows[k] + 1, 256], BF16)
            for k in range(NCH)]
    h1T_sh = [nc.dram_tensor("h1T_shA", [32, NHA], BF16),
              nc.dram_tensor("h1T_shB", [32, NSHP - NHA], BF16)]
    h1T_all = [nc.dram_tensor("h1T_allA", [NCORES, 32, NHA], BF16,
                              addr_space="Shared"),
               nc.dram_tensor("h1T_allB", [NCORES, 32, NSHP - NHA], BF16,
                              addr_space="Shared")]

    with tile.TileContext(nc) as tc:
        nc.gpsimd.load_library(mlp_lib)

        # ---------- consts / pads ----------
        with tc.tile_pool(name="konst", bufs=1) as kp:
            w1sb = kp.tile([128, 132], BF16)
            nc.sync.dma_start(out=w1sb[:], in_=Waug1[:, :])
            w2sb = kp.tile([32, 68], BF16)
            nc.sync.dma_start(out=w2sb[:], in_=Waug2[:, :])
            wad1sb = kp.tile([128, 4], BF16)
            nc.sync.dma_start(out=wad1sb[:], in_=Wad1[:, :])
            wad2sb = kp.tile([32, 4], BF16)
            nc.sync.dma_start(out=wad2sb[:], in_=Wad2[:, :])
            # edge/merge index streams, loaded up front so the edge pipelines
            # can start as soon as their table chunks are written
            SW = stream_len // 16
            AW = sum(grid_rows) // 16
            MW = (NCH * NSHP) // 16
            sidx2 = kp.tile([128, SW], I16)
            nc.sync.dma_start(out=sidx2[:], in_=slot2w[:, :])
            aidx = kp.tile([128, AW], I16)
            nc.sync.dma_start(out=aidx[:], in_=adw[:, :])
            midx = kp.tile([128, MW], I16)
            nc.sync.dma_start(out=midx[:], in_=mrgw[:, :])
            b1sb = kp.tile([128, HID], F32)
            nc.sync.dma_start(out=b1sb[:], in_=b1rep[:, :])
            b2sb = kp.tile([128, OUT_DIM], F32)
            nc.sync.dma_start(out=b2sb[:], in_=b2rep[:, :])
            for k in range(NCH):
                nc.sync.dma_start(out=table1[k][CH1, :], in_=pad1[0, :])
                nc.sync.dma_start(out=table2[k][CH2, :], in_=pad2[0, :])
                nc.sync.dma_start(out=stg1[k][grid_rows[k], :], in_=z256[0, :])
                nc.sync.dma_start(out=stg2[k][grid_rows[k], :], in_=z256[0, :])

            # ---------- ad1 pre-pass: own-shard x @ Wad1 -> ad1c ----------
            pid = nc.sync.partition_id()
            with (nc.named_scope("ad1pass"),
                  tc.tile_pool(name="a1", bufs=2) as apool,
                  tc.tile_pool(name="a1p", bufs=2, space="PSUM") as aps):
                ADB = 16
                base = pid * NSH
                b0 = 0
                while b0 < NT:
                    nb = min(ADB, NT - b0)
                    ncol = min(nb * 128, NSH - b0 * 128)
                    xmA = apool.tile([128, ADB * 128], BF16, tag="xmA")
                    nc.sync.dma_start(
                        out=xmA[:, 0:ncol],
                        in_=xT[:, bass.ds(base + b0 * 128, ncol)])
                    psA = aps.tile([128, ADB, 4], F32, tag="psA")
                    if ncol < nb * 128:
                        nc.vector.memset(psA[:], 0.0)
                    for t in range(nb):
                        nn = min(128, ncol - t * 128)
                        if nn <= 0:
                            break
                        nc.tensor.matmul(
                            out=bass.AP(psA[:].tensor, psA[:].offset + t * 4,
                                        [[list(psA[:].ap[0])[0], nn], [1, 4]]),
                            lhsT=xmA[:, t * 128:t * 128 + nn],
                            rhs=wad1sb[:], start=True, stop=True)
                    adt = apool.tile([128, ADB, 256], BF16, tag="adt")
                    nc.vector.tensor_copy(out=vap(adt, 0, [[256, nb], [1, 4]]),
                                          in_=psA[:, 0:nb, :])
                    nc.sync.dma_start(
                        out=bass.AP(ad1c[:, :].tensor, b0 * 128 * 256,
                                    [[256, 128], [256 * 128, nb], [1, 256]]),
                        in_=adt[:, 0:nb, :])
                    b0 += nb

            # ---------- dense1 + edge1 (pools coexist so both overlap) ----
            # sidx1 lives only in this block so its 16KB frees for layer 2
            with (tc.tile_pool(name="ix1", bufs=1) as ixp1,
                  tc.tile_pool(name="eg1", bufs=2) as gp1,
                  tc.tile_pool(name="ea1", bufs=2) as ap1,
                  tc.tile_pool(name="eso1", bufs=2) as sop1,
                  tc.tile_pool(name="ew1", bufs=2) as wp1):
                sidx1 = ixp1.tile([128, SW], I16)
                nc.sync.dma_start(out=sidx1[:], in_=slot1w[:, :])
                with (nc.named_scope("dense1"),
                      tc.tile_pool(name="d1", bufs=3) as dpool,
                      tc.tile_pool(name="d1b", bufs=2) as bpool,
                      tc.tile_pool(name="d1p", bufs=2, space="PSUM") as dps):
                    _dense_pass(nc, tc, dpool, bpool, dps, layer=1,
                                src=xT, wsb=w1sb, table=table1, h1T_all=None)

                # ---------- edge pass L1 ----------
                with nc.named_scope("edge1"):
                    _edge_pass(nc, tc, meta, layer=1, sidx=sidx1, aidx=aidx,
                               table=table1, stg=stg1, ad_core=ad1c,
                               pools=(gp1, ap1, sop1, wp1))

            # ---------- merge1 + dense2 + edge2 (L2 pools open early so
            # dense2 need not wait for merge1's pool region to free) ------
            with (tc.tile_pool(name="eg2", bufs=2) as gp2,
                  tc.tile_pool(name="ea2", bufs=2) as ap2,
                  tc.tile_pool(name="eso2", bufs=2) as sop2,
                  tc.tile_pool(name="ew2", bufs=2) as wp2,
                  tc.tile_pool(name="d2", bufs=3) as dpool2,
                  tc.tile_pool(name="d2b", bufs=2) as bpool2,
                  tc.tile_pool(name="d2p", bufs=4, space="PSUM") as dps2):
                with nc.named_scope("merge1"):
                    _merge_pass(nc, tc, meta, layer=1, midx=midx, stg=stg1,
                                bsb=b1sb, out2=None, h1T_sh=h1T_sh,
                                wadsb=wad2sb, adc=ad2c, nta=NTA,
                                h1T_all=h1T_all)

                with nc.named_scope("dense2"):
                    _dense_pass(nc, tc, dpool2, bpool2, dps2, layer=2,
                                src=None, wsb=w2sb, table=table2,
                                h1T_all=h1T_all)

                # ---------- edge pass L2 ----------
                with nc.named_scope("edge2"):
                    _edge_pass(nc, tc, meta, layer=2, sidx=sidx2, aidx=aidx,
                               table=table2, stg=stg2, ad_core=ad2c,
                               pools=(gp2, ap2, sop2, wp2))

            # ---------- merge L2 -> out2 ----------
            with nc.named_scope("merge2"):
                _merge_pass(nc, tc, meta, layer=2, midx=midx, stg=stg2,
                            bsb=b2sb, out2=out2, h1T_sh=None,
                            wadsb=None, adc=None)

    nc.finalize()
    return nc


def _dense_pass(nc, tc, dpool, bpool, dps, layer, src, wsb, table, h1T_all):
    """Replicated dense phase: DB-tile batches, PG-tile PSUM groups.
    layer 1: in xT f32 [128, N] -> table1 rows [h(128)|as_hi(4)|as_lo(4)] bf16.
    layer 2: in h1T_all bf16 -> table2 rows [h(64)|as_hi|as_lo] bf16.
    """
    if layer == 1:
        ntiles, K, MC = (N + 127) // 128, 128, 132   # matmul out cols
        FD = 128
        CHN, Ntot = CH1, N
        PG, PST = 4, 512       # PSUM group; slot stride padded to a full bank
    else:
        ntiles, K, MC = R2 // 128, 32, 68
        FD = 64
        CHN, Ntot = CH2, R2
        PG, PST = 7, 68        # 7 x 272B fits one bank
    RW = 256

    b0 = 0
    eng_i = 0
    while b0 < ntiles:
        nb = min(DB, ntiles - b0)
        n0 = b0 * 128
        # ---- batched input load ----
        xm = dpool.tile([K, DB * 128], BF16, tag="xm")
        if layer == 1:
            nn = min(nb * 128, N - n0)
            nc.sync.dma_start(out=xm[:, 0:nn], in_=bass.AP(
                src[:, :].tensor, n0, [[N, K], [1, nn]]))
        else:
            # h1T_all halves [NCORES, 32, NHA/(NSHP-NHA)]; split loads at
            # core and half boundaries
            q = b0
            col = 0
            while q < b0 + nb:
                cc, tt = q // NT, q % NT
                if tt < NTA:
                    hf, tb, hw = 0, 0, NHA
                else:
                    hf, tb, hw = 1, NTA, NSHP - NHA
                run = min((NTA if tt < NTA else NT) - tt, b0 + nb - q)
                nc.sync.dma_start(
                    out=xm[:, col * 128:(col + run) * 128],
                    in_=bass.AP(h1T_all[hf][:, :, :].tensor,
                                cc * 32 * hw + (tt - tb) * 128,
                                [[hw, 32], [1, run * 128]]))
                q += run
                col += run
        # ---- batch output tiles (row = [h | as | garbage pad to RW]) ----
        # full-RW rows make the table write a single linear DMA region
        hrowB = bpool.tile([128, DB, RW], BF16, tag="hrowB")
        g0 = 0
        while g0 < nb:
            ng = min(PG, nb - g0)
            ps = dps.tile([128, PG, PST], F32, tag="ps")
            partial = (n0 + (g0 + ng) * 128) > Ntot
            if partial:
                nc.vector.memset(ps[:], 0.0)
            for t in range(ng):
                tt = g0 + t
                nn = min(128, Ntot - (n0 + tt * 128))
                nc.tensor.matmul(
                    out=bass.AP(ps[:].tensor, ps[:].offset + (t * PST),
                                [[list(ps[:].ap[0])[0], nn], [1, MC]]),
                    lhsT=xm[:, tt * 128:tt * 128 + nn],
                    rhs=wsb[:], start=True, stop=True)
            use_act = (eng_i % 2 == 1)
            eng_i += 1

            def _copy(out, in_):
                if use_act:
                    nc.scalar.activation(out=out, in_=in_, func=ActFn.Copy)
                else:
                    nc.vector.tensor_copy(out=out, in_=in_)

            # bulk copy [h | as_hi] (+ leave as_lo slot) per PSUM group
            _copy(vap(hrowB, g0 * RW, [[RW, ng], [1, FD + 4]]),
                  vap(ps, 0, [[PST, ng], [1, FD + 4]]))
            g0 += ng
        # table rows per chunk tensor; split at chunk boundary
        t0 = 0
        while t0 < nb:
            gn0 = n0 + t0 * 128
            rows = min(128, Ntot - gn0)
            k = gn0 // CHN
            avail = (k + 1) * CHN - gn0
            if avail >= rows:
                if rows == 128:
                    run = min(nb - t0, avail // 128)
                else:
                    run = 1
                r0 = gn0 - k * CHN
                nc.sync.dma_start(
                    out=bass.AP(table[k][:, :].tensor, r0 * RW,
                                [[RW, rows], [RW * 128 if run > 1 else 1, run],
                                 [1, RW]])
                    if run > 1 else
                    bass.AP(table[k][:, :].tensor, r0 * RW,
                            [[RW, rows], [1, RW]]),
                    in_=hrowB[:, t0:t0 + run, 0:RW] if run > 1
                    else hrowB[0:rows, t0, 0:RW])
                t0 += run
            else:
                # tile straddles the chunk boundary: split by partition range
                nsplit = avail
                r0 = gn0 - k * CHN
                nc.sync.dma_start(
                    out=bass.AP(table[k][:, :].tensor, r0 * RW,
                                [[RW, nsplit], [1, RW]]),
                    in_=hrowB[0:nsplit, t0, 0:RW])
                nc.sync.dma_start(
                    out=bass.AP(table[k + 1][:, :].tensor, 0,
                                [[RW, rows - nsplit], [1, RW]]),
                    in_=hrowB[nsplit:rows, t0, 0:RW])
                t0 += 1
        b0 += nb


def _edge_pass(nc, tc, meta, layer, sidx, aidx, table, stg, ad_core, pools):
    rowbase = meta["rowbase"]
    grid_rows = meta["grid_rows"]
    RW = 256                              # table row elems (bf16)
    FD = 128 if layer == 1 else 64        # feature elems

    # stream offset of each bucket's first slot (buckets are contiguous)
    bstart = {}
    for (k, bi, t, c0, w, off) in meta["calls"]:
        bstart.setdefault((k, bi), off)

    gp, ap_pool, sop, wp = pools
    if True:
        abase = 0
        for k in range(NCH):
            for bi, D in enumerate(BUCKETS):
                Tb = int(meta["T"][k][bi])
                if Tb == 0:
                    continue
                rb0 = rowbase[(k, bi, 0)]
                # per-bucket ad gather (512B bf16 rows, fast DMA class)
                ADG = ap_pool.tile([128, Tb, 256], BF16, tag="ADG")
                na = Tb * 128
                o = 0
                while o < na:
                    nbv = min(MAXD, na - o)
                    gv = 128 if "adgather_small" in SKIP else nbv
                    nc.gpsimd.dma_gather(
                        ADG[:, o // 128:(o + gv) // 128, :], ad_core[:, :],
                        aidx[:, (abase + rb0 + o) // 16:
                                (abase + rb0 + o + gv) // 16],
                        gv, gv, 256, queue_num=qn())
                    o += nbv
                # per-bucket f32 accumulator + bf16 staging copy
                fsB = sop.tile([128, Tb, FD + 4], F32, tag="fsB")
                soB = sop.tile([128, Tb, FD + 4], BF16, tag="soB")
                # segments of <= CAP cols (tile-aligned); gathered in <= SEG
                # col calls, processed per segment to amortize DVE dispatch
                gt = max(1, CAP // D)      # tiles per segment
                t0 = 0
                off = bstart[(k, bi)]
                while t0 < Tb:
                    gn = min(gt, Tb - t0)
                    ncols = gn * D
                    G = gp.tile([128, max(CAP, D), RW], BF16, tag="G")
                    so = off + 128 * (t0 * D)
                    c = 0
                    while c < ncols:
                        w = min(SEG, ncols - c)
                        gw = 1 if "egather_small" in SKIP else w
                        nc.gpsimd.dma_gather(
                            G[:, c:c + gw, :], table[k][:, :],
                            sidx[:, (so + 128 * c) // 16:
                                    (so + 128 * (c + gw)) // 16],
                            128 * gw, 128 * gw, RW, queue_num=qn())
                        c += w
                    # e = exp(leaky(as_hi + as_lo + ad))  [f32]
                    e = wp.tile([128, max(CAP, D) * 4], F32, tag="e")
                    ebf = wp.tile([128, max(CAP, D) * 4], BF16, tag="ebf")
                    if "eops" not in SKIP:
                        nc.vector.tensor_tensor(
                            out=e[:, 0:ncols * 4],
                            in0=vap(G, FD, [[RW, ncols], [1, 4]]),
                            in1=bass.AP(ADG[:].tensor,
                                        ADG[:].offset + t0 * 256,
                                        [list(ADG[:].ap[0]), [256, gn], [0, D],
                                         [1, 4]]),
                            op=AluOp.add)
                        nc.vector.scalar_tensor_tensor(
                            out=e[:, 0:ncols * 4], in0=e[:, 0:ncols * 4],
                            scalar=0.2, in1=e[:, 0:ncols * 4],
                            op0=AluOp.mult, op1=AluOp.max)
                        # single bf16 exp serves weights AND denominators
                        nc.scalar.activation(out=ebf[:, 0:ncols * 4],
                                             in_=e[:, 0:ncols * 4],
                                             func=ActFn.Exp)
                        # denominators (f32 accumulate)
                        nc.vector.tensor_reduce(
                            out=bass.AP(fsB[:].tensor,
                                        fsB[:].offset + t0 * (FD + 4) + FD,
                                        [list(fsB[:].ap[0]), [FD + 4, gn],
                                         [1, 4]]),
                            in_=vap(ebf, 0, [[4 * D, gn], [1, 4], [4, D]]),
                            axis=Axis.X, op=AluOp.add)
                    # segment-fused weighted features
                    val = wp.tile([128, max(CAP, D) * FD], BF16, tag="val")
                    if "val" not in SKIP:
                        nc.vector.tensor_tensor(
                            out=vap(val, 0, [[D * FD, gn], [FD, D],
                                             [FD // 4, 4], [1, FD // 4]]),
                            in0=vap(G, 0, [[RW * D, gn], [RW, D],
                                           [FD // 4, 4], [1, FD // 4]]),
                            in1=vap(ebf, 0, [[4 * D, gn], [4, D],
                                             [1, 4], [0, FD // 4]]),
                            op=AluOp.mult)
                        nc.vector.tensor_reduce(
                            out=bass.AP(fsB[:].tensor,
                                        fsB[:].offset + t0 * (FD + 4),
                                        [list(fsB[:].ap[0]), [FD + 4, gn],
                                         [1, FD]]),
                            in_=vap(val, 0, [[D * FD, gn], [1, FD], [FD, D]]),
                            axis=Axis.X, op=AluOp.add)
                    t0 += gn
                # one bf16 round + one staging write per bucket
                nc.vector.tensor_copy(out=soB[:], in_=fsB[:])
                nc.sync.dma_start(
                    out=bass.AP(stg[k][:, :].tensor, rb0 * RW,
                                [[RW, 128], [RW * 128, Tb], [1, FD + 4]]),
                    in_=soB[:])
            abase += grid_rows[k]


def _merge_pass(nc, tc, meta, layer, midx, stg, bsb, out2, h1T_sh,
                wadsb, adc, nta=None, h1T_all=None):
    RW = 256
    FD = 128 if layer == 1 else 64
    OD = HID if layer == 1 else OUT_DIM
    W = FD + 4
    if layer == 1:
        ranges = [(0, nta, 0), (nta, NT, 1)]
    else:
        ranges = [(0, NT, 0)]

    with (tc.tile_pool(name=f"mi{layer}", bufs=1) as ip,
          tc.tile_pool(name=f"mg{layer}", bufs=2) as gp,
          tc.tile_pool(name=f"ms{layer}", bufs=2) as sp_pool,
          tc.tile_pool(name=f"mw{layer}", bufs=2) as wp,
          tc.tile_pool(name=f"mp{layer}", bufs=2, space="PSUM") as pp):
        if layer == 1:
            from concourse.masks import make_identity
            ident = ip.tile([128, 128], F32, tag="ident")
            make_identity(nc, ident[:])

        for (t_lo, t_hi, hf) in ranges:
            _merge_range(nc, meta, layer, midx, stg, bsb, out2,
                         h1T_sh[hf] if layer == 1 else None,
                         wadsb, adc, gp, sp_pool, wp, pp,
                         ident if layer == 1 else None,
                         t_lo, t_hi, RW, FD, OD, W)
        if layer == 1:
            for hf in range(2):
                if "ag_small" in SKIP:
                    nc.gpsimd.collective_compute(
                        "AllGather", AluOp.bypass,
                        replica_groups=[list(range(NCORES))],
                        ins=[h1T_sh[hf][:, 0:64]],
                        outs=[h1T_all[hf][:, :, 0:64]])
                else:
                    nc.gpsimd.collective_compute(
                        "AllGather", AluOp.bypass,
                        replica_groups=[list(range(NCORES))],
                        ins=[h1T_sh[hf][:, :]], outs=[h1T_all[hf][:, :, :]])


def _merge_range(nc, meta, layer, midx, stg, bsb, out2, h1T_sh, wadsb, adc,
                 gp, sp_pool, wp, pp, ident, t_lo, t_hi, RW, FD, OD, W):
        mt = t_lo
        while mt < t_hi:
            nb = min(MB, t_hi - mt)
            s = sp_pool.tile([128, MB * W], F32, tag="s")
            s01 = wp.tile([128, MB * W], BF16, tag="s01")
            Gprev = None
            for k in range(NCH):
                Gk = gp.tile([128, MB, RW], BF16, tag="MG")
                ioff = k * NSHP + mt * 128
                o = 0
                while o < nb * 128:
                    nbv = min(MAXD, nb * 128 - o)
                    gv = 128 if "mgather_small" in SKIP else nbv
                    nc.gpsimd.dma_gather(
                        Gk[:, o // 128:(o + gv) // 128, :], stg[k][:, :],
                        midx[:, (ioff + o) // 16:(ioff + o + gv) // 16],
                        gv, gv, RW, queue_num=qn())
                    o += nbv
                if k == 1:
                    # bf16 pair-add runs in the DVE 2x fast mode
                    with nc.allow_low_precision(reason="bf16 staged pair"):
                        nc.vector.tensor_tensor(
                            out=s01[:, 0:nb * W],
                            in0=vap(Gprev, 0, [[RW, nb], [1, W]]),
                            in1=vap(Gk, 0, [[RW, nb], [1, W]]), op=AluOp.add)
                elif k == 2:
                    nc.vector.tensor_tensor(
                        out=vap(s, 0, [[W, nb], [1, W]]),
                        in0=s01[:, 0:nb * W],
                        in1=vap(Gk, 0, [[RW, nb], [1, W]]), op=AluOp.add)
                elif k == 3:
                    nc.vector.tensor_tensor(
                        out=vap(s, 0, [[W, nb], [1, W]]),
                        in0=vap(s, 0, [[W, nb], [1, W]]),
                        in1=vap(Gk, 0, [[RW, nb], [1, W]]), op=AluOp.add)
                Gprev = Gk
            rec = wp.tile([128, MB * 4], F32, tag="rec")
            nc.vector.tensor_scalar_add(
                out=vap(rec, 0, [[4, nb], [1, 4]]),
                in0=vap(s, FD, [[W, nb], [1, 4]]), scalar1=EPS)
            nc.vector.reciprocal(out=rec[:, 0:nb * 4], in_=rec[:, 0:nb * 4])
            nc.vector.tensor_scalar_mul(out=rec[:, 0:nb * 4],
                                        in0=rec[:, 0:nb * 4], scalar1=0.25)
            sc = wp.tile([128, MB * FD], F32, tag="sc")
            nc.vector.tensor_tensor(
                out=vap(sc, 0, [[FD, nb], [FD // 4, 4], [1, FD // 4]]),
                in0=vap(s, 0, [[W, nb], [FD // 4, 4], [1, FD // 4]]),
                in1=vap(rec, 0, [[4, nb], [1, 4], [0, FD // 4]]),
                op=AluOp.mult)
            hs = wp.tile([128, MB * OD], F32, tag="hs")
            nc.vector.tensor_reduce(
                out=vap(hs, 0, [[OD, nb], [1, OD]]),
                in_=vap(sc, 0, [[FD, nb], [1, OD], [OD, 4]]),
                axis=Axis.X, op=AluOp.add)
            nc.vector.tensor_tensor(
                out=vap(hs, 0, [[OD, nb], [1, OD]]),
                in0=vap(hs, 0, [[OD, nb], [1, OD]]),
                in1=vap(bsb, 0, [[0, nb], [1, OD]]), op=AluOp.add)
            if layer == 1:
                nc.scalar.activation(out=hs[:, 0:nb * OD], in_=hs[:, 0:nb * OD],
                                     func=ActFn.Relu)
                hsbB = wp.tile([32, MB * 128], BF16, tag="hsbB")
                ti = 0
                while ti < nb:
                    jn = min(4, nb - ti)
                    psT = pp.tile([32, 4, 128], F32, tag="psT")
                    for j in range(jn):
                        nc.tensor.transpose(
                            out=psT[:, j, :],
                            in_=hs[:, (ti + j) * OD:(ti + j + 1) * OD],
                            identity=ident[:])
                    nc.vector.tensor_copy(
                        out=hsbB[:, ti * 128:(ti + jn) * 128],
                        in_=psT[:, 0:jn, :])
                    ti += jn
                nc.scalar.dma_start(
                    out=h1T_sh[:, (mt - t_lo) * 128:(mt - t_lo + nb) * 128],
                    in_=hsbB[:, 0:nb * 128])
                # ad2 for next layer: h1 @ Wad2, straight into ad2c
                psA = pp.tile([128, MB, 4], F32, tag="psA2")
                for ti in range(nb):
                    nc.tensor.matmul(
                        out=bass.AP(psA[:].tensor, psA[:].offset + ti * 4,
                                    [[list(psA[:].ap[0])[0], 128], [1, 4]]),
                        lhsT=hsbB[:, ti * 128:(ti + 1) * 128],
                        rhs=wadsb[:], start=True, stop=True)
                adt = wp.tile([128, MB, 256], BF16, tag="adt2")
                nc.vector.tensor_copy(out=vap(adt, 0, [[256, nb], [1, 4]]),
                                      in_=psA[:, 0:nb, :])
                nc.scalar.dma_start(
                    out=bass.AP(adc[:, :].tensor, mt * 128 * 256,
                                [[256, 128], [256 * 128, nb], [1, 256]]),
                    in_=adt[:, 0:nb, :])
            else:
                nc.sync.dma_start(
                    out=bass.AP(out2[:, :].tensor, mt * 128 * OD,
                                [[OD, 128], [OD * 128, nb], [1, OD]]),
                    in_=vap(hs, 0, [[OD, nb], [1, OD]]))
            mt += nb


_CACHE = {}


def kernel(**inputs):
    in_maps, meta = host_prep(**inputs)
    key = str(meta["T"])
    _CACHE["k"] = key
    if key not in _CACHE:
        nc = build_nc(meta)
        _CACHE[key] = (nc, make_runner(nc, NCORES))
    nc, run = _CACHE[key]
    results, best = run(in_maps, repeats=1)
    _CACHE["last_time"] = best
    out = np.empty((N, OUT_DIM), np.float32)
    for c in range(NCORES):
        out[c * NSH:(c + 1) * NSH] = results[c]["out2"][:NSH]
    return out

